# revision 1
# baseline (speedup 1.0000x reference)
"""HGNN layer (hypergraph message passing) Trainium2 kernel, 8 NeuronCores.

Sharding: one graph per PAIR of cores (4 graphs x 2 cores). Within a pair
each core owns half the hyperedge/node range. Matmuls keep the big matrix
as the MOVING operand (free dim 512, fp32r / bf16) and the [4096,128]
intermediate as the stationary operand, so every big matrix streams from
HBM once at line rate in the layout the PE needs (the host supplies
transposed shards where the PE requires contraction-major layout).
Intermediates flow in "transposed" [128, 4096] form; PE transposes
convert back to contraction-major tiles between stages. 3 pair-AllReduces
merge the split contractions. Softmax is computed unnormalized; 1/Z is
folded in after the first AllReduce (Z rides along in the collective
buffer).
"""

import numpy as np

B, N, E, D = 4, 4096, 4096, 128
HALF = N // 2
NCORES = 8
PAIRS = [[0, 1], [2, 3], [4, 5], [6, 7]]
BN_EPS = 1e-5
F = 512                 # moving free-dim per matmul
NT = N // 128           # 32 k-tiles over a full 4096 dim
HT = HALF // 128        # 16 k-tiles over a half
RESIDENT_N = 6          # how many of the 16 Ht bf16 tiles stay SBUF-resident

_CACHE = {}


def _build():
    import concourse.bacc as bacc
    import concourse.mybir as mybir
    import concourse.tile as tile
    from concourse.masks import make_identity
    from contextlib import ExitStack

    fp32 = mybir.dt.float32
    fp32r = mybir.dt.float32r
    bf16 = mybir.dt.bfloat16
    Act = mybir.ActivationFunctionType
    Alu = mybir.AluOpType

    nc = bacc.Bacc("TRN2", target_bir_lowering=False, debug=False,
                   num_devices=NCORES)

    # ---- per-core DRAM inputs (shards; see kernel() for host layout) ----
    xT_d = nc.dram_tensor("xT", [D, N], fp32, kind="ExternalInput")
    hcol_d = nc.dram_tensor("hcol", [N, HALF], fp32, kind="ExternalInput")
    htr_d = nc.dram_tensor("htr", [HALF, N], fp32, kind="ExternalInput")
    hrow_d = nc.dram_tensor("hrow", [HALF, N], fp32, kind="ExternalInput")
    dvT_d = nc.dram_tensor("dvT", [N, HALF], fp32, kind="ExternalInput")
    deT_d = nc.dram_tensor("deT", [N, HALF], fp32, kind="ExternalInput")
    w_d = nc.dram_tensor("w", [D, D], fp32, kind="ExternalInput")
    b_d = nc.dram_tensor("b", [D, 1], fp32, kind="ExternalInput")
    th_d = nc.dram_tensor("th", [D, 1], fp32, kind="ExternalInput")
    mask_d = nc.dram_tensor("mask", [1, HALF], fp32, kind="ExternalInput")
    eps_d = nc.dram_tensor("eps", [D, 1], fp32, kind="ExternalInput")
    bng_d = nc.dram_tensor("bng", [D, 1], fp32, kind="ExternalInput")
    bnb_d = nc.dram_tensor("bnb", [D, 1], fp32, kind="ExternalInput")
    bnm_d = nc.dram_tensor("bnm", [D, 1], fp32, kind="ExternalInput")
    bnv_d = nc.dram_tensor("bnv", [D, 1], fp32, kind="ExternalInput")
    y_d = nc.dram_tensor("y", [D, N], fp32, kind="ExternalOutput")

    def r(ap):
        return ap.bitcast(fp32r)

    with tile.TileContext(nc) as tc, ExitStack() as ctx:
        const = ctx.enter_context(tc.tile_pool(name="const", bufs=1))
        resident = ctx.enter_context(tc.tile_pool(name="resident", bufs=1))
        stream = ctx.enter_context(tc.tile_pool(name="stream", bufs=4))
        streamb = ctx.enter_context(tc.tile_pool(name="streamb", bufs=2))
        big = ctx.enter_context(tc.tile_pool(name="big", bufs=1))
        med = ctx.enter_context(tc.tile_pool(name="med", bufs=1))
        small = ctx.enter_context(tc.tile_pool(name="small", bufs=1))
        ps = ctx.enter_context(tc.tile_pool(name="ps", bufs=8, space="PSUM"))
        dram = ctx.enter_context(tc.tile_pool(name="dram", bufs=1, space="DRAM"))

        ident = const.tile([128, 128], fp32)
        make_identity(nc, ident)
        one11 = const.tile([1, 1], fp32)
        nc.vector.memset(one11[:], 1.0)
        ones_row = const.tile([1, 128], fp32)
        nc.vector.memset(ones_row[:], 1.0)
        ones2 = const.tile([2, 1], fp32)
        nc.vector.memset(ones2[:], 1.0)

        def load_param(dt_):
            t = const.tile([D, 1], fp32, tag=dt_.name + "_p")
            nc.sync.dma_start(out=t[:], in_=dt_.ap())
            return t

        w_t = const.tile([D, D], fp32)
        nc.sync.dma_start(out=w_t[:], in_=w_d.ap())
        b_t = load_param(b_d)
        th_t = load_param(th_d)
        eps_t = load_param(eps_d)
        bng_t = load_param(bng_d)
        bnb_t = load_param(bnb_d)
        bnm_t = load_param(bnm_d)
        bnv_t = load_param(bnv_d)
        mask_t = const.tile([1, HALF], fp32)
        nc.sync.dma_start(out=mask_t[:], in_=mask_d.ap())

        # resident Ht bf16 tiles [128e, N] (first RESIDENT_N of HT tiles),
        # loaded once via SWDGE cast-DMA; used by stages 6 and 11.
        ht_res = resident.tile([128, RESIDENT_N * N], bf16)
        for t in range(RESIDENT_N):
            nc.gpsimd.dma_start(
                out=ht_res[:, t * N:(t + 1) * N],
                in_=htr_d.ap()[t * 128:(t + 1) * 128, :])

        def ht_tile(t, tag):
            if t < RESIDENT_N:
                return ht_res[:, t * N:(t + 1) * N]
            tt = streamb.tile([128, N], bf16, tag="htstream", name="htt")
            nc.gpsimd.dma_start(
                out=tt[:], in_=htr_d.ap()[t * 128:(t + 1) * 128, :])
            return tt[:]

        def transpose_cols(src, j, out_ap, scale=None, w128=128):
            """PE-transpose src[:, 128j:128j+128] -> out_ap (optionally
            scaled per-partition by `scale` [128,1]) via psum."""
            pt = ps.tile([128, 128], fp32, tag="ps")
            nc.tensor.transpose(pt[:, 0:w128], src[:, j * 128:j * 128 + w128],
                                ident[:])
            if scale is None:
                nc.vector.tensor_copy(out_ap, pt[:, 0:w128])
            else:
                nc.vector.tensor_scalar_mul(out_ap, pt[:, 0:w128], scale)

        # ------- stage 1: x_wT = (x@W+b).T [D,N]; xthT = (x@th).T [1,N] ----
        xT_t = big.tile([D, N], fp32, tag="bigA")
        nc.sync.dma_start(out=xT_t[:], in_=xT_d.ap())
        x_wT = big.tile([D, N], fp32, tag="bigB")
        xthT = small.tile([1, N], fp32, tag="xthT")
        for blk in range(N // F):
            sl = slice(blk * F, (blk + 1) * F)
            p1 = ps.tile([128, F], fp32, tag="ps")
            nc.tensor.matmul(p1[:], w_t[:], xT_t[:, sl],
                             start=True, stop=True)
            nc.vector.tensor_scalar_add(x_wT[:, sl], p1[:], b_t[:])
            p2 = ps.tile([1, F], fp32, tag="ps")
            nc.tensor.matmul(p2[:], th_t[:], xT_t[:, sl],
                             start=True, stop=True)
            nc.vector.tensor_copy(xthT[:, sl], p2[:])

        # x_w vN tiles [128n, 128d] packed as x_wv[:, j*128:...] and
        # xth vN columns [128n, 1] packed as xthv[:, j]
        x_wv = med.tile([D, N], bf16, tag="x_wv")
        for j in range(NT):
            transpose_cols(x_wT[:], j, x_wv[:, j * 128:(j + 1) * 128])
        xthv = med.tile([128, 2 * NT], bf16, tag="xthv")
        xthv32 = med.tile([128, 1], fp32, tag="xthv32")
        for j in range(NT):
            pt = ps.tile([128, 1], fp32, tag="ps")
            nc.tensor.matmul(pt[:], xthT[:, j * 128:(j + 1) * 128], one11[:],
                             start=True, stop=True)
            # hi/lo bf16 split so the attention scores keep ~fp32 accuracy
            nc.vector.tensor_copy(xthv[:, 2 * j:2 * j + 1], pt[:])
            nc.vector.tensor_tensor(xthv32[:], pt[:], xthv[:, 2 * j:2 * j + 1],
                                    op=Alu.subtract)
            nc.vector.tensor_copy(xthv[:, 2 * j + 1:2 * j + 2], xthv32[:])

        # ------- stage 2: hxT[d, e_half] = (Ht@x_w).T ; sth[1, e_half] ----
        hx_ps = [ps.tile([128, F], fp32, tag="ps", name=f"hx_ps{i}") for i in range(HALF // F)]
        st_ps = [ps.tile([2, F], fp32, tag="ps", name=f"st_ps{i}") for i in range(HALF // F)]
        for j in range(NT):
            hj = stream.tile([128, HALF], bf16, tag="stream")
            nc.gpsimd.dma_start(out=hj[:],
                                in_=hcol_d.ap()[j * 128:(j + 1) * 128, :])
            for blk in range(HALF // F):
                sl = slice(blk * F, (blk + 1) * F)
                nc.tensor.matmul(hx_ps[blk][:],
                                 x_wv[:, j * 128:(j + 1) * 128],
                                 hj[:, sl],
                                 start=(j == 0), stop=(j == NT - 1))
                nc.tensor.matmul(st_ps[blk][:], xthv[:, 2 * j:2 * j + 2],
                                 hj[:, sl],
                                 start=(j == 0), stop=(j == NT - 1))
        hxT = med.tile([D, HALF], fp32, tag="hxT")
        sth = small.tile([1, HALF], fp32, tag="sth")
        for blk in range(HALF // F):
            sl = slice(blk * F, (blk + 1) * F)
            nc.vector.tensor_copy(hxT[:, sl], hx_ps[blk][:])
            s2sb = med.tile([2, F], fp32, tag="s2sb", name=f"s2sb{blk}")
            nc.vector.tensor_copy(s2sb[:], st_ps[blk][0:2, :])
            sp = ps.tile([1, F], fp32, tag="ps", name=f"sp{blk}")
            nc.tensor.matmul(sp[:], ones2[:], s2sb[:], start=True, stop=True)
            nc.vector.tensor_copy(sth[:, sl], sp[:])

        # ------- softmax pieces: attn_u = exp(sth)*mask ; z = sum(attn_u) --
        attn_u = small.tile([1, HALF], fp32, tag="attn_u")
        nc.scalar.activation(attn_u[:], sth[:], Act.Exp)
        nc.vector.tensor_mul(attn_u[:], attn_u[:], mask_t[:])
        z_t = small.tile([1, 1], fp32, tag="z_t")
        nc.vector.reduce_sum(z_t[:], attn_u[:], axis=mybir.AxisListType.X)
        # attn as per-partition columns attnv[:, t]
        attnv = med.tile([128, HT], fp32, tag="attnv")
        for t in range(HT):
            pt = ps.tile([128, 1], fp32, tag="ps")
            nc.tensor.matmul(pt[:], attn_u[:, t * 128:(t + 1) * 128], one11[:],
                             start=True, stop=True)
            nc.vector.tensor_copy(attnv[:, t:t + 1], pt[:])
        # eps-scaled hxT for stage 10
        ehxT = med.tile([D, HALF], fp32, tag="ehxT")
        nc.vector.tensor_scalar_mul(ehxT[:], hxT[:], eps_t[:])

        # ------- h1a vE tiles (bf16): h1a[:, t] = attn*hx tile t ----------
        h1a = med.tile([128, HALF], bf16, tag="h1a")
        for t in range(HT):
            pt = ps.tile([128, 128], fp32, tag="ps")
            nc.tensor.transpose(pt[:], hxT[:, t * 128:(t + 1) * 128], ident[:])
            nc.vector.tensor_scalar_mul(h1a[:, t * 128:(t + 1) * 128], pt[:],
                                        attnv[:, t:t + 1])

        # ------- stage 6: h1bT_part [D, N] = (H @ h1a)_partial.T ----------
        h1b_ps = [ps.tile([128, F], fp32, tag="ps", name=f"h1b_ps{i}") for i in range(N // F)]
        for t in range(HT):
            htt = ht_tile(t, "s6")
            for blk in range(N // F):
                sl = slice(blk * F, (blk + 1) * F)
                nc.tensor.matmul(h1b_ps[blk][:],
                                 h1a[:, t * 128:(t + 1) * 128], htt[:, sl],
                                 start=(t == 0), stop=(t == HT - 1))
        # evict with z riding in col N (cols N..N+7 zeroed)
        cc1_sb = big.tile([D, N + 8], fp32, tag="bigA")
        for blk in range(N // F):
            sl = slice(blk * F, (blk + 1) * F)
            nc.vector.tensor_copy(cc1_sb[:, sl], h1b_ps[blk][:])
        nc.vector.memset(cc1_sb[:, N:], 0.0)
        nc.vector.tensor_copy(cc1_sb[0:1, N:N + 1], z_t[:])
        cc1_in = dram.tile([D, N + 8], fp32, tag="cc1i")
        cc1_out = dram.tile([D, N + 8], fp32, tag="cc1o")
        nc.sync.dma_start(out=cc1_in[:], in_=cc1_sb[:])
        nc.gpsimd.collective_compute(
            "AllReduce", Alu.add, replica_groups=PAIRS,
            ins=[cc1_in.opt()], outs=[cc1_out.opt()])
        h1b_full = big.tile([D, N + 8], fp32, tag="bigB")
        nc.sync.dma_start(out=h1b_full[:], in_=cc1_out[:])

        # 1/z broadcast to [128, 1]
        rz = small.tile([1, 1], fp32, tag="rz")
        nc.vector.reciprocal(rz[:], h1b_full[0:1, N:N + 1])
        rz_ps = ps.tile([128, 1], fp32, tag="ps")
        nc.tensor.matmul(rz_ps[:], ones_row[:], rz[:], start=True, stop=True)
        rz_bc = small.tile([128, 1], fp32, tag="rz_bc")
        nc.vector.tensor_copy(rz_bc[:], rz_ps[:])

        # h1b vN tiles scaled by 1/z
        h1bv = med.tile([D, N], bf16, tag="x_wv")
        for j in range(NT):
            transpose_cols(h1b_full[:], j, h1bv[:, j * 128:(j + 1) * 128],
                           scale=rz_bc[:])

        # ------- stage 7: h1cT [D, HALF] = (Dv @ h1b).T rows-half ---------
        h1c_ps = [ps.tile([128, F], fp32, tag="ps", name=f"h1c_ps{i}") for i in range(HALF // F)]
        for j in range(NT):
            dj = stream.tile([128, HALF], bf16, tag="stream")
            nc.gpsimd.dma_start(out=dj[:],
                                in_=dvT_d.ap()[j * 128:(j + 1) * 128, :])
            for blk in range(HALF // F):
                sl = slice(blk * F, (blk + 1) * F)
                nc.tensor.matmul(h1c_ps[blk][:],
                                 h1bv[:, j * 128:(j + 1) * 128],
                                 dj[:, sl],
                                 start=(j == 0), stop=(j == NT - 1))
        h1cT = med.tile([D, HALF], fp32, tag="hxT2")
        for blk in range(HALF // F):
            sl = slice(blk * F, (blk + 1) * F)
            nc.vector.tensor_copy(h1cT[:, sl], h1c_ps[blk][:])

        # h1c vN tiles
        h1cv = med.tile([D, HALF], bf16, tag="h1cv")
        for t in range(HT):
            transpose_cols(h1cT[:], t, h1cv[:, t * 128:(t + 1) * 128])

        # ------- stage 8: h1dT_part [D, N] = (Ht @ h1c)_partial.T ---------
        h1d_ps = [ps.tile([128, F], fp32, tag="ps", name=f"h1d_ps{i}") for i in range(N // F)]
        for t in range(HT):
            rj1 = stream.tile([128, HALF], bf16, tag="stream", name="rj1")
            nc.gpsimd.dma_start(out=rj1[:],
                                in_=hrow_d.ap()[t * 128:(t + 1) * 128, 0:HALF])
            rj2 = stream.tile([128, HALF], bf16, tag="stream", name="rj2")
            nc.gpsimd.dma_start(out=rj2[:],
                                in_=hrow_d.ap()[t * 128:(t + 1) * 128, HALF:N])
            for blk in range(N // F):
                sl = slice((blk % 4) * F, (blk % 4 + 1) * F)
                rj = rj1 if blk < 4 else rj2
                nc.tensor.matmul(h1d_ps[blk][:],
                                 h1cv[:, t * 128:(t + 1) * 128],
                                 rj[:, sl],
                                 start=(t == 0), stop=(t == HT - 1))
        cc2_sb = big.tile([D, N], fp32, tag="bigA")
        for blk in range(N // F):
            sl = slice(blk * F, (blk + 1) * F)
            nc.vector.tensor_copy(cc2_sb[:, sl], h1d_ps[blk][:])
        cc2_in = dram.tile([D, N], fp32, tag="cc2i")
        cc2_out = dram.tile([D, N], fp32, tag="cc2o")
        nc.sync.dma_start(out=cc2_in[:], in_=cc2_sb[:])
        nc.gpsimd.collective_compute(
            "AllReduce", Alu.add, replica_groups=PAIRS,
            ins=[cc2_in.opt()], outs=[cc2_out.opt()])
        h1d_full = big.tile([D, N], fp32, tag="bigB")
        nc.sync.dma_start(out=h1d_full[:], in_=cc2_out[:])

        # h1d vE tiles
        h1dv = med.tile([D, N], bf16, tag="x_wv")
        for j in range(NT):
            transpose_cols(h1d_full[:], j, h1dv[:, j * 128:(j + 1) * 128])

        # ------- stage 9: h1eT [D, HALF] = (De @ h1d).T rows-half ---------
        h1e_ps = [ps.tile([128, F], fp32, tag="ps", name=f"h1e_ps{i}") for i in range(HALF // F)]
        for j in range(NT):
            ej = stream.tile([128, HALF], bf16, tag="stream")
            nc.gpsimd.dma_start(out=ej[:],
                                in_=deT_d.ap()[j * 128:(j + 1) * 128, :])
            for blk in range(HALF // F):
                sl = slice(blk * F, (blk + 1) * F)
                nc.tensor.matmul(h1e_ps[blk][:],
                                 h1dv[:, j * 128:(j + 1) * 128],
                                 ej[:, sl],
                                 start=(j == 0), stop=(j == NT - 1))
        # ------- stage 10: hT = h1eT + eps*hxT ; hv bf16 tiles ------------
        hT = med.tile([D, HALF], fp32, tag="hxT2b")
        for blk in range(HALF // F):
            sl = slice(blk * F, (blk + 1) * F)
            nc.vector.tensor_tensor(hT[:, sl], h1e_ps[blk][:], ehxT[:, sl],
                                    op=Alu.add)
        hv = med.tile([128, HALF], bf16, tag="h1a")
        for t in range(HT):
            pt = ps.tile([128, 128], fp32, tag="ps")
            nc.tensor.transpose(pt[:], hT[:, t * 128:(t + 1) * 128], ident[:])
            nc.vector.tensor_copy(hv[:, t * 128:(t + 1) * 128], pt[:])

        # ------- stage 11: outT_part [D, N] = (H @ h)_partial.T -----------
        out_ps = [ps.tile([128, F], fp32, tag="ps", name=f"out_ps{i}") for i in range(N // F)]
        for t in range(HT):
            htt = ht_tile(t, "s11")
            for blk in range(N // F):
                sl = slice(blk * F, (blk + 1) * F)
                nc.tensor.matmul(out_ps[blk][:],
                                 hv[:, t * 128:(t + 1) * 128], htt[:, sl],
                                 start=(t == 0), stop=(t == HT - 1))
        cc3_sb = big.tile([D, N], fp32, tag="bigA")
        for blk in range(N // F):
            sl = slice(blk * F, (blk + 1) * F)
            nc.vector.tensor_copy(cc3_sb[:, sl], out_ps[blk][:])
        cc3_in = dram.tile([D, N], fp32, tag="cc3i")
        cc3_out = dram.tile([D, N], fp32, tag="cc3o")
        nc.sync.dma_start(out=cc3_in[:], in_=cc3_sb[:])
        nc.gpsimd.collective_compute(
            "AllReduce", Alu.add, replica_groups=PAIRS,
            ins=[cc3_in.opt()], outs=[cc3_out.opt()])
        outT = big.tile([D, N], fp32, tag="bigB")
        nc.sync.dma_start(out=outT[:], in_=cc3_out[:])

        # ------- stage 12: epilogue: bn(leaky_relu(outT)) -----------------
        # bn scale s = gamma * rsqrt(var + eps_bn); shift t = beta - mean*s
        s_bn = small.tile([D, 1], fp32, tag="s_bn")
        nc.vector.tensor_scalar_add(s_bn[:], bnv_t[:], BN_EPS)
        nc.scalar.activation(s_bn[:], s_bn[:], Act.Sqrt)
        nc.vector.reciprocal(s_bn[:], s_bn[:])
        nc.vector.tensor_mul(s_bn[:], s_bn[:], bng_t[:])
        t_bn = small.tile([D, 1], fp32, tag="t_bn")
        nc.vector.tensor_mul(t_bn[:], bnm_t[:], s_bn[:])
        nc.vector.tensor_tensor(t_bn[:], bnb_t[:], t_bn[:],
                                op=Alu.subtract)
        nc.scalar.activation(outT[:], outT[:], Act.Lrelu, alpha=0.01)
        nc.vector.tensor_scalar(outT[:], outT[:], s_bn[:], t_bn[:],
                                op0=Alu.mult, op1=Alu.add)
        nc.sync.dma_start(out=y_d.ap(), in_=outT[:])

    nc.finalize()
    return nc


def _get_nc():
    if "nc" not in _CACHE:
        _CACHE["nc"] = _build()
    return _CACHE["nc"]


def _shard(inputs):
    H = np.asarray(inputs["incident_mat"], dtype=np.float32)
    Dv = np.asarray(inputs["degree_v"], dtype=np.float32)
    De = np.asarray(inputs["degree_e"], dtype=np.float32)
    x = np.asarray(inputs["x"], dtype=np.float32)
    em = np.asarray(inputs["e_masks"])
    w = np.ascontiguousarray(np.asarray(inputs["mlp_W"], dtype=np.float32))
    b = np.ascontiguousarray(
        np.asarray(inputs["mlp_b"], dtype=np.float32).reshape(D, 1))
    th = np.ascontiguousarray(
        np.asarray(inputs["theta_att"], dtype=np.float32).reshape(D, 1))
    eps = np.full((D, 1), float(np.asarray(inputs["eps"]).reshape(-1)[0]),
                  dtype=np.float32)

    def col(v):
        return np.ascontiguousarray(
            np.asarray(v, dtype=np.float32).reshape(D, 1))

    bng, bnb = col(inputs["bn_gamma"]), col(inputs["bn_beta"])
    bnm, bnv = col(inputs["bn_mean"]), col(inputs["bn_var"])

    in_maps = []
    for core in range(NCORES):
        g, c = core // 2, core % 2
        lo, hi = c * HALF, (c + 1) * HALF
        Hg = H[g]
        htr = np.ascontiguousarray(Hg.T[lo:hi, :])
        in_maps.append({
            "xT": np.ascontiguousarray(x[g].T),
            "hcol": np.ascontiguousarray(Hg[:, lo:hi]),
            "htr": htr,
            "hrow": np.ascontiguousarray(Hg[lo:hi, :]),
            "dvT": np.ascontiguousarray(Dv[g][lo:hi, :].T),
            "deT": np.ascontiguousarray(De[g][lo:hi, :].T),
            "w": w, "b": b, "th": th,
            "mask": np.ascontiguousarray(
                em[g, lo:hi].astype(np.float32).reshape(1, HALF)),
            "eps": eps,
            "bng": bng, "bnb": bnb, "bnm": bnm, "bnv": bnv,
        })
    return in_maps


def kernel(**inputs):
    from concourse.bass_utils import run_bass_kernel_spmd

    nc = _get_nc()
    in_maps = _shard(inputs)
    res = run_bass_kernel_spmd(nc, in_maps, list(range(NCORES)))
    out = np.empty((B, N, D), dtype=np.float32)
    for g in range(B):
        ya = res.results[2 * g]["y"]
        yb = res.results[2 * g + 1]["y"]
        out[g, :HALF, :] = ya[:, :HALF].T
        out[g, HALF:, :] = yb[:, HALF:].T
    return out



# revision 8
# speedup vs baseline: 1.2616x; 1.2616x over previous
"""HGNN layer (hypergraph message passing) Trainium2 kernel, 8 NeuronCores.

Sharding: one graph per PAIR of cores; within a pair each core owns half the
hyperedge (Ec) / node (Nc) range. Host pre-casts the big matrices: the 0/1
incident matrix H ships as fp8e4 (exact) in the three layouts the PE needs
(hcol [n,Ec], htr [Ec,n], hrow [Nc,e]); Dv/De/x ship as bf16. All device DMA
is plain HWDGE (no cast-DMA). Dataflow computes hxx = H^T x first, then
applies the MLP weight in E-space (hx = hxx W + b (x) dege with the bias as a
rank-1 PE accumulate against a host-computed edge-degree row); attention
scores ride a hi/lo bf16 split of hxx/theta for fp32-grade logits. Softmax is
unnormalized; z rides the first pair-AllReduce and 1/z is folded into the
transposes after it. htr stays SBUF-resident for its two uses. 3 fp32
pair-AllReduces merge split contractions.
"""

import numpy as np

B, N, E, D = 4, 4096, 4096, 128
HALF = N // 2
NCORES = 8
PAIRS = [[0, 1], [2, 3], [4, 5], [6, 7]]
BN_EPS = 1e-5
F = 512                 # moving free-dim per matmul
NT = N // 128           # 32 k-tiles over a full 4096 dim
HT = HALF // 128        # 16 k-tiles over a half

_CACHE = {}


def _build():
    import concourse.bacc as bacc
    import concourse.mybir as mybir
    import concourse.tile as tile
    from concourse.masks import make_identity
    from contextlib import ExitStack

    fp32 = mybir.dt.float32
    bf16 = mybir.dt.bfloat16
    fp8 = mybir.dt.float8e4
    Act = mybir.ActivationFunctionType
    Alu = mybir.AluOpType

    nc = bacc.Bacc("TRN2", target_bir_lowering=False, debug=False,
                   num_devices=NCORES)

    # ---- per-core DRAM inputs (see _shard for host layout) ----
    x_d = nc.dram_tensor("x", [N, D], bf16, kind="ExternalInput")
    hcol_d = nc.dram_tensor("hcol", [N, HALF], fp8, kind="ExternalInput")
    htr_d = nc.dram_tensor("htr", [HALF, N], fp8, kind="ExternalInput")
    hrow_d = nc.dram_tensor("hrow", [HALF, N], fp8, kind="ExternalInput")
    dvT_d = nc.dram_tensor("dvT", [N, HALF], bf16, kind="ExternalInput")
    deT_d = nc.dram_tensor("deT", [N, HALF], bf16, kind="ExternalInput")
    dege_d = nc.dram_tensor("dege", [1, HALF], fp32, kind="ExternalInput")
    whi_d = nc.dram_tensor("whi", [D, D], bf16, kind="ExternalInput")
    th2_d = nc.dram_tensor("th2", [D, 2], bf16, kind="ExternalInput")
    brow_d = nc.dram_tensor("brow", [1, D], fp32, kind="ExternalInput")
    mask_d = nc.dram_tensor("mask", [1, HALF], fp32, kind="ExternalInput")
    eps_d = nc.dram_tensor("eps", [D, 1], fp32, kind="ExternalInput")
    bng_d = nc.dram_tensor("bng", [D, 1], fp32, kind="ExternalInput")
    bnb_d = nc.dram_tensor("bnb", [D, 1], fp32, kind="ExternalInput")
    bnm_d = nc.dram_tensor("bnm", [D, 1], fp32, kind="ExternalInput")
    bnv_d = nc.dram_tensor("bnv", [D, 1], fp32, kind="ExternalInput")
    y_d = nc.dram_tensor("y", [D, N], fp32, kind="ExternalOutput")

    with tile.TileContext(nc) as tc, ExitStack() as ctx:
        const = ctx.enter_context(tc.tile_pool(name="const", bufs=1))
        resident = ctx.enter_context(tc.tile_pool(name="resident", bufs=1))
        stream = ctx.enter_context(tc.tile_pool(name="stream", bufs=3))
        med = ctx.enter_context(tc.tile_pool(name="med", bufs=1))
        small = ctx.enter_context(tc.tile_pool(name="small", bufs=1))
        ps = ctx.enter_context(tc.tile_pool(name="ps", bufs=8, space="PSUM"))
        dram = ctx.enter_context(tc.tile_pool(name="dram", bufs=1, space="DRAM"))

        ident = const.tile([128, 128], fp32)
        make_identity(nc, ident)
        one11 = const.tile([1, 1], fp32)
        nc.vector.memset(one11[:], 1.0)
        ones_row = const.tile([1, 128], fp32)
        nc.vector.memset(ones_row[:], 1.0)
        ones2 = const.tile([2, 1], fp32)
        nc.vector.memset(ones2[:], 1.0)

        def load_param(dt_):
            t = const.tile([D, 1], fp32, tag=dt_.name + "_p")
            nc.sync.dma_start(out=t[:], in_=dt_.ap())
            return t

        whi_t = const.tile([D, D], bf16)
        nc.sync.dma_start(out=whi_t[:], in_=whi_d.ap())
        th2_t = const.tile([D, 2], bf16)
        nc.sync.dma_start(out=th2_t[:], in_=th2_d.ap())
        brow_t = const.tile([1, D], fp32)
        nc.sync.dma_start(out=brow_t[:], in_=brow_d.ap())
        dege_t = const.tile([1, HALF], fp32)
        nc.sync.dma_start(out=dege_t[:], in_=dege_d.ap())
        eps_t = load_param(eps_d)
        bng_t = load_param(bng_d)
        bnb_t = load_param(bnb_d)
        bnm_t = load_param(bnm_d)
        bnv_t = load_param(bnv_d)
        mask_t = const.tile([1, HALF], fp32)
        nc.sync.dma_start(out=mask_t[:], in_=mask_d.ap())

        # x as n-partition tiles [128, NT*128] bf16 (tile j at cols j*128:..)
        xv = const.tile([128, NT * D], bf16)
        for j in range(NT):
            nc.sync.dma_start(out=xv[:, j * D:(j + 1) * D],
                              in_=x_d.ap()[j * 128:(j + 1) * 128, :])

        # htr resident fp8 [128, HT*N]
        htr_res = resident.tile([128, HT * N], fp8)
        for t in range(HT):
            nc.sync.dma_start(out=htr_res[:, t * N:(t + 1) * N],
                              in_=htr_d.ap()[t * 128:(t + 1) * 128, :])

        # ---- S2: hxxT [D, HALF] = (H[:,Ec]^T x)^T ------------------------
        hxx_ps = [ps.tile([128, F], fp32, tag="ps", name=f"hxx{i}")
                  for i in range(HALF // F)]
        for j in range(NT):
            hj = stream.tile([128, HALF], fp8, tag="stream8", name="hj")
            nc.sync.dma_start(out=hj[:],
                              in_=hcol_d.ap()[j * 128:(j + 1) * 128, :])
            for blk in range(HALF // F):
                sl = slice(blk * F, (blk + 1) * F)
                nc.tensor.matmul(hxx_ps[blk][:],
                                 xv[:, j * D:(j + 1) * D], hj[:, sl],
                                 start=(j == 0), stop=(j == NT - 1))
        hxxT = med.tile([D, HALF], fp32, tag="hxxT")
        hxx_hi = med.tile([D, HALF], bf16, tag="hxx_hi")
        hxx_lo = med.tile([D, HALF], bf16, tag="hxx_lo")
        tmp32 = med.tile([D, HALF], fp32, tag="ehxT")
        for blk in range(HALF // F):
            sl = slice(blk * F, (blk + 1) * F)
            nc.vector.tensor_copy(hxxT[:, sl], hxx_ps[blk][:])
        nc.vector.tensor_copy(hxx_hi[:], hxxT[:])
        nc.vector.tensor_copy(tmp32[:], hxx_hi[:])
        nc.vector.tensor_tensor(tmp32[:], hxxT[:], tmp32[:], op=Alu.subtract)
        nc.vector.tensor_copy(hxx_lo[:], tmp32[:])

        # ---- S3: hxT = W^T hxx + b (x) dege ; st = th^T hxx --------------
        hxT = med.tile([D, HALF], fp32, tag="hxT")
        st_sb = small.tile([1, HALF], fp32, tag="st_sb")
        for blk in range(HALF // F):
            sl = slice(blk * F, (blk + 1) * F)
            hx2 = ps.tile([128, F], fp32, tag="ps", name=f"hx2_{blk}")
            nc.tensor.matmul(hx2[:], whi_t[:], hxx_hi[:, sl],
                             start=True, stop=False)
            nc.tensor.matmul(hx2[:], whi_t[:], hxx_lo[:, sl],
                             start=False, stop=False)
            nc.tensor.matmul(hx2[:], brow_t[:], dege_t[:, sl],
                             start=False, stop=True)
            nc.vector.tensor_copy(hxT[:, sl], hx2[:])
            st2 = ps.tile([2, F], fp32, tag="ps", name=f"st2_{blk}")
            nc.tensor.matmul(st2[:], th2_t[:], hxx_hi[:, sl],
                             start=True, stop=False)
            nc.tensor.matmul(st2[:], th2_t[:], hxx_lo[:, sl],
                             start=False, stop=True)
            s2sb = med.tile([2, F], fp32, tag="s2sb", name=f"s2sb{blk}")
            nc.vector.tensor_copy(s2sb[:], st2[0:2, :])
            sp = ps.tile([1, F], fp32, tag="ps", name=f"sp{blk}")
            nc.tensor.matmul(sp[:], ones2[:], s2sb[:],
                             start=True, stop=True)
            nc.vector.tensor_copy(st_sb[:, sl], sp[:])

        # ---- S4: softmax pieces (in-place on st_sb) ----------------------
        attn_u = st_sb
        nc.scalar.activation(attn_u[:], st_sb[:], Act.Exp)
        nc.vector.tensor_mul(attn_u[:], attn_u[:], mask_t[:])
        z_t = small.tile([1, 1], fp32, tag="z_t")
        nc.vector.reduce_sum(z_t[:], attn_u[:], axis=mybir.AxisListType.X)
        attnv = med.tile([128, HT], fp32, tag="attnv")
        for t in range(HT):
            pt = ps.tile([128, 1], fp32, tag="ps")
            nc.tensor.matmul(pt[:], attn_u[:, t * 128:(t + 1) * 128], one11[:],
                             start=True, stop=True)
            nc.vector.tensor_copy(attnv[:, t:t + 1], pt[:])
        # eps-scaled hxT for S10
        ehxT = med.tile([D, HALF], fp32, tag="ehxT")
        nc.vector.tensor_scalar_mul(ehxT[:], hxT[:], eps_t[:])

        # ---- S5: h1av [128, HT*D] bf16 = attn * hx (e-part tiles) --------
        h1av = med.tile([128, HALF], bf16, tag="h1av")
        for t in range(HT):
            pt = ps.tile([128, 128], fp32, tag="ps")
            nc.tensor.transpose(pt[:], hxT[:, t * 128:(t + 1) * 128], ident[:])
            nc.vector.tensor_scalar_mul(h1av[:, t * 128:(t + 1) * 128], pt[:],
                                        attnv[:, t:t + 1])

        # ---- S6: h1bT_part [D, N] = (H h1a)^T partial --------------------
        h1b_ps = [ps.tile([128, F], fp32, tag="ps", name=f"h1b{i}")
                  for i in range(N // F)]
        for t in range(HT):
            for blk in range(N // F):
                sl = slice(blk * F, (blk + 1) * F)
                nc.tensor.matmul(h1b_ps[blk][:],
                                 h1av[:, t * 128:(t + 1) * 128],
                                 htr_res[:, t * N + blk * F:t * N + blk * F + F],
                                 start=(t == 0), stop=(t == HT - 1))
        cc1_sb = med.tile([D, N + 8], fp32, tag="ccsb")
        for blk in range(N // F):
            sl = slice(blk * F, (blk + 1) * F)
            nc.vector.tensor_copy(cc1_sb[:, sl], h1b_ps[blk][:])
        nc.vector.memset(cc1_sb[:, N:], 0.0)
        nc.vector.tensor_copy(cc1_sb[0:1, N:N + 1], z_t[:])
        cc1_in = dram.tile([D, N + 8], fp32, tag="cc1i")
        cc1_out = dram.tile([D, N + 8], fp32, tag="cc1o")
        nc.sync.dma_start(out=cc1_in[:], in_=cc1_sb[:])
        nc.gpsimd.collective_compute(
            "AllReduce", Alu.add, replica_groups=PAIRS,
            ins=[cc1_in.opt()], outs=[cc1_out.opt()])
        h1b_full = med.tile([D, N + 8], fp32, tag="h1b_full")
        nc.sync.dma_start(out=h1b_full[:], in_=cc1_out[:])

        # 1/z broadcast to [128, 1]
        rz = small.tile([1, 1], fp32, tag="rz")
        nc.vector.reciprocal(rz[:], h1b_full[0:1, N:N + 1])
        rz_ps = ps.tile([128, 1], fp32, tag="ps")
        nc.tensor.matmul(rz_ps[:], ones_row[:], rz[:], start=True, stop=True)
        rz_bc = small.tile([128, 1], fp32, tag="rz_bc")
        nc.vector.tensor_copy(rz_bc[:], rz_ps[:])

        # h1bv [128, NT*D] bf16 scaled by 1/z
        h1bv = med.tile([128, N], bf16, tag="h1bv")
        for j in range(NT):
            pt = ps.tile([128, 128], fp32, tag="ps")
            nc.tensor.transpose(pt[:], h1b_full[:, j * 128:(j + 1) * 128],
                                ident[:])
            nc.vector.tensor_scalar_mul(h1bv[:, j * 128:(j + 1) * 128], pt[:],
                                        rz_bc[:])

        # ---- S7: h1cT [D, Nc] = (Dv[Nc,:] h1b)^T -------------------------
        h1c_ps = [ps.tile([128, F], fp32, tag="ps", name=f"h1c{i}")
                  for i in range(HALF // F)]
        for j in range(NT):
            dj = stream.tile([128, HALF], bf16, tag="streamw", name="dj")
            nc.sync.dma_start(out=dj[:],
                              in_=dvT_d.ap()[j * 128:(j + 1) * 128, :])
            for blk in range(HALF // F):
                sl = slice(blk * F, (blk + 1) * F)
                nc.tensor.matmul(h1c_ps[blk][:],
                                 h1bv[:, j * 128:(j + 1) * 128], dj[:, sl],
                                 start=(j == 0), stop=(j == NT - 1))
        h1cT = med.tile([D, HALF], fp32, tag="hxxT")
        for blk in range(HALF // F):
            sl = slice(blk * F, (blk + 1) * F)
            nc.vector.tensor_copy(h1cT[:, sl], h1c_ps[blk][:])
        h1cv = med.tile([128, HALF], bf16, tag="h1cv")
        for t in range(HT):
            pt = ps.tile([128, 128], fp32, tag="ps")
            nc.tensor.transpose(pt[:], h1cT[:, t * 128:(t + 1) * 128], ident[:])
            nc.vector.tensor_copy(h1cv[:, t * 128:(t + 1) * 128], pt[:])

        # ---- S8: h1dT_part [D, N] = (H[Nc,:]^T h1c)^T partial ------------
        h1d_ps = [ps.tile([128, F], fp32, tag="ps", name=f"h1d{i}")
                  for i in range(N // F)]
        for t in range(HT):
            rj = stream.tile([128, N], fp8, tag="stream8", name="rj")
            nc.sync.dma_start(out=rj[:],
                              in_=hrow_d.ap()[t * 128:(t + 1) * 128, :])
            for blk in range(N // F):
                sl = slice(blk * F, (blk + 1) * F)
                nc.tensor.matmul(h1d_ps[blk][:],
                                 h1cv[:, t * 128:(t + 1) * 128], rj[:, sl],
                                 start=(t == 0), stop=(t == HT - 1))
        cc2_sb = med.tile([D, N + 8], fp32, tag="ccsb")
        for blk in range(N // F):
            sl = slice(blk * F, (blk + 1) * F)
            nc.vector.tensor_copy(cc2_sb[:, sl], h1d_ps[blk][:])
        cc2_in = dram.tile([D, N], fp32, tag="cc2i")
        cc2_out = dram.tile([D, N], fp32, tag="cc2o")
        nc.sync.dma_start(out=cc2_in[:], in_=cc2_sb[:, 0:N])
        nc.gpsimd.collective_compute(
            "AllReduce", Alu.add, replica_groups=PAIRS,
            ins=[cc2_in.opt()], outs=[cc2_out.opt()])
        h1d_full = med.tile([D, N + 8], fp32, tag="h1b_full")
        nc.sync.dma_start(out=h1d_full[:, 0:N], in_=cc2_out[:])
        h1dv = med.tile([128, N], bf16, tag="h1bv")
        for j in range(NT):
            pt = ps.tile([128, 128], fp32, tag="ps")
            nc.tensor.transpose(pt[:], h1d_full[:, j * 128:(j + 1) * 128],
                                ident[:])
            nc.vector.tensor_copy(h1dv[:, j * 128:(j + 1) * 128], pt[:])

        # ---- S9+S10: hT [D, Ec] = (De[Ec,:] h1d)^T + eps*hx --------------
        h1e_ps = [ps.tile([128, F], fp32, tag="ps", name=f"h1e{i}")
                  for i in range(HALF // F)]
        for j in range(NT):
            ej = stream.tile([128, HALF], bf16, tag="streamw", name="ej")
            nc.sync.dma_start(out=ej[:],
                              in_=deT_d.ap()[j * 128:(j + 1) * 128, :])
            for blk in range(HALF // F):
                sl = slice(blk * F, (blk + 1) * F)
                nc.tensor.matmul(h1e_ps[blk][:],
                                 h1dv[:, j * 128:(j + 1) * 128], ej[:, sl],
                                 start=(j == 0), stop=(j == NT - 1))
        hT = med.tile([D, HALF], fp32, tag="hxT")
        for blk in range(HALF // F):
            sl = slice(blk * F, (blk + 1) * F)
            nc.vector.tensor_tensor(hT[:, sl], h1e_ps[blk][:], ehxT[:, sl],
                                    op=Alu.add)
        hv = med.tile([128, HALF], bf16, tag="h1av")
        for t in range(HT):
            pt = ps.tile([128, 128], fp32, tag="ps")
            nc.tensor.transpose(pt[:], hT[:, t * 128:(t + 1) * 128], ident[:])
            nc.vector.tensor_copy(hv[:, t * 128:(t + 1) * 128], pt[:])

        # ---- S11: outT_part [D, N] = (H h)^T partial ---------------------
        out_ps = [ps.tile([128, F], fp32, tag="ps", name=f"out{i}")
                  for i in range(N // F)]
        for t in range(HT):
            for blk in range(N // F):
                nc.tensor.matmul(out_ps[blk][:],
                                 hv[:, t * 128:(t + 1) * 128],
                                 htr_res[:, t * N + blk * F:t * N + blk * F + F],
                                 start=(t == 0), stop=(t == HT - 1))
        cc3_sb = med.tile([D, N + 8], fp32, tag="ccsb")
        for blk in range(N // F):
            sl = slice(blk * F, (blk + 1) * F)
            nc.vector.tensor_copy(cc3_sb[:, sl], out_ps[blk][:])
        cc3_in = dram.tile([D, N], fp32, tag="cc3i")
        cc3_out = dram.tile([D, N], fp32, tag="cc3o")
        nc.sync.dma_start(out=cc3_in[:], in_=cc3_sb[:, 0:N])
        nc.gpsimd.collective_compute(
            "AllReduce", Alu.add, replica_groups=PAIRS,
            ins=[cc3_in.opt()], outs=[cc3_out.opt()])
        outT = med.tile([D, N + 8], fp32, tag="h1b_full")
        nc.sync.dma_start(out=outT[:, 0:N], in_=cc3_out[:])

        # ---- S12: epilogue bn(leaky_relu(outT)) --------------------------
        s_bn = small.tile([D, 1], fp32, tag="s_bn")
        nc.vector.tensor_scalar_add(s_bn[:], bnv_t[:], BN_EPS)
        nc.scalar.activation(s_bn[:], s_bn[:], Act.Sqrt)
        nc.vector.reciprocal(s_bn[:], s_bn[:])
        nc.vector.tensor_mul(s_bn[:], s_bn[:], bng_t[:])
        t_bn = small.tile([D, 1], fp32, tag="t_bn")
        nc.vector.tensor_mul(t_bn[:], bnm_t[:], s_bn[:])
        nc.vector.tensor_tensor(t_bn[:], bnb_t[:], t_bn[:], op=Alu.subtract)
        nc.scalar.activation(outT[:, 0:N], outT[:, 0:N], Act.Lrelu, alpha=0.01)
        nc.vector.tensor_scalar(outT[:, 0:N], outT[:, 0:N], s_bn[:], t_bn[:],
                                op0=Alu.mult, op1=Alu.add)
        nc.sync.dma_start(out=y_d.ap(), in_=outT[:, 0:N])

    nc.finalize()
    return nc


def _get_nc():
    if "nc" not in _CACHE:
        _CACHE["nc"] = _build()
    return _CACHE["nc"]


def _shard(inputs):
    import ml_dtypes
    bf16 = ml_dtypes.bfloat16
    fp8 = ml_dtypes.float8_e4m3

    H = np.asarray(inputs["incident_mat"], dtype=np.float32)
    Dv = np.asarray(inputs["degree_v"], dtype=np.float32)
    De = np.asarray(inputs["degree_e"], dtype=np.float32)
    x = np.asarray(inputs["x"], dtype=np.float32)
    em = np.asarray(inputs["e_masks"])
    w = np.asarray(inputs["mlp_W"], dtype=np.float32)
    b = np.asarray(inputs["mlp_b"], dtype=np.float32)
    th = np.asarray(inputs["theta_att"], dtype=np.float32).reshape(D)
    eps = np.full((D, 1), float(np.asarray(inputs["eps"]).reshape(-1)[0]),
                  dtype=np.float32)

    def col(v):
        return np.ascontiguousarray(
            np.asarray(v, dtype=np.float32).reshape(D, 1))

    bng, bnb = col(inputs["bn_gamma"]), col(inputs["bn_beta"])
    bnm, bnv = col(inputs["bn_mean"]), col(inputs["bn_var"])

    whi = w.astype(bf16)
    th_hi = th.astype(bf16)
    th_lo = (th - th_hi.astype(np.float32)).astype(bf16)
    th2 = np.ascontiguousarray(np.stack(
        [th_hi.astype(np.float32), th_lo.astype(np.float32)], axis=1)
    ).astype(bf16)
    brow = np.ascontiguousarray(b.reshape(1, D))

    in_maps = []
    for g in range(B):
        Hg8 = H[g].astype(fp8)
        HgT8 = np.ascontiguousarray(H[g].T).astype(fp8)
        dege_full = H[g].sum(axis=0, dtype=np.float32)
        xg = np.ascontiguousarray(x[g]).astype(bf16)
        DvT = np.ascontiguousarray(Dv[g].T).astype(bf16)
        DeT = np.ascontiguousarray(De[g].T).astype(bf16)
        for c in range(2):
            lo, hi = c * HALF, (c + 1) * HALF
            in_maps.append({
                "x": xg,
                "hcol": np.ascontiguousarray(Hg8[:, lo:hi]),
                "htr": np.ascontiguousarray(HgT8[lo:hi, :]),
                "hrow": np.ascontiguousarray(Hg8[lo:hi, :]),
                "dvT": np.ascontiguousarray(DvT[:, lo:hi]),
                "deT": np.ascontiguousarray(DeT[:, lo:hi]),
                "dege": np.ascontiguousarray(
                    dege_full[lo:hi].reshape(1, HALF)),
                "whi": whi, "th2": th2, "brow": brow,
                "mask": np.ascontiguousarray(
                    em[g, lo:hi].astype(np.float32).reshape(1, HALF)),
                "eps": eps,
                "bng": bng, "bnb": bnb, "bnm": bnm, "bnv": bnv,
            })
    return in_maps


def kernel(**inputs):
    from concourse.bass_utils import run_bass_kernel_spmd

    nc = _get_nc()
    in_maps = _shard(inputs)
    res = run_bass_kernel_spmd(nc, in_maps, list(range(NCORES)))
    out = np.empty((B, N, D), dtype=np.float32)
    for g in range(B):
        ya = res.results[2 * g]["y"]
        yb = res.results[2 * g + 1]["y"]
        out[g, :HALF, :] = ya[:, :HALF].T
        out[g, HALF:, :] = yb[:, HALF:].T
    return out


# revision 9
# speedup vs baseline: 1.6489x; 1.3071x over previous
"""HGNN layer (hypergraph message passing) Trainium2 kernel, 8 NeuronCores.

Sharding: one graph per PAIR of cores; within a pair each core owns half the
hyperedge (Ec) / node (Nc) range. Host pre-casts the big matrices: the 0/1
incident matrix H ships as fp8e4 (exact) in the three layouts the PE needs
(hcol [n,Ec], htr [Ec,n], hrow [Nc,e]); Dv/De ship as fp8e3 scaled by 64
(descale folded into existing per-tile scale ops); x ships as bf16 in the
block-transposed stationary layout. All DMA is plain HWDGE; big streams ride
the ACT-engine DMA queue, bounce buffers the SP queue. Dataflow computes
hxx = H^T x first, then hx = hxx W + b (x) dege (bias as a rank-1 PE
accumulate against a host-computed edge-degree row); attention scores use a
hi/lo bf16 split of hxx and theta. Softmax is unnormalized; z rides the first
AllReduce; 1/z (and the Dv descale) fold into the post-AR transposes. htr is
SBUF-resident for its two uses. Each of the 3 pair-AllReduces is split into
two half-width collectives so the second half overlaps the consumers of the
first."""

import numpy as np

B, N, E, D = 4, 4096, 4096, 128
HALF = N // 2
NCORES = 8
PAIRS = [[0, 1], [2, 3], [4, 5], [6, 7]]
BN_EPS = 1e-5
F = 512                 # moving free-dim per matmul
NT = N // 128           # 32 k-tiles over a full 4096 dim
HT = HALF // 128        # 16 k-tiles over a half
DSCALE = 64.0           # host-side scale on Dv/De before fp8e3 cast

_CACHE = {}


def _build():
    import concourse.bacc as bacc
    import concourse.mybir as mybir
    import concourse.tile as tile
    from concourse.masks import make_identity
    from contextlib import ExitStack

    fp32 = mybir.dt.float32
    bf16 = mybir.dt.bfloat16
    fp8 = mybir.dt.float8e4
    fp8d = mybir.dt.float8e3
    Act = mybir.ActivationFunctionType
    Alu = mybir.AluOpType

    nc = bacc.Bacc("TRN2", target_bir_lowering=False, debug=False,
                   num_devices=NCORES)

    xv_d = nc.dram_tensor("xv", [128, N], bf16, kind="ExternalInput")
    hcol_d = nc.dram_tensor("hcol", [N, HALF], fp8, kind="ExternalInput")
    htr_d = nc.dram_tensor("htr", [HALF, N], fp8, kind="ExternalInput")
    hrow_d = nc.dram_tensor("hrow", [HALF, N], fp8, kind="ExternalInput")
    dvT_d = nc.dram_tensor("dvT", [N, HALF], fp8d, kind="ExternalInput")
    deT_d = nc.dram_tensor("deT", [N, HALF], fp8d, kind="ExternalInput")
    dege_d = nc.dram_tensor("dege", [1, HALF], fp32, kind="ExternalInput")
    whi_d = nc.dram_tensor("whi", [D, D], bf16, kind="ExternalInput")
    th2_d = nc.dram_tensor("th2", [D, 2], bf16, kind="ExternalInput")
    brow_d = nc.dram_tensor("brow", [1, D], fp32, kind="ExternalInput")
    mask_d = nc.dram_tensor("mask", [1, HALF], fp32, kind="ExternalInput")
    eps_d = nc.dram_tensor("eps", [D, 1], fp32, kind="ExternalInput")
    bng_d = nc.dram_tensor("bng", [D, 1], fp32, kind="ExternalInput")
    bnb_d = nc.dram_tensor("bnb", [D, 1], fp32, kind="ExternalInput")
    bnm_d = nc.dram_tensor("bnm", [D, 1], fp32, kind="ExternalInput")
    bnv_d = nc.dram_tensor("bnv", [D, 1], fp32, kind="ExternalInput")
    y_d = nc.dram_tensor("y", [D, N], fp32, kind="ExternalOutput")

    with tile.TileContext(nc) as tc, ExitStack() as ctx:
        const = ctx.enter_context(tc.tile_pool(name="const", bufs=1))
        resident = ctx.enter_context(tc.tile_pool(name="resident", bufs=1))
        stream = ctx.enter_context(tc.tile_pool(name="stream", bufs=3))
        med = ctx.enter_context(tc.tile_pool(name="med", bufs=1))
        small = ctx.enter_context(tc.tile_pool(name="small", bufs=1))
        ps = ctx.enter_context(tc.tile_pool(name="ps", bufs=8, space="PSUM"))
        dram = ctx.enter_context(tc.tile_pool(name="dram", bufs=1, space="DRAM"))

        ident = const.tile([128, 128], fp32)
        make_identity(nc, ident)
        one11 = const.tile([1, 1], fp32)
        nc.vector.memset(one11[:], 1.0)
        ones_row = const.tile([1, 128], fp32)
        nc.vector.memset(ones_row[:], 1.0)
        ones2 = const.tile([2, 1], fp32)
        nc.vector.memset(ones2[:], 1.0)
        c64 = const.tile([128, 1], fp32)
        nc.vector.memset(c64[:], 1.0 / DSCALE)

        def load_param(dt_):
            t = const.tile([D, 1], fp32, tag=dt_.name + "_p")
            nc.sync.dma_start(out=t[:], in_=dt_.ap())
            return t

        whi_t = const.tile([D, D], bf16)
        nc.sync.dma_start(out=whi_t[:], in_=whi_d.ap())
        th2_t = const.tile([D, 2], bf16)
        nc.sync.dma_start(out=th2_t[:], in_=th2_d.ap())
        brow_t = const.tile([1, D], fp32)
        nc.sync.dma_start(out=brow_t[:], in_=brow_d.ap())
        dege_t = const.tile([1, HALF], fp32)
        nc.sync.dma_start(out=dege_t[:], in_=dege_d.ap())
        eps_t = load_param(eps_d)
        bng_t = load_param(bng_d)
        bnb_t = load_param(bnb_d)
        bnm_t = load_param(bnm_d)
        bnv_t = load_param(bnv_d)
        mask_t = const.tile([1, HALF], fp32)
        nc.sync.dma_start(out=mask_t[:], in_=mask_d.ap())

        xv = const.tile([128, N], bf16)
        nc.sync.dma_start(out=xv[:], in_=xv_d.ap())

        # htr resident fp8 [128, HT*N]; loaded on the sync queue while the
        # hcol stream rides the ACT queue.
        htr_res = resident.tile([128, HT * N], fp8)
        for t in range(HT):
            nc.sync.dma_start(out=htr_res[:, t * N:(t + 1) * N],
                              in_=htr_d.ap()[t * 128:(t + 1) * 128, :])

        # ---- S2: hxxT [D, HALF] = (H[:,Ec]^T x)^T ------------------------
        hxx_ps = [ps.tile([128, F], fp32, tag="ps", name=f"hxx{i}")
                  for i in range(HALF // F)]
        for j in range(NT):
            hj = stream.tile([128, HALF], fp8, tag="stream8", name="hj")
            nc.scalar.dma_start(out=hj[:],
                                in_=hcol_d.ap()[j * 128:(j + 1) * 128, :])
            for blk in range(HALF // F):
                sl = slice(blk * F, (blk + 1) * F)
                nc.tensor.matmul(hxx_ps[blk][:],
                                 xv[:, j * D:(j + 1) * D], hj[:, sl],
                                 start=(j == 0), stop=(j == NT - 1))
        hxxT = med.tile([D, HALF], fp32, tag="hxxT")
        hxx_hi = med.tile([D, HALF], bf16, tag="hxx_hi")
        hxx_lo = med.tile([D, HALF], bf16, tag="hxx_lo")
        tmp32 = med.tile([D, HALF], fp32, tag="ehxT")
        for blk in range(HALF // F):
            sl = slice(blk * F, (blk + 1) * F)
            nc.vector.tensor_copy(hxxT[:, sl], hxx_ps[blk][:])
        nc.vector.tensor_copy(hxx_hi[:], hxxT[:])
        nc.vector.tensor_copy(tmp32[:], hxx_hi[:])
        nc.vector.tensor_tensor(tmp32[:], hxxT[:], tmp32[:], op=Alu.subtract)
        nc.vector.tensor_copy(hxx_lo[:], tmp32[:])

        # ---- S3: hxT = W^T hxx + b (x) dege ; st = th^T hxx --------------
        hxT = med.tile([D, HALF], fp32, tag="hxT")
        st_sb = small.tile([1, HALF], fp32, tag="st_sb")
        for blk in range(HALF // F):
            sl = slice(blk * F, (blk + 1) * F)
            hx2 = ps.tile([128, F], fp32, tag="ps", name=f"hx2_{blk}")
            nc.tensor.matmul(hx2[:], whi_t[:], hxx_hi[:, sl],
                             start=True, stop=False)
            nc.tensor.matmul(hx2[:], whi_t[:], hxx_lo[:, sl],
                             start=False, stop=False)
            nc.tensor.matmul(hx2[:], brow_t[:], dege_t[:, sl],
                             start=False, stop=True)
            nc.vector.tensor_copy(hxT[:, sl], hx2[:])
            st2 = ps.tile([2, F], fp32, tag="ps", name=f"st2_{blk}")
            nc.tensor.matmul(st2[:], th2_t[:], hxx_hi[:, sl],
                             start=True, stop=False)
            nc.tensor.matmul(st2[:], th2_t[:], hxx_lo[:, sl],
                             start=False, stop=True)
            s2sb = med.tile([2, F], fp32, tag="s2sb", name=f"s2sb{blk}")
            nc.vector.tensor_copy(s2sb[:], st2[0:2, :])
            sp = ps.tile([1, F], fp32, tag="ps", name=f"sp{blk}")
            nc.tensor.matmul(sp[:], ones2[:], s2sb[:],
                             start=True, stop=True)
            nc.vector.tensor_copy(st_sb[:, sl], sp[:])

        # ---- S4: softmax pieces (in-place on st_sb) ----------------------
        attn_u = st_sb
        nc.scalar.activation(attn_u[:], st_sb[:], Act.Exp)
        nc.vector.tensor_mul(attn_u[:], attn_u[:], mask_t[:])
        z_t = small.tile([1, 1], fp32, tag="z_t")
        nc.vector.reduce_sum(z_t[:], attn_u[:], axis=mybir.AxisListType.X)
        attnv = med.tile([128, HT], fp32, tag="attnv")
        for t in range(HT):
            pt = ps.tile([128, 1], fp32, tag="ps")
            nc.tensor.matmul(pt[:], attn_u[:, t * 128:(t + 1) * 128], one11[:],
                             start=True, stop=True)
            nc.vector.tensor_copy(attnv[:, t:t + 1], pt[:])
        ehxT = med.tile([D, HALF], fp32, tag="ehxT")
        nc.vector.tensor_scalar_mul(ehxT[:], hxT[:], eps_t[:])

        # ---- S5: h1av [128, HT*D] bf16 = attn * hx (e-part tiles) --------
        h1av = med.tile([128, HALF], bf16, tag="h1av")
        for t in range(HT):
            pt = ps.tile([128, 128], fp32, tag="ps")
            nc.tensor.transpose(pt[:], hxT[:, t * 128:(t + 1) * 128], ident[:])
            nc.vector.tensor_scalar_mul(h1av[:, t * 128:(t + 1) * 128], pt[:],
                                        attnv[:, t:t + 1])

        def chunked_bmm_ar(stationary, moving_of, tagbase, z_tile=None):
            """Two half-width (2048-col) partial bmms, each AllReduced
            separately so the second overlaps the first's consumers.
            stationary: [128, HT*128] SBUF tile (e- or n-part tiles)
            moving_of(t, lo, width): moving AP for k-tile t, cols lo:lo+width
            Returns (outA_sb, outB_sb) fp32 SBUF tiles [D, 2056]/[D, 2048]."""
            wA = HALF + 8 if z_tile is not None else HALF
            ccA_sb = med.tile([D, HALF + 8], fp32, tag="ccsb")
            ccB_sb = med.tile([D, HALF + 8], fp32, tag="ccsb2")
            outs = []
            for chunk in range(2):
                pss = [ps.tile([128, F], fp32, tag="ps",
                               name=f"{tagbase}_{chunk}_{i}")
                       for i in range(HALF // F)]
                for t in range(HT):
                    for blk in range(HALF // F):
                        nc.tensor.matmul(
                            pss[blk][:],
                            stationary[:, t * 128:(t + 1) * 128],
                            moving_of(t, chunk * HALF + blk * F, F),
                            start=(t == 0), stop=(t == HT - 1))
                cc_sb = ccA_sb if chunk == 0 else ccB_sb
                for blk in range(HALF // F):
                    sl = slice(blk * F, (blk + 1) * F)
                    nc.vector.tensor_copy(cc_sb[:, sl], pss[blk][:])
                if chunk == 0 and z_tile is not None:
                    nc.vector.memset(cc_sb[:, HALF:], 0.0)
                    nc.vector.tensor_copy(cc_sb[0:1, HALF:HALF + 1], z_tile[:])
                w = wA if chunk == 0 else HALF
                cc_in = dram.tile([D, w], fp32, tag=f"{tagbase}i{chunk}")
                cc_out = dram.tile([D, w], fp32, tag=f"{tagbase}o{chunk}")
                nc.sync.dma_start(out=cc_in[:], in_=cc_sb[:, 0:w])
                nc.gpsimd.collective_compute(
                    "AllReduce", Alu.add, replica_groups=PAIRS,
                    ins=[cc_in.opt()], outs=[cc_out.opt()])
                res_sb = med.tile([D, HALF + 8], fp32,
                                  tag=("h1bA" if chunk == 0 else "h1bB"))
                nc.sync.dma_start(out=res_sb[:, 0:w], in_=cc_out[:])
                outs.append(res_sb)
            return outs

        # ---- S6: h1b = H h1a (partial over Ec), chunked AR ---------------
        h1bA, h1bB = chunked_bmm_ar(
            h1av,
            lambda t, lo, w: htr_res[:, t * N + lo:t * N + lo + w],
            "cc1", z_tile=z_t)

        # 1/(z*DSCALE) broadcast to [128, 1]
        rz = small.tile([1, 1], fp32, tag="rz")
        nc.vector.reciprocal(rz[:], h1bA[0:1, HALF:HALF + 1])
        rz_ps = ps.tile([128, 1], fp32, tag="ps")
        nc.tensor.matmul(rz_ps[:], ones_row[:], rz[:], start=True, stop=True)
        rz_bc = small.tile([128, 1], fp32, tag="rz_bc")
        nc.vector.tensor_copy(rz_bc[:], rz_ps[:])
        nc.vector.tensor_mul(rz_bc[:], rz_bc[:], c64[:])

        # h1bv [128, NT*D] bf16 scaled by 1/(z*DSCALE)
        h1bv = med.tile([128, N], bf16, tag="h1bv")
        for j in range(NT):
            src = h1bA if j < HT else h1bB
            jj = j if j < HT else j - HT
            pt = ps.tile([128, 128], fp32, tag="ps")
            nc.tensor.transpose(pt[:], src[:, jj * 128:(jj + 1) * 128],
                                ident[:])
            nc.vector.tensor_scalar_mul(h1bv[:, j * 128:(j + 1) * 128], pt[:],
                                        rz_bc[:])

        # ---- S7: h1cT [D, Nc] = (Dv[Nc,:] h1b)^T  (descale via rz_bc) ----
        h1c_ps = [ps.tile([128, F], fp32, tag="ps", name=f"h1c{i}")
                  for i in range(HALF // F)]
        for j in range(NT):
            dj = stream.tile([128, HALF], fp8d, tag="stream8", name="dj")
            nc.scalar.dma_start(out=dj[:],
                                in_=dvT_d.ap()[j * 128:(j + 1) * 128, :])
            for blk in range(HALF // F):
                sl = slice(blk * F, (blk + 1) * F)
                nc.tensor.matmul(h1c_ps[blk][:],
                                 h1bv[:, j * 128:(j + 1) * 128], dj[:, sl],
                                 start=(j == 0), stop=(j == NT - 1))
        h1cT = med.tile([D, HALF], fp32, tag="hxxT")
        for blk in range(HALF // F):
            sl = slice(blk * F, (blk + 1) * F)
            nc.vector.tensor_copy(h1cT[:, sl], h1c_ps[blk][:])
        h1cv = med.tile([128, HALF], bf16, tag="h1cv")
        for t in range(HT):
            pt = ps.tile([128, 128], fp32, tag="ps")
            nc.tensor.transpose(pt[:], h1cT[:, t * 128:(t + 1) * 128], ident[:])
            nc.vector.tensor_copy(h1cv[:, t * 128:(t + 1) * 128], pt[:])

        # ---- S8: h1d = H[Nc,:]^T h1c (partial over Nc), chunked AR -------
        # stream hrow half-rows per chunk on the ACT queue
        hrow_tiles = {}

        def hrow_moving(t, lo, w):
            key = (t, lo // HALF)
            if key not in hrow_tiles:
                rj = stream.tile([128, HALF], fp8, tag="stream8",
                                 name=f"rj{key[1]}")
                nc.scalar.dma_start(
                    out=rj[:],
                    in_=hrow_d.ap()[t * 128:(t + 1) * 128,
                                    key[1] * HALF:(key[1] + 1) * HALF])
                hrow_tiles[key] = rj
            base = lo % HALF
            return hrow_tiles[key][:, base:base + w]

        h1dA, h1dB = chunked_bmm_ar(h1cv, hrow_moving, "cc2")

        # h1dv bf16 scaled by 1/DSCALE (descale for the De matmul)
        h1dv = med.tile([128, N], bf16, tag="h1bv")
        for j in range(NT):
            src = h1dA if j < HT else h1dB
            jj = j if j < HT else j - HT
            pt = ps.tile([128, 128], fp32, tag="ps")
            nc.tensor.transpose(pt[:], src[:, jj * 128:(jj + 1) * 128],
                                ident[:])
            nc.vector.tensor_scalar_mul(h1dv[:, j * 128:(j + 1) * 128], pt[:],
                                        c64[:])

        # ---- S9+S10: hT [D, Ec] = (De[Ec,:] h1d)^T + eps*hx --------------
        h1e_ps = [ps.tile([128, F], fp32, tag="ps", name=f"h1e{i}")
                  for i in range(HALF // F)]
        for j in range(NT):
            ej = stream.tile([128, HALF], fp8d, tag="stream8", name="ej")
            nc.scalar.dma_start(out=ej[:],
                                in_=deT_d.ap()[j * 128:(j + 1) * 128, :])
            for blk in range(HALF // F):
                sl = slice(blk * F, (blk + 1) * F)
                nc.tensor.matmul(h1e_ps[blk][:],
                                 h1dv[:, j * 128:(j + 1) * 128], ej[:, sl],
                                 start=(j == 0), stop=(j == NT - 1))
        hT = med.tile([D, HALF], fp32, tag="hxT")
        for blk in range(HALF // F):
            sl = slice(blk * F, (blk + 1) * F)
            nc.vector.tensor_tensor(hT[:, sl], h1e_ps[blk][:], ehxT[:, sl],
                                    op=Alu.add)
        hv = med.tile([128, HALF], bf16, tag="h1av")
        for t in range(HT):
            pt = ps.tile([128, 128], fp32, tag="ps")
            nc.tensor.transpose(pt[:], hT[:, t * 128:(t + 1) * 128], ident[:])
            nc.vector.tensor_copy(hv[:, t * 128:(t + 1) * 128], pt[:])

        # ---- S11: out = H h (partial over Ec), chunked AR + epilogue -----
        s_bn = small.tile([D, 1], fp32, tag="s_bn")
        nc.vector.tensor_scalar_add(s_bn[:], bnv_t[:], BN_EPS)
        nc.scalar.activation(s_bn[:], s_bn[:], Act.Sqrt)
        nc.vector.reciprocal(s_bn[:], s_bn[:])
        nc.vector.tensor_mul(s_bn[:], s_bn[:], bng_t[:])
        t_bn = small.tile([D, 1], fp32, tag="t_bn")
        nc.vector.tensor_mul(t_bn[:], bnm_t[:], s_bn[:])
        nc.vector.tensor_tensor(t_bn[:], bnb_t[:], t_bn[:], op=Alu.subtract)

        outA, outB = chunked_bmm_ar(
            hv,
            lambda t, lo, w: htr_res[:, t * N + lo:t * N + lo + w],
            "cc3")
        for chunk, ot in enumerate((outA, outB)):
            nc.scalar.activation(ot[:, 0:HALF], ot[:, 0:HALF], Act.Lrelu,
                                 alpha=0.01)
            nc.vector.tensor_scalar(ot[:, 0:HALF], ot[:, 0:HALF], s_bn[:],
                                    t_bn[:], op0=Alu.mult, op1=Alu.add)
            nc.sync.dma_start(out=y_d.ap()[:, chunk * HALF:(chunk + 1) * HALF],
                              in_=ot[:, 0:HALF])

    nc.finalize()
    return nc


def _get_nc():
    if "nc" not in _CACHE:
        _CACHE["nc"] = _build()
    return _CACHE["nc"]


def _shard(inputs):
    import ml_dtypes
    bf16 = ml_dtypes.bfloat16
    fp8 = ml_dtypes.float8_e4m3
    fp8d = ml_dtypes.float8_e3m4

    H = np.asarray(inputs["incident_mat"], dtype=np.float32)
    Dv = np.asarray(inputs["degree_v"], dtype=np.float32)
    De = np.asarray(inputs["degree_e"], dtype=np.float32)
    x = np.asarray(inputs["x"], dtype=np.float32)
    em = np.asarray(inputs["e_masks"])
    w = np.asarray(inputs["mlp_W"], dtype=np.float32)
    b = np.asarray(inputs["mlp_b"], dtype=np.float32)
    th = np.asarray(inputs["theta_att"], dtype=np.float32).reshape(D)
    eps = np.full((D, 1), float(np.asarray(inputs["eps"]).reshape(-1)[0]),
                  dtype=np.float32)

    def col(v):
        return np.ascontiguousarray(
            np.asarray(v, dtype=np.float32).reshape(D, 1))

    bng, bnb = col(inputs["bn_gamma"]), col(inputs["bn_beta"])
    bnm, bnv = col(inputs["bn_mean"]), col(inputs["bn_var"])

    whi = w.astype(bf16)
    th_hi = th.astype(bf16)
    th_lo = (th - th_hi.astype(np.float32)).astype(bf16)
    th2 = np.ascontiguousarray(np.stack(
        [th_hi.astype(np.float32), th_lo.astype(np.float32)], axis=1)
    ).astype(bf16)
    brow = np.ascontiguousarray(b.reshape(1, D))

    in_maps = []
    for g in range(B):
        Hg8 = H[g].astype(fp8)
        HgT8 = np.ascontiguousarray(H[g].T).astype(fp8)
        dege_full = H[g].sum(axis=0, dtype=np.float32)
        xv = np.ascontiguousarray(
            x[g].reshape(NT, 128, D).transpose(1, 0, 2).reshape(128, NT * D)
        ).astype(bf16)
        DvT = np.ascontiguousarray(Dv[g].T * DSCALE).astype(fp8d)
        DeT = np.ascontiguousarray(De[g].T * DSCALE).astype(fp8d)
        for c in range(2):
            lo, hi = c * HALF, (c + 1) * HALF
            in_maps.append({
                "xv": xv,
                "hcol": np.ascontiguousarray(Hg8[:, lo:hi]),
                "htr": np.ascontiguousarray(HgT8[lo:hi, :]),
                "hrow": np.ascontiguousarray(Hg8[lo:hi, :]),
                "dvT": np.ascontiguousarray(DvT[:, lo:hi]),
                "deT": np.ascontiguousarray(DeT[:, lo:hi]),
                "dege": np.ascontiguousarray(
                    dege_full[lo:hi].reshape(1, HALF)),
                "whi": whi, "th2": th2, "brow": brow,
                "mask": np.ascontiguousarray(
                    em[g, lo:hi].astype(np.float32).reshape(1, HALF)),
                "eps": eps,
                "bng": bng, "bnb": bnb, "bnm": bnm, "bnv": bnv,
            })
    return in_maps


def kernel(**inputs):
    from concourse.bass_utils import run_bass_kernel_spmd

    nc = _get_nc()
    in_maps = _shard(inputs)
    res = run_bass_kernel_spmd(nc, in_maps, list(range(NCORES)))
    out = np.empty((B, N, D), dtype=np.float32)
    for g in range(B):
        ya = res.results[2 * g]["y"]
        yb = res.results[2 * g + 1]["y"]
        out[g, :HALF, :] = ya[:, :HALF].T
        out[g, HALF:, :] = yb[:, HALF:].T
    return out


# revision 11
# speedup vs baseline: 2.0810x; 1.2620x over previous
"""HGNN layer (hypergraph message passing) Trainium2 kernel, 8 NeuronCores.

Sharding: one graph per PAIR of cores; within a pair each core owns half the
hyperedge (Ec) / node (Nc) range. Host pre-casts the big matrices: the 0/1
incident matrix H ships as fp8e4 (exact) in the three layouts the PE needs;
Dv/De ship as fp8e3 scaled by 64 (descale folded into later evacuations); x
ships bf16 in block-transposed stationary layout. Streams use host-tiled
[128, k*HALF] layouts so slab DMAs move 1 MB at a time on the ACT HWDGE
queue. Dataflow computes hxx = H^T x first, then hx = hxx W + b (x) dege
(bias as a rank-1 PE accumulate against a host-computed edge-degree row);
attention scores use a hi/lo bf16 split of hxx and theta. Softmax is
unnormalized; z rides the first AllReduce as a bf16 hi/lo pair; 1/z and the
Dv descale fold into the h1c evacuation. htr stays SBUF-resident for its two
uses. Each of the 3 pair-AllReduces is split into two half-width bf16
collectives whose payloads are pre-transposed into the consumer's layout, so
the second half overlaps the first half's consumers and there is zero
post-AR rearrangement."""

import numpy as np

B, N, E, D = 4, 4096, 4096, 128
HALF = N // 2
NCORES = 8
PAIRS = [[0, 1], [2, 3], [4, 5], [6, 7]]
BN_EPS = 1e-5
F = 512                 # moving free-dim per matmul
NT = N // 128           # 32 k-tiles over a full 4096 dim
HT = HALF // 128        # 16 k-tiles over a half
SLAB = 4                # k-tiles per stream DMA (1 MB slabs)
DSCALE = 64.0           # host-side scale on Dv/De before fp8e3 cast
ZPAD = 16               # extra bf16 cols on the first AR chunk for z hi/lo

_CACHE = {}


def _build():
    import concourse.bacc as bacc
    import concourse.mybir as mybir
    import concourse.tile as tile
    from concourse.masks import make_identity
    from contextlib import ExitStack

    fp32 = mybir.dt.float32
    bf16 = mybir.dt.bfloat16
    fp8 = mybir.dt.float8e4
    fp8d = mybir.dt.float8e3
    Act = mybir.ActivationFunctionType
    Alu = mybir.AluOpType

    nc = bacc.Bacc("TRN2", target_bir_lowering=False, debug=False,
                   num_devices=NCORES)

    xv_d = nc.dram_tensor("xv", [128, N], bf16, kind="ExternalInput")
    hcol_d = nc.dram_tensor("hcol", [128, NT * HALF], fp8, kind="ExternalInput")
    htr_d = nc.dram_tensor("htr", [HALF, N], fp8, kind="ExternalInput")
    hrow_d = nc.dram_tensor("hrow", [128, 2 * HT * HALF], fp8,
                            kind="ExternalInput")
    dvT_d = nc.dram_tensor("dvT", [128, NT * HALF], fp8d, kind="ExternalInput")
    deT_d = nc.dram_tensor("deT", [128, NT * HALF], fp8d, kind="ExternalInput")
    dege_d = nc.dram_tensor("dege", [1, HALF], fp32, kind="ExternalInput")
    whi_d = nc.dram_tensor("whi", [D, D], bf16, kind="ExternalInput")
    th2_d = nc.dram_tensor("th2", [D, 2], bf16, kind="ExternalInput")
    brow_d = nc.dram_tensor("brow", [1, D], fp32, kind="ExternalInput")
    mask_d = nc.dram_tensor("mask", [1, HALF], fp32, kind="ExternalInput")
    eps_d = nc.dram_tensor("eps", [D, 1], fp32, kind="ExternalInput")
    bng_d = nc.dram_tensor("bng", [D, 1], fp32, kind="ExternalInput")
    bnb_d = nc.dram_tensor("bnb", [D, 1], fp32, kind="ExternalInput")
    bnm_d = nc.dram_tensor("bnm", [D, 1], fp32, kind="ExternalInput")
    bnv_d = nc.dram_tensor("bnv", [D, 1], fp32, kind="ExternalInput")
    y_d = nc.dram_tensor("y", [D, N], fp32, kind="ExternalOutput")

    with tile.TileContext(nc) as tc, ExitStack() as ctx:
        const = ctx.enter_context(tc.tile_pool(name="const", bufs=1))
        resident = ctx.enter_context(tc.tile_pool(name="resident", bufs=1))
        stream = ctx.enter_context(tc.tile_pool(name="stream", bufs=3))
        med = ctx.enter_context(tc.tile_pool(name="med", bufs=1))
        small = ctx.enter_context(tc.tile_pool(name="small", bufs=1))
        ps = ctx.enter_context(tc.tile_pool(name="ps", bufs=8, space="PSUM"))
        dram = ctx.enter_context(tc.tile_pool(name="dram", bufs=1, space="DRAM"))

        ident = const.tile([128, 128], fp32)
        make_identity(nc, ident)
        one11 = const.tile([1, 1], fp32)
        nc.vector.memset(one11[:], 1.0)
        ones_row = const.tile([1, 128], fp32)
        nc.vector.memset(ones_row[:], 1.0)
        ones2 = const.tile([2, 1], fp32)
        nc.vector.memset(ones2[:], 1.0)
        c64 = const.tile([128, 1], fp32)
        nc.vector.memset(c64[:], 1.0 / DSCALE)

        def load_param(dt_):
            t = const.tile([D, 1], fp32, tag=dt_.name + "_p")
            nc.sync.dma_start(out=t[:], in_=dt_.ap())
            return t

        whi_t = const.tile([D, D], bf16)
        nc.sync.dma_start(out=whi_t[:], in_=whi_d.ap())
        th2_t = const.tile([D, 2], bf16)
        nc.sync.dma_start(out=th2_t[:], in_=th2_d.ap())
        brow_t = const.tile([1, D], fp32)
        nc.sync.dma_start(out=brow_t[:], in_=brow_d.ap())
        dege_t = const.tile([1, HALF], fp32)
        nc.sync.dma_start(out=dege_t[:], in_=dege_d.ap())
        eps_t = load_param(eps_d)
        bng_t = load_param(bng_d)
        bnb_t = load_param(bnb_d)
        bnm_t = load_param(bnm_d)
        bnv_t = load_param(bnv_d)
        mask_t = const.tile([1, HALF], fp32)
        nc.sync.dma_start(out=mask_t[:], in_=mask_d.ap())

        xv = const.tile([128, N], bf16)
        nc.sync.dma_start(out=xv[:], in_=xv_d.ap())

        # htr resident fp8 (sync queue; hcol stream rides the ACT queue)
        htr_res = resident.tile([128, HT * N], fp8)
        for t in range(HT):
            nc.sync.dma_start(out=htr_res[:, t * N:(t + 1) * N],
                              in_=htr_d.ap()[t * 128:(t + 1) * 128, :])

        def slab_stream(dram_t, dt, n_tiles, name):
            """Yield (k_tile_index, moving_tile_fn) streaming 1MB slabs."""
            for s in range(n_tiles // SLAB):
                sb = stream.tile([128, SLAB * HALF], dt, tag="slab",
                                 name=name)
                nc.scalar.dma_start(
                    out=sb[:],
                    in_=dram_t.ap()[:, s * SLAB * HALF:(s + 1) * SLAB * HALF])
                for jj in range(SLAB):
                    j = s * SLAB + jj
                    yield j, sb[:, jj * HALF:(jj + 1) * HALF]

        # ---- S2: hxxT [D, HALF] = (H[:,Ec]^T x)^T ------------------------
        hxx_ps = [ps.tile([128, F], fp32, tag="ps", name=f"hxx{i}")
                  for i in range(HALF // F)]
        for j, hj in slab_stream(hcol_d, fp8, NT, "hj"):
            for blk in range(HALF // F):
                nc.tensor.matmul(hxx_ps[blk][:],
                                 xv[:, j * D:(j + 1) * D],
                                 hj[:, blk * F:(blk + 1) * F],
                                 start=(j == 0), stop=(j == NT - 1))
        hxxT = med.tile([D, HALF], fp32, tag="hxxT")
        hxx_hi = med.tile([D, HALF], bf16, tag="hxx_hi")
        hxx_lo = med.tile([D, HALF], bf16, tag="hxx_lo")
        tmp32 = med.tile([D, HALF], fp32, tag="ehxT")
        for blk in range(HALF // F):
            sl = slice(blk * F, (blk + 1) * F)
            nc.vector.tensor_copy(hxxT[:, sl], hxx_ps[blk][:])
        nc.vector.tensor_copy(hxx_hi[:], hxxT[:])
        nc.vector.tensor_copy(tmp32[:], hxx_hi[:])
        nc.vector.tensor_tensor(tmp32[:], hxxT[:], tmp32[:], op=Alu.subtract)
        nc.vector.tensor_copy(hxx_lo[:], tmp32[:])

        # ---- S3: hxT = W^T hxx + b (x) dege ; st = th^T hxx --------------
        hxT = med.tile([D, HALF], fp32, tag="hxT")
        st_sb = small.tile([1, HALF], fp32, tag="st_sb")
        for blk in range(HALF // F):
            sl = slice(blk * F, (blk + 1) * F)
            hx2 = ps.tile([128, F], fp32, tag="ps", name=f"hx2_{blk}")
            nc.tensor.matmul(hx2[:], whi_t[:], hxx_hi[:, sl],
                             start=True, stop=False)
            nc.tensor.matmul(hx2[:], whi_t[:], hxx_lo[:, sl],
                             start=False, stop=False)
            nc.tensor.matmul(hx2[:], brow_t[:], dege_t[:, sl],
                             start=False, stop=True)
            nc.vector.tensor_copy(hxT[:, sl], hx2[:])
            st2 = ps.tile([2, F], fp32, tag="ps", name=f"st2_{blk}")
            nc.tensor.matmul(st2[:], th2_t[:], hxx_hi[:, sl],
                             start=True, stop=False)
            nc.tensor.matmul(st2[:], th2_t[:], hxx_lo[:, sl],
                             start=False, stop=True)
            s2sb = med.tile([2, F], fp32, tag="s2sb", name=f"s2sb{blk}")
            nc.vector.tensor_copy(s2sb[:], st2[0:2, :])
            sp = ps.tile([1, F], fp32, tag="ps", name=f"sp{blk}")
            nc.tensor.matmul(sp[:], ones2[:], s2sb[:], start=True, stop=True)
            nc.vector.tensor_copy(st_sb[:, sl], sp[:])

        # ---- S4: softmax pieces (in-place on st_sb) ----------------------
        attn_u = st_sb
        nc.scalar.activation(attn_u[:], st_sb[:], Act.Exp)
        nc.vector.tensor_mul(attn_u[:], attn_u[:], mask_t[:])
        z_t = small.tile([1, 1], fp32, tag="z_t")
        nc.vector.reduce_sum(z_t[:], attn_u[:], axis=mybir.AxisListType.X)
        # z hi/lo bf16 pieces
        zhi = small.tile([1, 1], bf16, tag="zhi")
        zlo = small.tile([1, 1], bf16, tag="zlo")
        zf = small.tile([1, 1], fp32, tag="zf")
        nc.vector.tensor_copy(zhi[:], z_t[:])
        nc.vector.tensor_copy(zf[:], zhi[:])
        nc.vector.tensor_tensor(zf[:], z_t[:], zf[:], op=Alu.subtract)
        nc.vector.tensor_copy(zlo[:], zf[:])
        attnv = med.tile([128, HT], fp32, tag="attnv")
        for t in range(HT):
            pt = ps.tile([128, 1], fp32, tag="ps")
            nc.tensor.matmul(pt[:], attn_u[:, t * 128:(t + 1) * 128], one11[:],
                             start=True, stop=True)
            nc.vector.tensor_copy(attnv[:, t:t + 1], pt[:])
        ehxT = med.tile([D, HALF], fp32, tag="ehxT")
        nc.vector.tensor_scalar_mul(ehxT[:], hxT[:], eps_t[:])

        # ---- S5: h1av [128, HT*D] bf16 = attn * hx (e-part tiles) --------
        h1av = med.tile([128, HALF], bf16, tag="h1av")
        for t in range(HT):
            pt = ps.tile([128, 128], fp32, tag="ps")
            nc.tensor.transpose(pt[:], hxT[:, t * 128:(t + 1) * 128], ident[:])
            nc.vector.tensor_scalar_mul(h1av[:, t * 128:(t + 1) * 128], pt[:],
                                        attnv[:, t:t + 1])

        def chunked_bmm_ar_v(stationary, moving_of, tagbase, with_z=False):
            """Two half-width partial bmms; payload pre-transposed to the
            consumer's v-layout, cast bf16, AllReduced per chunk. Returns
            (resA, resB) bf16 [128, HALF(+ZPAD)] v-layout tiles."""
            outs = []
            for chunk in range(2):
                w = HALF + ZPAD if (with_z and chunk == 0) else HALF
                pss = [ps.tile([128, F], fp32, tag="ps",
                               name=f"{tagbase}_{chunk}_{i}")
                       for i in range(HALF // F)]
                for t in range(HT):
                    for blk in range(HALF // F):
                        nc.tensor.matmul(
                            pss[blk][:],
                            stationary[:, t * 128:(t + 1) * 128],
                            moving_of(t, chunk * HALF + blk * F, F),
                            start=(t == 0), stop=(t == HT - 1))
                ccT = med.tile([D, HALF], fp32, tag="ccsbT")
                for blk in range(HALF // F):
                    sl = slice(blk * F, (blk + 1) * F)
                    nc.vector.tensor_copy(ccT[:, sl], pss[blk][:])
                ccv = med.tile([128, HALF + ZPAD], bf16,
                               tag=f"ccv{chunk}")
                for t in range(HT):
                    pt = ps.tile([128, 128], fp32, tag="ps")
                    nc.tensor.transpose(pt[:], ccT[:, t * 128:(t + 1) * 128],
                                        ident[:])
                    nc.vector.tensor_copy(ccv[:, t * 128:(t + 1) * 128], pt[:])
                if with_z and chunk == 0:
                    nc.vector.memset(ccv[:, HALF:], 0.0)
                    nc.vector.tensor_copy(ccv[0:1, HALF:HALF + 1], zhi[:])
                    nc.vector.tensor_copy(ccv[0:1, HALF + 1:HALF + 2], zlo[:])
                cc_in = dram.tile([128, w], bf16, tag=f"{tagbase}i{chunk}")
                cc_out = dram.tile([128, w], bf16, tag=f"{tagbase}o{chunk}")
                nc.sync.dma_start(out=cc_in[:], in_=ccv[:, 0:w])
                nc.gpsimd.collective_compute(
                    "AllReduce", Alu.add, replica_groups=PAIRS,
                    ins=[cc_in.opt()], outs=[cc_out.opt()])
                res = med.tile([128, HALF + ZPAD], bf16,
                               tag=f"resv{chunk}")
                nc.sync.dma_start(out=res[:, 0:w], in_=cc_out[:])
                outs.append(res)
            return outs

        # ---- S6: h1b = H h1a (partial over Ec), v-layout chunked AR ------
        h1bA, h1bB = chunked_bmm_ar_v(
            h1av,
            lambda t, lo, w: htr_res[:, t * N + lo:t * N + lo + w],
            "cc1", with_z=True)

        # rz = 1/(z), folded with 1/DSCALE, broadcast to [128, 1]
        rz = small.tile([1, 1], fp32, tag="rz")
        zs = small.tile([1, 1], fp32, tag="zs")
        nc.vector.tensor_copy(rz[:], h1bA[0:1, HALF:HALF + 1])
        nc.vector.tensor_copy(zs[:], h1bA[0:1, HALF + 1:HALF + 2])
        nc.vector.tensor_tensor(rz[:], rz[:], zs[:], op=Alu.add)
        nc.vector.reciprocal(rz[:], rz[:])
        rz_ps = ps.tile([128, 1], fp32, tag="ps")
        nc.tensor.matmul(rz_ps[:], ones_row[:], rz[:], start=True, stop=True)
        rz_bc = small.tile([128, 1], fp32, tag="rz_bc")
        nc.vector.tensor_copy(rz_bc[:], rz_ps[:])
        nc.vector.tensor_mul(rz_bc[:], rz_bc[:], c64[:])

        def vtile(resA, resB, j):
            src = resA if j < HT else resB
            jj = j % HT
            return src[:, jj * 128:(jj + 1) * 128]

        # ---- S7: h1cT = (Dv[Nc,:] h1b)^T * rz/DSCALE ---------------------
        h1c_ps = [ps.tile([128, F], fp32, tag="ps", name=f"h1c{i}")
                  for i in range(HALF // F)]
        for j, dj in slab_stream(dvT_d, fp8d, NT, "dj"):
            for blk in range(HALF // F):
                nc.tensor.matmul(h1c_ps[blk][:], vtile(h1bA, h1bB, j),
                                 dj[:, blk * F:(blk + 1) * F],
                                 start=(j == 0), stop=(j == NT - 1))
        h1cT = med.tile([D, HALF], fp32, tag="hxxT")
        for blk in range(HALF // F):
            sl = slice(blk * F, (blk + 1) * F)
            nc.vector.tensor_scalar_mul(h1cT[:, sl], h1c_ps[blk][:], rz_bc[:])
        h1cv = med.tile([128, HALF], bf16, tag="h1cv")
        for t in range(HT):
            pt = ps.tile([128, 128], fp32, tag="ps")
            nc.tensor.transpose(pt[:], h1cT[:, t * 128:(t + 1) * 128], ident[:])
            nc.vector.tensor_copy(h1cv[:, t * 128:(t + 1) * 128], pt[:])

        # ---- S8: h1d = H[Nc,:]^T h1c (partial over Nc), chunked AR -------
        hrow_state = {}

        def hrow_moving(t, lo, w):
            chunk = lo // HALF
            slab_i = t // SLAB
            key = (chunk, slab_i)
            if key not in hrow_state:
                sb = stream.tile([128, SLAB * HALF], fp8, tag="slab",
                                 name=f"rj{chunk}")
                base = chunk * HT * HALF + slab_i * SLAB * HALF
                nc.scalar.dma_start(
                    out=sb[:], in_=hrow_d.ap()[:, base:base + SLAB * HALF])
                hrow_state[key] = sb
            return hrow_state[key][:, (t % SLAB) * HALF + (lo % HALF):
                                   (t % SLAB) * HALF + (lo % HALF) + w]

        h1dA, h1dB = chunked_bmm_ar_v(h1cv, hrow_moving, "cc2")

        # ---- S9+S10: hT = (De[Ec,:] h1d)^T / DSCALE + eps*hx -------------
        h1e_ps = [ps.tile([128, F], fp32, tag="ps", name=f"h1e{i}")
                  for i in range(HALF // F)]
        for j, ej in slab_stream(deT_d, fp8d, NT, "ej"):
            for blk in range(HALF // F):
                nc.tensor.matmul(h1e_ps[blk][:], vtile(h1dA, h1dB, j),
                                 ej[:, blk * F:(blk + 1) * F],
                                 start=(j == 0), stop=(j == NT - 1))
        hT = med.tile([D, HALF], fp32, tag="hxT")
        for blk in range(HALF // F):
            sl = slice(blk * F, (blk + 1) * F)
            nc.vector.scalar_tensor_tensor(hT[:, sl], h1e_ps[blk][:],
                                           1.0 / DSCALE, ehxT[:, sl],
                                           op0=Alu.mult, op1=Alu.add)
        hv = med.tile([128, HALF], bf16, tag="h1av")
        for t in range(HT):
            pt = ps.tile([128, 128], fp32, tag="ps")
            nc.tensor.transpose(pt[:], hT[:, t * 128:(t + 1) * 128], ident[:])
            nc.vector.tensor_copy(hv[:, t * 128:(t + 1) * 128], pt[:])

        # ---- S11: out = H h (partial over Ec), bf16 T-layout chunked AR --
        s_bn = small.tile([D, 1], fp32, tag="s_bn")
        nc.vector.tensor_scalar_add(s_bn[:], bnv_t[:], BN_EPS)
        nc.scalar.activation(s_bn[:], s_bn[:], Act.Sqrt)
        nc.vector.reciprocal(s_bn[:], s_bn[:])
        nc.vector.tensor_mul(s_bn[:], s_bn[:], bng_t[:])
        t_bn = small.tile([D, 1], fp32, tag="t_bn")
        nc.vector.tensor_mul(t_bn[:], bnm_t[:], s_bn[:])
        nc.vector.tensor_tensor(t_bn[:], bnb_t[:], t_bn[:], op=Alu.subtract)

        for chunk in range(2):
            pss = [ps.tile([128, F], fp32, tag="ps", name=f"out_{chunk}_{i}")
                   for i in range(HALF // F)]
            for t in range(HT):
                for blk in range(HALF // F):
                    lo = chunk * HALF + blk * F
                    nc.tensor.matmul(pss[blk][:],
                                     hv[:, t * 128:(t + 1) * 128],
                                     htr_res[:, t * N + lo:t * N + lo + F],
                                     start=(t == 0), stop=(t == HT - 1))
            ccv = med.tile([128, HALF + ZPAD], bf16, tag=f"ccv{chunk}")
            for blk in range(HALF // F):
                sl = slice(blk * F, (blk + 1) * F)
                nc.vector.tensor_copy(ccv[:, sl], pss[blk][:])
            cc_in = dram.tile([128, HALF], bf16, tag=f"cc3i{chunk}")
            cc_out = dram.tile([128, HALF], bf16, tag=f"cc3o{chunk}")
            nc.sync.dma_start(out=cc_in[:], in_=ccv[:, 0:HALF])
            nc.gpsimd.collective_compute(
                "AllReduce", Alu.add, replica_groups=PAIRS,
                ins=[cc_in.opt()], outs=[cc_out.opt()])
            res = med.tile([128, HALF + ZPAD], bf16, tag=f"resv{chunk}")
            nc.sync.dma_start(out=res[:, 0:HALF], in_=cc_out[:])
            # epilogue on this half: bn(leaky_relu(out))
            of = med.tile([D, HALF], fp32, tag="ccsbT")
            nc.scalar.activation(of[:], res[:, 0:HALF], Act.Lrelu, alpha=0.01)
            nc.vector.tensor_scalar(of[:], of[:], s_bn[:], t_bn[:],
                                    op0=Alu.mult, op1=Alu.add)
            nc.sync.dma_start(out=y_d.ap()[:, chunk * HALF:(chunk + 1) * HALF],
                              in_=of[:])

    nc.finalize()
    return nc


def _get_nc():
    if "nc" not in _CACHE:
        _CACHE["nc"] = _build()
    return _CACHE["nc"]


def _tile128(a):
    """[K*128, W] -> [128, K*W] block-transposed stream layout."""
    K = a.shape[0] // 128
    return np.ascontiguousarray(
        a.reshape(K, 128, a.shape[1]).transpose(1, 0, 2).reshape(
            128, K * a.shape[1]))


def _shard(inputs):
    import ml_dtypes
    bf16 = ml_dtypes.bfloat16
    fp8 = ml_dtypes.float8_e4m3
    fp8d = ml_dtypes.float8_e3m4

    H = np.asarray(inputs["incident_mat"], dtype=np.float32)
    Dv = np.asarray(inputs["degree_v"], dtype=np.float32)
    De = np.asarray(inputs["degree_e"], dtype=np.float32)
    x = np.asarray(inputs["x"], dtype=np.float32)
    em = np.asarray(inputs["e_masks"])
    w = np.asarray(inputs["mlp_W"], dtype=np.float32)
    b = np.asarray(inputs["mlp_b"], dtype=np.float32)
    th = np.asarray(inputs["theta_att"], dtype=np.float32).reshape(D)
    eps = np.full((D, 1), float(np.asarray(inputs["eps"]).reshape(-1)[0]),
                  dtype=np.float32)

    def col(v):
        return np.ascontiguousarray(
            np.asarray(v, dtype=np.float32).reshape(D, 1))

    bng, bnb = col(inputs["bn_gamma"]), col(inputs["bn_beta"])
    bnm, bnv = col(inputs["bn_mean"]), col(inputs["bn_var"])

    whi = w.astype(bf16)
    th_hi = th.astype(bf16)
    th_lo = (th - th_hi.astype(np.float32)).astype(bf16)
    th2 = np.ascontiguousarray(np.stack(
        [th_hi.astype(np.float32), th_lo.astype(np.float32)], axis=1)
    ).astype(bf16)
    brow = np.ascontiguousarray(b.reshape(1, D))

    in_maps = []
    for g in range(B):
        Hg8 = H[g].astype(fp8)
        HgT8 = np.ascontiguousarray(H[g].T).astype(fp8)
        dege_full = H[g].sum(axis=0, dtype=np.float32)
        xv = _tile128(x[g]).astype(bf16)
        DvT = (Dv[g].T * DSCALE).astype(fp8d)
        DeT = (De[g].T * DSCALE).astype(fp8d)
        for c in range(2):
            lo, hi = c * HALF, (c + 1) * HALF
            # hrow chunk-major: [128, 2*HT*HALF], chunk c2 at t*HALF+e
            hrow_g = Hg8[lo:hi, :].reshape(HT, 128, 2, HALF)
            hrow_t = np.ascontiguousarray(
                hrow_g.transpose(1, 2, 0, 3).reshape(128, 2 * HT * HALF))
            in_maps.append({
                "xv": xv,
                "hcol": _tile128(np.ascontiguousarray(Hg8[:, lo:hi])),
                "htr": np.ascontiguousarray(HgT8[lo:hi, :]),
                "hrow": hrow_t,
                "dvT": _tile128(np.ascontiguousarray(DvT[:, lo:hi])),
                "deT": _tile128(np.ascontiguousarray(DeT[:, lo:hi])),
                "dege": np.ascontiguousarray(
                    dege_full[lo:hi].reshape(1, HALF)),
                "whi": whi, "th2": th2, "brow": brow,
                "mask": np.ascontiguousarray(
                    em[g, lo:hi].astype(np.float32).reshape(1, HALF)),
                "eps": eps,
                "bng": bng, "bnb": bnb, "bnm": bnm, "bnv": bnv,
            })
    return in_maps


def kernel(**inputs):
    from concourse.bass_utils import run_bass_kernel_spmd

    nc = _get_nc()
    in_maps = _shard(inputs)
    res = run_bass_kernel_spmd(nc, in_maps, list(range(NCORES)))
    out = np.empty((B, N, D), dtype=np.float32)
    for g in range(B):
        ya = res.results[2 * g]["y"]
        yb = res.results[2 * g + 1]["y"]
        out[g, :HALF, :] = ya[:, :HALF].T
        out[g, HALF:, :] = yb[:, HALF:].T
    return out


# revision 19
# speedup vs baseline: 2.4239x; 1.1648x over previous
"""HGNN layer (hypergraph message passing) Trainium2 kernel, 8 NeuronCores.

Sharding: one graph per PAIR of cores; within a pair each core owns half the
hyperedge (Ec) / node (Nc) range. Host pre-casts the big matrices: the 0/1
incident matrix H ships as fp8e4 (exact) in the three layouts the PE needs;
Dv/De ship as fp8e3 scaled by 64 (descale folded into later evacuations); x
ships bf16 in block-transposed stationary layout. Streams use host-tiled
[128, k*HALF] layouts so slab DMAs move 1 MB at a time on the ACT HWDGE
queue. Dataflow computes hxx = H^T x first, then hx = hxx W + b (x) dege
(bias as a rank-1 PE accumulate against a host-computed edge-degree row);
attention scores use a hi/lo bf16 split of hxx and theta. Softmax is
unnormalized; z rides the first AllReduce as a bf16 hi/lo pair; 1/z and the
Dv descale fold into the h1c evacuation. htr stays SBUF-resident for its two
uses. Each of the 3 pair-AllReduces is split into two half-width bf16
collectives whose payloads are pre-transposed into the consumer's layout, so
the second half overlaps the first half's consumers and there is zero
post-AR rearrangement."""

import numpy as np

B, N, E, D = 4, 4096, 4096, 128
HALF = N // 2
NCORES = 8
PAIRS = [[0, 1], [2, 3], [4, 5], [6, 7]]
BN_EPS = 1e-5
F = 512                 # moving free-dim per matmul
NT = N // 128           # 32 k-tiles over a full 4096 dim
HT = HALF // 128        # 16 k-tiles over a half
SLAB = 4                # k-tiles per stream DMA (1 MB slabs)
DSCALE = 64.0           # host-side scale on Dv/De before fp8e3 cast
ZPAD = 16               # extra bf16 cols on the first AR chunk for z hi/lo

_CACHE = {}


def _build():
    import concourse.bacc as bacc
    import concourse.mybir as mybir
    import concourse.tile as tile
    from concourse.masks import make_identity
    from contextlib import ExitStack

    fp32 = mybir.dt.float32
    bf16 = mybir.dt.bfloat16
    fp8 = mybir.dt.float8e4
    fp8d = mybir.dt.float8e3
    Act = mybir.ActivationFunctionType
    Alu = mybir.AluOpType

    nc = bacc.Bacc("TRN2", target_bir_lowering=False, debug=False,
                   num_devices=NCORES)

    xv_d = nc.dram_tensor("xv", [128, N], bf16, kind="ExternalInput")
    hcol_d = nc.dram_tensor("hcol", [128, NT * HALF], fp8, kind="ExternalInput")
    htr_d = nc.dram_tensor("htr", [HALF, N], fp8, kind="ExternalInput")
    hrow_d = nc.dram_tensor("hrow", [128, 2 * HT * HALF], fp8,
                            kind="ExternalInput")
    dvT_d = nc.dram_tensor("dvT", [128, NT * HALF], fp8d, kind="ExternalInput")
    deT_d = nc.dram_tensor("deT", [128, NT * HALF], fp8d, kind="ExternalInput")
    dege_d = nc.dram_tensor("dege", [1, HALF], fp32, kind="ExternalInput")
    whi_d = nc.dram_tensor("whi", [D, D], bf16, kind="ExternalInput")
    th2_d = nc.dram_tensor("th2", [D, 2], bf16, kind="ExternalInput")
    brow_d = nc.dram_tensor("brow", [1, D], fp32, kind="ExternalInput")
    mask_d = nc.dram_tensor("mask", [1, HALF], fp32, kind="ExternalInput")
    eps_d = nc.dram_tensor("eps", [D, 1], fp32, kind="ExternalInput")
    bng_d = nc.dram_tensor("bng", [D, 1], fp32, kind="ExternalInput")
    bnb_d = nc.dram_tensor("bnb", [D, 1], fp32, kind="ExternalInput")
    bnm_d = nc.dram_tensor("bnm", [D, 1], fp32, kind="ExternalInput")
    bnv_d = nc.dram_tensor("bnv", [D, 1], fp32, kind="ExternalInput")
    y_d = nc.dram_tensor("y", [D, HALF], fp32, kind="ExternalOutput")

    with tile.TileContext(nc) as tc, ExitStack() as ctx:
        const = ctx.enter_context(tc.tile_pool(name="const", bufs=1))
        resident = ctx.enter_context(tc.tile_pool(name="resident", bufs=1))
        stream = ctx.enter_context(tc.tile_pool(name="stream", bufs=3))
        med = ctx.enter_context(tc.tile_pool(name="med", bufs=1))
        small = ctx.enter_context(tc.tile_pool(name="small", bufs=1))
        ps = ctx.enter_context(tc.tile_pool(name="ps", bufs=8, space="PSUM"))
        dram = ctx.enter_context(tc.tile_pool(name="dram", bufs=1, space="DRAM"))

        ident = const.tile([128, 128], fp32)
        make_identity(nc, ident)
        one11 = const.tile([1, 1], fp32)
        nc.vector.memset(one11[:], 1.0)
        ones_row = const.tile([1, 128], fp32)
        nc.vector.memset(ones_row[:], 1.0)
        ones2 = const.tile([2, 1], fp32)
        nc.vector.memset(ones2[:], 1.0)
        c64 = const.tile([128, 1], fp32)
        nc.vector.memset(c64[:], 1.0 / DSCALE)

        xv = const.tile([128, N], bf16)
        nc.sync.dma_start(out=xv[:], in_=xv_d.ap())

        def load_param(dt_):
            t = const.tile([D, 1], fp32, tag=dt_.name + "_p")
            nc.sync.dma_start(out=t[:], in_=dt_.ap())
            return t

        whi_t = const.tile([D, D], bf16)
        nc.sync.dma_start(out=whi_t[:], in_=whi_d.ap())
        th2_t = const.tile([D, 2], bf16)
        nc.sync.dma_start(out=th2_t[:], in_=th2_d.ap())
        brow_t = const.tile([1, D], fp32)
        nc.sync.dma_start(out=brow_t[:], in_=brow_d.ap())
        dege_t = const.tile([1, HALF], fp32)
        nc.sync.dma_start(out=dege_t[:], in_=dege_d.ap())
        eps_t = load_param(eps_d)
        bng_t = load_param(bng_d)
        bnb_t = load_param(bnb_d)
        bnm_t = load_param(bnm_d)
        bnv_t = load_param(bnv_d)
        mask_t = const.tile([1, HALF], fp32)
        nc.sync.dma_start(out=mask_t[:], in_=mask_d.ap())

        # htr resident fp8 (sync queue; hcol stream rides the ACT queue)
        htr_res = resident.tile([128, HT * N], fp8)
        for t in range(HT):
            nc.sync.dma_start(out=htr_res[:, t * N:(t + 1) * N],
                              in_=htr_d.ap()[t * 128:(t + 1) * 128, :])

        def slab_stream(dram_t, dt, n_tiles, name):
            """Yield (k_tile_index, moving_tile_fn) streaming 1MB slabs."""
            for s in range(n_tiles // SLAB):
                sb = stream.tile([128, SLAB * HALF], dt, tag="slab",
                                 name=name)
                nc.scalar.dma_start(
                    out=sb[:],
                    in_=dram_t.ap()[:, s * SLAB * HALF:(s + 1) * SLAB * HALF])
                for jj in range(SLAB):
                    j = s * SLAB + jj
                    yield j, sb[:, jj * HALF:(jj + 1) * HALF]

        # ---- S2: hxxT [D, HALF] = (H[:,Ec]^T x)^T ------------------------
        hxx_ps = [ps.tile([128, F], fp32, tag="ps", name=f"hxx{i}")
                  for i in range(HALF // F)]
        for j, hj in slab_stream(hcol_d, fp8, NT, "hj"):
            for blk in range(HALF // F):
                nc.tensor.matmul(hxx_ps[blk][:],
                                 xv[:, j * D:(j + 1) * D],
                                 hj[:, blk * F:(blk + 1) * F],
                                 start=(j == 0), stop=(j == NT - 1))
        hxxT = med.tile([D, HALF], fp32, tag="hxxT")
        hxx_hi = med.tile([D, HALF], bf16, tag="hxx_hi")
        hxx_lo = med.tile([D, HALF], bf16, tag="hxx_lo")
        tmp32 = med.tile([D, HALF], fp32, tag="ehxT")
        for blk in range(HALF // F):
            sl = slice(blk * F, (blk + 1) * F)
            nc.vector.tensor_copy(hxxT[:, sl], hxx_ps[blk][:])
        nc.vector.tensor_copy(hxx_hi[:], hxxT[:])
        nc.vector.tensor_copy(tmp32[:], hxx_hi[:])
        nc.vector.tensor_tensor(tmp32[:], hxxT[:], tmp32[:], op=Alu.subtract)
        nc.vector.tensor_copy(hxx_lo[:], tmp32[:])

        # ---- S3: hxT = W^T hxx + b (x) dege ; st = th^T hxx --------------
        hxT = med.tile([D, HALF], fp32, tag="hxT")
        st_sb = small.tile([1, HALF], fp32, tag="st_sb")
        for blk in range(HALF // F):
            sl = slice(blk * F, (blk + 1) * F)
            hx2 = ps.tile([128, F], fp32, tag="ps", name=f"hx2_{blk}")
            nc.tensor.matmul(hx2[:], whi_t[:], hxx_hi[:, sl],
                             start=True, stop=False)
            nc.tensor.matmul(hx2[:], whi_t[:], hxx_lo[:, sl],
                             start=False, stop=False)
            nc.tensor.matmul(hx2[:], brow_t[:], dege_t[:, sl],
                             start=False, stop=True)
            nc.vector.tensor_copy(hxT[:, sl], hx2[:])
            st2 = ps.tile([2, F], fp32, tag="ps", name=f"st2_{blk}")
            nc.tensor.matmul(st2[:], th2_t[:], hxx_hi[:, sl],
                             start=True, stop=False)
            nc.tensor.matmul(st2[:], th2_t[:], hxx_lo[:, sl],
                             start=False, stop=True)
            s2sb = med.tile([2, F], fp32, tag="s2sb", name=f"s2sb{blk}")
            nc.vector.tensor_copy(s2sb[:], st2[0:2, :])
            sp = ps.tile([1, F], fp32, tag="ps", name=f"sp{blk}")
            nc.tensor.matmul(sp[:], ones2[:], s2sb[:], start=True, stop=True)
            nc.vector.tensor_copy(st_sb[:, sl], sp[:])

        # ---- S4: softmax pieces (in-place on st_sb) ----------------------
        attn_u = st_sb
        nc.scalar.activation(attn_u[:], st_sb[:], Act.Exp)
        nc.vector.tensor_mul(attn_u[:], attn_u[:], mask_t[:])
        z_t = small.tile([1, 1], fp32, tag="z_t")
        nc.vector.reduce_sum(z_t[:], attn_u[:], axis=mybir.AxisListType.X)
        # z hi/lo bf16 pieces
        zhi = small.tile([1, 1], bf16, tag="zhi")
        zlo = small.tile([1, 1], bf16, tag="zlo")
        zf = small.tile([1, 1], fp32, tag="zf")
        nc.vector.tensor_copy(zhi[:], z_t[:])
        nc.vector.tensor_copy(zf[:], zhi[:])
        nc.vector.tensor_tensor(zf[:], z_t[:], zf[:], op=Alu.subtract)
        nc.vector.tensor_copy(zlo[:], zf[:])
        attnv = med.tile([128, HT], fp32, tag="attnv")
        for t in range(HT):
            pt = ps.tile([128, 1], fp32, tag="ps")
            nc.tensor.matmul(pt[:], attn_u[:, t * 128:(t + 1) * 128], one11[:],
                             start=True, stop=True)
            nc.vector.tensor_copy(attnv[:, t:t + 1], pt[:])
        ehxT = med.tile([D, HALF], fp32, tag="ehxT")
        nc.vector.tensor_scalar_mul(ehxT[:], hxT[:], eps_t[:])

        # ---- S5: h1av [128, HT*D] bf16 = attn * hx (e-part tiles) --------
        h1av = med.tile([128, HALF], bf16, tag="h1av")
        for t in range(HT):
            pt = ps.tile([128, 128], fp32, tag="ps")
            nc.tensor.transpose(pt[:], hxT[:, t * 128:(t + 1) * 128], ident[:])
            nc.vector.tensor_scalar_mul(h1av[:, t * 128:(t + 1) * 128], pt[:],
                                        attnv[:, t:t + 1])

        def chunked_bmm_ag_v(stationary, moving_of, tagbase, with_z=False):
            """Two half-width partial bmms; payload pre-transposed to the
            consumer's v-layout, cast bf16, AllGathered per chunk with a
            local DVE add of the two rank blocks (cheaper than ncfw
            AllReduce). Returns (resA, resB) bf16 [128, HALF(+ZPAD)]."""
            outs = []
            for chunk in range(2):
                w = HALF + ZPAD if (with_z and chunk == 0) else HALF
                pss = [ps.tile([128, F], fp32, tag="ps",
                               name=f"{tagbase}_{chunk}_{i}")
                       for i in range(HALF // F)]
                for t in range(HT):
                    for blk in range(HALF // F):
                        nc.tensor.matmul(
                            pss[blk][:],
                            stationary[:, t * 128:(t + 1) * 128],
                            moving_of(t, chunk * HALF + blk * F, F),
                            start=(t == 0), stop=(t == HT - 1))
                ccT = med.tile([D, HALF], fp32, tag="ccsbT")
                for blk in range(HALF // F):
                    sl = slice(blk * F, (blk + 1) * F)
                    nc.vector.tensor_copy(ccT[:, sl], pss[blk][:])
                ccv = med.tile([128, HALF + ZPAD], bf16,
                               tag=f"ccv{chunk}")
                for t in range(HT):
                    pt = ps.tile([128, 128], fp32, tag="ps")
                    nc.tensor.transpose(pt[:], ccT[:, t * 128:(t + 1) * 128],
                                        ident[:])
                    nc.vector.tensor_copy(ccv[:, t * 128:(t + 1) * 128], pt[:])
                if with_z and chunk == 0:
                    nc.vector.memset(ccv[:, HALF:], 0.0)
                    nc.vector.tensor_copy(ccv[0:1, HALF:HALF + 1], zhi[:])
                    nc.vector.tensor_copy(ccv[0:1, HALF + 1:HALF + 2], zlo[:])
                cc_in = dram.tile([128, w], bf16, tag=f"{tagbase}i{chunk}")
                cc_out = dram.tile([256, w], bf16, tag=f"{tagbase}o{chunk}")
                nc.sync.dma_start(out=cc_in[:], in_=ccv[:, 0:w])
                nc.gpsimd.collective_compute(
                    "AllGather", Alu.bypass, replica_groups=PAIRS,
                    ins=[cc_in.opt()], outs=[cc_out.opt()])
                res = med.tile([128, HALF + ZPAD], bf16,
                               tag=f"resv{chunk}")
                agt = med.tile([128, HALF + ZPAD], bf16, tag="agtmp")
                nc.sync.dma_start(out=res[:, 0:w], in_=cc_out[0:128, :])
                nc.sync.dma_start(out=agt[:, 0:w], in_=cc_out[128:256, :])
                nc.vector.tensor_tensor(res[:, 0:w], res[:, 0:w],
                                        agt[:, 0:w], op=Alu.add)
                outs.append(res)
            return outs

        # ---- S6: h1b = H h1a (partial over Ec), v-layout chunked AG ------
        h1bA, h1bB = chunked_bmm_ag_v(
            h1av,
            lambda t, lo, w: htr_res[:, t * N + lo:t * N + lo + w],
            "cc1", with_z=True)

        # rz = 1/(z), folded with 1/DSCALE, broadcast to [128, 1]
        rz = small.tile([1, 1], fp32, tag="rz")
        zs = small.tile([1, 1], fp32, tag="zs")
        nc.vector.tensor_copy(rz[:], h1bA[0:1, HALF:HALF + 1])
        nc.vector.tensor_copy(zs[:], h1bA[0:1, HALF + 1:HALF + 2])
        nc.vector.tensor_tensor(rz[:], rz[:], zs[:], op=Alu.add)
        nc.vector.reciprocal(rz[:], rz[:])
        rz_ps = ps.tile([128, 1], fp32, tag="ps")
        nc.tensor.matmul(rz_ps[:], ones_row[:], rz[:], start=True, stop=True)
        rz_bc = small.tile([128, 1], fp32, tag="rz_bc")
        nc.vector.tensor_copy(rz_bc[:], rz_ps[:])
        nc.vector.tensor_mul(rz_bc[:], rz_bc[:], c64[:])

        def vtile(resA, resB, j):
            src = resA if j < HT else resB
            jj = j % HT
            return src[:, jj * 128:(jj + 1) * 128]

        # ---- S7: h1cT = (Dv[Nc,:] h1b)^T * rz/DSCALE ---------------------
        h1c_ps = [ps.tile([128, F], fp32, tag="ps", name=f"h1c{i}")
                  for i in range(HALF // F)]
        for j, dj in slab_stream(dvT_d, fp8d, NT, "dj"):
            for blk in range(HALF // F):
                nc.tensor.matmul(h1c_ps[blk][:], vtile(h1bA, h1bB, j),
                                 dj[:, blk * F:(blk + 1) * F],
                                 start=(j == 0), stop=(j == NT - 1))
        h1cT = med.tile([D, HALF], fp32, tag="hxxT")
        for blk in range(HALF // F):
            sl = slice(blk * F, (blk + 1) * F)
            nc.vector.tensor_scalar_mul(h1cT[:, sl], h1c_ps[blk][:], rz_bc[:])
        h1cv = med.tile([128, HALF], bf16, tag="h1cv")
        for t in range(HT):
            pt = ps.tile([128, 128], fp32, tag="ps")
            nc.tensor.transpose(pt[:], h1cT[:, t * 128:(t + 1) * 128], ident[:])
            nc.vector.tensor_copy(h1cv[:, t * 128:(t + 1) * 128], pt[:])

        # ---- S8: h1d = H[Nc,:]^T h1c (partial over Nc), chunked AR -------
        hrow_state = {}

        def hrow_moving(t, lo, w):
            chunk = lo // HALF
            slab_i = t // SLAB
            key = (chunk, slab_i)
            if key not in hrow_state:
                sb = stream.tile([128, SLAB * HALF], fp8, tag="slab",
                                 name=f"rj{chunk}")
                base = chunk * HT * HALF + slab_i * SLAB * HALF
                nc.scalar.dma_start(
                    out=sb[:], in_=hrow_d.ap()[:, base:base + SLAB * HALF])
                hrow_state[key] = sb
            return hrow_state[key][:, (t % SLAB) * HALF + (lo % HALF):
                                   (t % SLAB) * HALF + (lo % HALF) + w]

        h1dA, h1dB = chunked_bmm_ag_v(h1cv, hrow_moving, "cc2")

        # ---- S9+S10: hT = (De[Ec,:] h1d)^T / DSCALE + eps*hx -------------
        h1e_ps = [ps.tile([128, F], fp32, tag="ps", name=f"h1e{i}")
                  for i in range(HALF // F)]
        for j, ej in slab_stream(deT_d, fp8d, NT, "ej"):
            for blk in range(HALF // F):
                nc.tensor.matmul(h1e_ps[blk][:], vtile(h1dA, h1dB, j),
                                 ej[:, blk * F:(blk + 1) * F],
                                 start=(j == 0), stop=(j == NT - 1))
        hT = med.tile([D, HALF], fp32, tag="hxT")
        for blk in range(HALF // F):
            sl = slice(blk * F, (blk + 1) * F)
            nc.vector.scalar_tensor_tensor(hT[:, sl], h1e_ps[blk][:],
                                           1.0 / DSCALE, ehxT[:, sl],
                                           op0=Alu.mult, op1=Alu.add)
        hv = med.tile([128, HALF], bf16, tag="h1av")
        for t in range(HT):
            pt = ps.tile([128, 128], fp32, tag="ps")
            nc.tensor.transpose(pt[:], hT[:, t * 128:(t + 1) * 128], ident[:])
            nc.vector.tensor_copy(hv[:, t * 128:(t + 1) * 128], pt[:])

        # ---- S11: out = H h (partial over Ec), bf16 T-layout chunked AR --
        s_bn = small.tile([D, 1], fp32, tag="s_bn")
        nc.vector.tensor_scalar_add(s_bn[:], bnv_t[:], BN_EPS)
        nc.scalar.activation(s_bn[:], s_bn[:], Act.Sqrt)
        nc.vector.reciprocal(s_bn[:], s_bn[:])
        nc.vector.tensor_mul(s_bn[:], s_bn[:], bng_t[:])
        t_bn = small.tile([D, 1], fp32, tag="t_bn")
        nc.vector.tensor_mul(t_bn[:], bnm_t[:], s_bn[:])
        nc.vector.tensor_tensor(t_bn[:], bnb_t[:], t_bn[:], op=Alu.subtract)

        # one ReduceScatter: core even gets summed cols 0:HALF, odd the rest
        cc3_in = dram.tile([256, HALF], bf16, tag="cc3i")
        cc3_out = dram.tile([128, HALF], bf16, tag="cc3o")
        for chunk in range(2):
            pss = [ps.tile([128, F], fp32, tag="ps", name=f"out_{chunk}_{i}")
                   for i in range(HALF // F)]
            for t in range(HT):
                for blk in range(HALF // F):
                    lo = chunk * HALF + blk * F
                    nc.tensor.matmul(pss[blk][:],
                                     hv[:, t * 128:(t + 1) * 128],
                                     htr_res[:, t * N + lo:t * N + lo + F],
                                     start=(t == 0), stop=(t == HT - 1))
            ccv = med.tile([128, HALF + ZPAD], bf16, tag=f"ccv{chunk}")
            for blk in range(HALF // F):
                sl = slice(blk * F, (blk + 1) * F)
                nc.vector.tensor_copy(ccv[:, sl], pss[blk][:])
            nc.sync.dma_start(out=cc3_in[chunk * 128:(chunk + 1) * 128, :],
                              in_=ccv[:, 0:HALF])
        nc.gpsimd.collective_compute(
            "ReduceScatter", Alu.add, replica_groups=PAIRS,
            ins=[cc3_in.opt()], outs=[cc3_out.opt()])
        res3 = med.tile([128, HALF + ZPAD], bf16, tag="resv0")
        nc.sync.dma_start(out=res3[:, 0:HALF], in_=cc3_out[:])
        of = med.tile([D, HALF], fp32, tag="ccsbT")
        nc.scalar.activation(of[:], res3[:, 0:HALF], Act.Lrelu, alpha=0.01)
        nc.vector.tensor_scalar(of[:], of[:], s_bn[:], t_bn[:],
                                op0=Alu.mult, op1=Alu.add)
        nc.sync.dma_start(out=y_d.ap(), in_=of[:])

    nc.finalize()
    return nc


def _get_nc():
    if "nc" not in _CACHE:
        _CACHE["nc"] = _build()
    return _CACHE["nc"]


def _tile128(a):
    """[K*128, W] -> [128, K*W] block-transposed stream layout."""
    K = a.shape[0] // 128
    return np.ascontiguousarray(
        a.reshape(K, 128, a.shape[1]).transpose(1, 0, 2).reshape(
            128, K * a.shape[1]))


def _shard(inputs):
    import ml_dtypes
    bf16 = ml_dtypes.bfloat16
    fp8 = ml_dtypes.float8_e4m3
    fp8d = ml_dtypes.float8_e3m4

    H = np.asarray(inputs["incident_mat"], dtype=np.float32)
    Dv = np.asarray(inputs["degree_v"], dtype=np.float32)
    De = np.asarray(inputs["degree_e"], dtype=np.float32)
    x = np.asarray(inputs["x"], dtype=np.float32)
    em = np.asarray(inputs["e_masks"])
    w = np.asarray(inputs["mlp_W"], dtype=np.float32)
    b = np.asarray(inputs["mlp_b"], dtype=np.float32)
    th = np.asarray(inputs["theta_att"], dtype=np.float32).reshape(D)
    eps = np.full((D, 1), float(np.asarray(inputs["eps"]).reshape(-1)[0]),
                  dtype=np.float32)

    def col(v):
        return np.ascontiguousarray(
            np.asarray(v, dtype=np.float32).reshape(D, 1))

    bng, bnb = col(inputs["bn_gamma"]), col(inputs["bn_beta"])
    bnm, bnv = col(inputs["bn_mean"]), col(inputs["bn_var"])

    whi = w.astype(bf16)
    th_hi = th.astype(bf16)
    th_lo = (th - th_hi.astype(np.float32)).astype(bf16)
    th2 = np.ascontiguousarray(np.stack(
        [th_hi.astype(np.float32), th_lo.astype(np.float32)], axis=1)
    ).astype(bf16)
    brow = np.ascontiguousarray(b.reshape(1, D))

    in_maps = []
    for g in range(B):
        Hg8 = H[g].astype(fp8)
        HgT8 = np.ascontiguousarray(H[g].T).astype(fp8)
        dege_full = H[g].sum(axis=0, dtype=np.float32)
        xv = _tile128(x[g]).astype(bf16)
        DvT = (Dv[g].T * DSCALE).astype(fp8d)
        DeT = (De[g].T * DSCALE).astype(fp8d)
        for c in range(2):
            lo, hi = c * HALF, (c + 1) * HALF
            # hrow chunk-major: [128, 2*HT*HALF], chunk c2 at t*HALF+e
            hrow_g = Hg8[lo:hi, :].reshape(HT, 128, 2, HALF)
            hrow_t = np.ascontiguousarray(
                hrow_g.transpose(1, 2, 0, 3).reshape(128, 2 * HT * HALF))
            in_maps.append({
                "xv": xv,
                "hcol": _tile128(np.ascontiguousarray(Hg8[:, lo:hi])),
                "htr": np.ascontiguousarray(HgT8[lo:hi, :]),
                "hrow": hrow_t,
                "dvT": _tile128(np.ascontiguousarray(DvT[:, lo:hi])),
                "deT": _tile128(np.ascontiguousarray(DeT[:, lo:hi])),
                "dege": np.ascontiguousarray(
                    dege_full[lo:hi].reshape(1, HALF)),
                "whi": whi, "th2": th2, "brow": brow,
                "mask": np.ascontiguousarray(
                    em[g, lo:hi].astype(np.float32).reshape(1, HALF)),
                "eps": eps,
                "bng": bng, "bnb": bnb, "bnm": bnm, "bnv": bnv,
            })
    return in_maps


def kernel(**inputs):
    from concourse.bass_utils import run_bass_kernel_spmd

    nc = _get_nc()
    in_maps = _shard(inputs)
    res = run_bass_kernel_spmd(nc, in_maps, list(range(NCORES)))
    out = np.empty((B, N, D), dtype=np.float32)
    for g in range(B):
        out[g, :HALF, :] = res.results[2 * g]["y"].T
        out[g, HALF:, :] = res.results[2 * g + 1]["y"].T
    return out


# revision 22
# speedup vs baseline: 2.4826x; 1.0242x over previous
"""HGNN layer (hypergraph message passing) Trainium2 kernel, 8 NeuronCores.

Sharding: one graph per PAIR of cores; within a pair each core owns half the
hyperedge (Ec) / node (Nc) range. Host pre-casts the big matrices: the 0/1
incident matrix H ships as fp8e4 (exact) in the three layouts the PE needs;
Dv/De ship as fp8e3 scaled by 64 (descale folded into later evacuations); x
ships bf16 in block-transposed stationary layout. Streams use host-tiled
[128, k*HALF] layouts so slab DMAs move 1 MB at a time on the ACT HWDGE
queue. Dataflow computes hxx = H^T x first, then hx = hxx W + b (x) dege
(bias as a rank-1 PE accumulate against a host-computed edge-degree row);
attention scores use a hi/lo bf16 split of hxx and theta. Softmax is
unnormalized; z rides the first AllReduce as a bf16 hi/lo pair; 1/z and the
Dv descale fold into the h1c evacuation. htr stays SBUF-resident for its two
uses. Each of the 3 pair-AllReduces is split into two half-width bf16
collectives whose payloads are pre-transposed into the consumer's layout, so
the second half overlaps the first half's consumers and there is zero
post-AR rearrangement."""

import numpy as np

B, N, E, D = 4, 4096, 4096, 128
HALF = N // 2
NCORES = 8
PAIRS = [[0, 1], [2, 3], [4, 5], [6, 7]]
BN_EPS = 1e-5
F = 512                 # moving free-dim per matmul
NT = N // 128           # 32 k-tiles over a full 4096 dim
HT = HALF // 128        # 16 k-tiles over a half
SLAB = 4                # k-tiles per stream DMA (1 MB slabs)
DSCALE = 64.0           # host-side scale on Dv/De before fp8e3 cast
ZPAD = 16               # extra bf16 cols on the first AR chunk for z hi/lo

_CACHE = {}


def _build():
    import concourse.bacc as bacc
    import concourse.mybir as mybir
    import concourse.tile as tile
    from concourse.masks import make_identity
    from contextlib import ExitStack

    fp32 = mybir.dt.float32
    bf16 = mybir.dt.bfloat16
    fp8 = mybir.dt.float8e4
    fp8d = mybir.dt.float8e3
    Act = mybir.ActivationFunctionType
    Alu = mybir.AluOpType

    nc = bacc.Bacc("TRN2", target_bir_lowering=False, debug=False,
                   num_devices=NCORES)

    xv_d = nc.dram_tensor("xv", [128, N], bf16, kind="ExternalInput")
    hcol_d = nc.dram_tensor("hcol", [128, NT * HALF], fp8, kind="ExternalInput")
    htr_d = nc.dram_tensor("htr", [HALF, N], fp8, kind="ExternalInput")
    hrow_d = nc.dram_tensor("hrow", [128, 2 * HT * HALF], fp8,
                            kind="ExternalInput")
    dvT_d = nc.dram_tensor("dvT", [128, NT * HALF], fp8d, kind="ExternalInput")
    deT_d = nc.dram_tensor("deT", [128, NT * HALF], fp8d, kind="ExternalInput")
    dege_d = nc.dram_tensor("dege", [1, HALF], fp32, kind="ExternalInput")
    whi_d = nc.dram_tensor("whi", [D, D], bf16, kind="ExternalInput")
    th2_d = nc.dram_tensor("th2", [D, 2], bf16, kind="ExternalInput")
    brow_d = nc.dram_tensor("brow", [1, D], fp32, kind="ExternalInput")
    mask_d = nc.dram_tensor("mask", [1, HALF], fp32, kind="ExternalInput")
    eps_d = nc.dram_tensor("eps", [D, 1], fp32, kind="ExternalInput")
    bng_d = nc.dram_tensor("bng", [D, 1], fp32, kind="ExternalInput")
    bnb_d = nc.dram_tensor("bnb", [D, 1], fp32, kind="ExternalInput")
    bnm_d = nc.dram_tensor("bnm", [D, 1], fp32, kind="ExternalInput")
    bnv_d = nc.dram_tensor("bnv", [D, 1], fp32, kind="ExternalInput")
    y_d = nc.dram_tensor("y", [D, HALF], fp32, kind="ExternalOutput")

    with tile.TileContext(nc) as tc, ExitStack() as ctx:
        const = ctx.enter_context(tc.tile_pool(name="const", bufs=1))
        resident = ctx.enter_context(tc.tile_pool(name="resident", bufs=1))
        stream = ctx.enter_context(tc.tile_pool(name="stream", bufs=3))
        med = ctx.enter_context(tc.tile_pool(name="med", bufs=1))
        small = ctx.enter_context(tc.tile_pool(name="small", bufs=1))
        ps = ctx.enter_context(tc.tile_pool(name="ps", bufs=8, space="PSUM"))
        dram = ctx.enter_context(tc.tile_pool(name="dram", bufs=1, space="DRAM"))

        ident = const.tile([128, 128], fp32)
        make_identity(nc, ident)
        one11 = const.tile([1, 1], fp32)
        nc.vector.memset(one11[:], 1.0)
        ones_row = const.tile([1, 128], fp32)
        nc.vector.memset(ones_row[:], 1.0)
        ones2 = const.tile([2, 1], fp32)
        nc.vector.memset(ones2[:], 1.0)
        c64 = const.tile([128, 1], fp32)
        nc.vector.memset(c64[:], 1.0 / DSCALE)

        xv = const.tile([128, N], bf16)
        nc.sync.dma_start(out=xv[:], in_=xv_d.ap())

        def load_param(dt_):
            t = const.tile([D, 1], fp32, tag=dt_.name + "_p")
            nc.sync.dma_start(out=t[:], in_=dt_.ap())
            return t

        whi_t = const.tile([D, D], bf16)
        nc.sync.dma_start(out=whi_t[:], in_=whi_d.ap())
        th2_t = const.tile([D, 2], bf16)
        nc.sync.dma_start(out=th2_t[:], in_=th2_d.ap())
        brow_t = const.tile([1, D], fp32)
        nc.sync.dma_start(out=brow_t[:], in_=brow_d.ap())
        dege_t = const.tile([1, HALF], fp32)
        nc.sync.dma_start(out=dege_t[:], in_=dege_d.ap())
        eps_t = load_param(eps_d)
        bng_t = load_param(bng_d)
        bnb_t = load_param(bnb_d)
        bnm_t = load_param(bnm_d)
        bnv_t = load_param(bnv_d)
        mask_t = const.tile([1, HALF], fp32)
        nc.sync.dma_start(out=mask_t[:], in_=mask_d.ap())

        # dummy collective to absorb the first-collective warmup cost while
        # S2 streams (result unused)
        warm_in = dram.tile([1, 16], bf16, tag="warmi")
        warm_out = dram.tile([2, 16], bf16, tag="warmo")
        nc.gpsimd.collective_compute(
            "AllGather", Alu.bypass, replica_groups=PAIRS,
            ins=[warm_in.opt()], outs=[warm_out.opt()])

        # htr resident fp8 (sync queue; hcol stream rides the ACT queue).
        # Loads are interleaved into the S2 loop so hcol keeps priority.
        htr_res = resident.tile([128, HT * N], fp8)

        def slab_stream(dram_t, dt, n_tiles, name):
            """Yield (k_tile_index, moving_tile_fn) streaming 1MB slabs."""
            for s in range(n_tiles // SLAB):
                sb = stream.tile([128, SLAB * HALF], dt, tag="slab",
                                 name=name)
                nc.scalar.dma_start(
                    out=sb[:],
                    in_=dram_t.ap()[:, s * SLAB * HALF:(s + 1) * SLAB * HALF])
                for jj in range(SLAB):
                    j = s * SLAB + jj
                    yield j, sb[:, jj * HALF:(jj + 1) * HALF]

        # ---- S2: hxxT [D, HALF] = (H[:,Ec]^T x)^T ------------------------
        hxx_ps = [ps.tile([128, F], fp32, tag="ps", name=f"hxx{i}")
                  for i in range(HALF // F)]
        for j, hj in slab_stream(hcol_d, fp8, NT, "hj"):
            for blk in range(HALF // F):
                nc.tensor.matmul(hxx_ps[blk][:],
                                 xv[:, j * D:(j + 1) * D],
                                 hj[:, blk * F:(blk + 1) * F],
                                 start=(j == 0), stop=(j == NT - 1))
            if j % 2 == 1 and j // 2 < HT:
                t = j // 2
                nc.sync.dma_start(out=htr_res[:, t * N:(t + 1) * N],
                                  in_=htr_d.ap()[t * 128:(t + 1) * 128, :])
        hxxT = med.tile([D, HALF], fp32, tag="hxxT")
        hxx_hi = med.tile([D, HALF], bf16, tag="hxx_hi")
        hxx_lo = med.tile([D, HALF], bf16, tag="hxx_lo")
        tmp32 = med.tile([D, HALF], fp32, tag="ehxT")
        for blk in range(HALF // F):
            sl = slice(blk * F, (blk + 1) * F)
            nc.vector.tensor_copy(hxxT[:, sl], hxx_ps[blk][:])
        nc.vector.tensor_copy(hxx_hi[:], hxxT[:])
        nc.vector.tensor_copy(tmp32[:], hxx_hi[:])
        nc.vector.tensor_tensor(tmp32[:], hxxT[:], tmp32[:], op=Alu.subtract)
        nc.vector.tensor_copy(hxx_lo[:], tmp32[:])

        # ---- S3: hxT = W^T hxx + b (x) dege ; st = th^T hxx --------------
        hxT = med.tile([D, HALF], fp32, tag="hxT")
        st_sb = small.tile([1, HALF], fp32, tag="st_sb")
        for blk in range(HALF // F):
            sl = slice(blk * F, (blk + 1) * F)
            hx2 = ps.tile([128, F], fp32, tag="ps", name=f"hx2_{blk}")
            nc.tensor.matmul(hx2[:], whi_t[:], hxx_hi[:, sl],
                             start=True, stop=False)
            nc.tensor.matmul(hx2[:], whi_t[:], hxx_lo[:, sl],
                             start=False, stop=False)
            nc.tensor.matmul(hx2[:], brow_t[:], dege_t[:, sl],
                             start=False, stop=True)
            nc.vector.tensor_copy(hxT[:, sl], hx2[:])
            st2 = ps.tile([2, F], fp32, tag="ps", name=f"st2_{blk}")
            nc.tensor.matmul(st2[:], th2_t[:], hxx_hi[:, sl],
                             start=True, stop=False)
            nc.tensor.matmul(st2[:], th2_t[:], hxx_lo[:, sl],
                             start=False, stop=True)
            s2sb = med.tile([2, F], fp32, tag="s2sb", name=f"s2sb{blk}")
            nc.vector.tensor_copy(s2sb[:], st2[0:2, :])
            sp = ps.tile([1, F], fp32, tag="ps", name=f"sp{blk}")
            nc.tensor.matmul(sp[:], ones2[:], s2sb[:], start=True, stop=True)
            nc.vector.tensor_copy(st_sb[:, sl], sp[:])

        # ---- S4: softmax pieces (in-place on st_sb) ----------------------
        attn_u = st_sb
        nc.scalar.activation(attn_u[:], st_sb[:], Act.Exp)
        nc.vector.tensor_mul(attn_u[:], attn_u[:], mask_t[:])
        z_t = small.tile([1, 1], fp32, tag="z_t")
        nc.vector.reduce_sum(z_t[:], attn_u[:], axis=mybir.AxisListType.X)
        # z hi/lo bf16 pieces
        zhi = small.tile([1, 1], bf16, tag="zhi")
        zlo = small.tile([1, 1], bf16, tag="zlo")
        zf = small.tile([1, 1], fp32, tag="zf")
        nc.vector.tensor_copy(zhi[:], z_t[:])
        nc.vector.tensor_copy(zf[:], zhi[:])
        nc.vector.tensor_tensor(zf[:], z_t[:], zf[:], op=Alu.subtract)
        nc.vector.tensor_copy(zlo[:], zf[:])
        attnv = med.tile([128, HT], fp32, tag="attnv")
        for t in range(HT):
            pt = ps.tile([128, 1], fp32, tag="ps")
            nc.tensor.matmul(pt[:], attn_u[:, t * 128:(t + 1) * 128], one11[:],
                             start=True, stop=True)
            nc.vector.tensor_copy(attnv[:, t:t + 1], pt[:])
        ehxT = med.tile([D, HALF], fp32, tag="ehxT")
        nc.vector.tensor_scalar_mul(ehxT[:], hxT[:], eps_t[:])

        # ---- S5: h1av [128, HT*D] bf16 = attn * hx (e-part tiles) --------
        h1av = med.tile([128, HALF], bf16, tag="h1av")
        for t in range(HT):
            pt = ps.tile([128, 128], fp32, tag="ps")
            nc.tensor.transpose(pt[:], hxT[:, t * 128:(t + 1) * 128], ident[:])
            nc.vector.tensor_scalar_mul(h1av[:, t * 128:(t + 1) * 128], pt[:],
                                        attnv[:, t:t + 1])

        def chunked_bmm_ag_v(stationary, moving_of, tagbase, with_z=False):
            """Two half-width partial bmms; payload pre-transposed to the
            consumer's v-layout, cast bf16, AllGathered per chunk with a
            local DVE add of the two rank blocks (cheaper than ncfw
            AllReduce). Returns (resA, resB) bf16 [128, HALF(+ZPAD)]."""
            outs = []
            for chunk in range(2):
                w = HALF + ZPAD if (with_z and chunk == 0) else HALF
                pss = [ps.tile([128, F], fp32, tag="ps",
                               name=f"{tagbase}_{chunk}_{i}")
                       for i in range(HALF // F)]
                for t in range(HT):
                    for blk in range(HALF // F):
                        nc.tensor.matmul(
                            pss[blk][:],
                            stationary[:, t * 128:(t + 1) * 128],
                            moving_of(t, chunk * HALF + blk * F, F),
                            start=(t == 0), stop=(t == HT - 1))
                ccT = med.tile([D, HALF], fp32, tag="ccsbT")
                for blk in range(HALF // F):
                    sl = slice(blk * F, (blk + 1) * F)
                    nc.vector.tensor_copy(ccT[:, sl], pss[blk][:])
                ccv = med.tile([128, HALF + ZPAD], bf16,
                               tag=f"ccv{chunk}")
                for t in range(HT):
                    pt = ps.tile([128, 128], fp32, tag="ps")
                    nc.tensor.transpose(pt[:], ccT[:, t * 128:(t + 1) * 128],
                                        ident[:])
                    nc.vector.tensor_copy(ccv[:, t * 128:(t + 1) * 128], pt[:])
                if with_z and chunk == 0:
                    nc.vector.memset(ccv[:, HALF:], 0.0)
                    nc.vector.tensor_copy(ccv[0:1, HALF:HALF + 1], zhi[:])
                    nc.vector.tensor_copy(ccv[0:1, HALF + 1:HALF + 2], zlo[:])
                cc_in = dram.tile([128, w], bf16, tag=f"{tagbase}i{chunk}")
                cc_out = dram.tile([256, w], bf16, tag=f"{tagbase}o{chunk}")
                nc.sync.dma_start(out=cc_in[:], in_=ccv[:, 0:w])
                nc.gpsimd.collective_compute(
                    "AllGather", Alu.bypass, replica_groups=PAIRS,
                    ins=[cc_in.opt()], outs=[cc_out.opt()])
                res = med.tile([128, HALF + ZPAD], bf16,
                               tag=f"resv{chunk}")
                agt = med.tile([128, HALF + ZPAD], bf16, tag="agtmp")
                nc.sync.dma_start(out=res[:, 0:w], in_=cc_out[0:128, :])
                nc.sync.dma_start(out=agt[:, 0:w], in_=cc_out[128:256, :])
                nc.vector.tensor_tensor(res[:, 0:w], res[:, 0:w],
                                        agt[:, 0:w], op=Alu.add)
                outs.append(res)
            return outs

        # ---- S6: h1b = H h1a (partial over Ec), v-layout chunked AG ------
        h1bA, h1bB = chunked_bmm_ag_v(
            h1av,
            lambda t, lo, w: htr_res[:, t * N + lo:t * N + lo + w],
            "cc1", with_z=True)

        # rz = 1/(z), folded with 1/DSCALE, broadcast to [128, 1]
        rz = small.tile([1, 1], fp32, tag="rz")
        zs = small.tile([1, 1], fp32, tag="zs")
        nc.vector.tensor_copy(rz[:], h1bA[0:1, HALF:HALF + 1])
        nc.vector.tensor_copy(zs[:], h1bA[0:1, HALF + 1:HALF + 2])
        nc.vector.tensor_tensor(rz[:], rz[:], zs[:], op=Alu.add)
        nc.vector.reciprocal(rz[:], rz[:])
        rz_ps = ps.tile([128, 1], fp32, tag="ps")
        nc.tensor.matmul(rz_ps[:], ones_row[:], rz[:], start=True, stop=True)
        rz_bc = small.tile([128, 1], fp32, tag="rz_bc")
        nc.vector.tensor_copy(rz_bc[:], rz_ps[:])
        nc.vector.tensor_mul(rz_bc[:], rz_bc[:], c64[:])

        def vtile(resA, resB, j):
            src = resA if j < HT else resB
            jj = j % HT
            return src[:, jj * 128:(jj + 1) * 128]

        # ---- S7: h1cT = (Dv[Nc,:] h1b)^T * rz/DSCALE ---------------------
        h1c_ps = [ps.tile([128, F], fp32, tag="ps", name=f"h1c{i}")
                  for i in range(HALF // F)]
        for j, dj in slab_stream(dvT_d, fp8d, NT, "dj"):
            for blk in range(HALF // F):
                nc.tensor.matmul(h1c_ps[blk][:], vtile(h1bA, h1bB, j),
                                 dj[:, blk * F:(blk + 1) * F],
                                 start=(j == 0), stop=(j == NT - 1))
        h1cT = med.tile([D, HALF], fp32, tag="hxxT")
        for blk in range(HALF // F):
            sl = slice(blk * F, (blk + 1) * F)
            nc.vector.tensor_scalar_mul(h1cT[:, sl], h1c_ps[blk][:], rz_bc[:])
        h1cv = med.tile([128, HALF], bf16, tag="h1cv")
        for t in range(HT):
            pt = ps.tile([128, 128], fp32, tag="ps")
            nc.tensor.transpose(pt[:], h1cT[:, t * 128:(t + 1) * 128], ident[:])
            nc.vector.tensor_copy(h1cv[:, t * 128:(t + 1) * 128], pt[:])

        # ---- S8: h1d = H[Nc,:]^T h1c (partial over Nc), chunked AR -------
        hrow_state = {}

        def hrow_moving(t, lo, w):
            chunk = lo // HALF
            slab_i = t // SLAB
            key = (chunk, slab_i)
            if key not in hrow_state:
                sb = stream.tile([128, SLAB * HALF], fp8, tag="slab",
                                 name=f"rj{chunk}")
                base = chunk * HT * HALF + slab_i * SLAB * HALF
                nc.scalar.dma_start(
                    out=sb[:], in_=hrow_d.ap()[:, base:base + SLAB * HALF])
                hrow_state[key] = sb
            return hrow_state[key][:, (t % SLAB) * HALF + (lo % HALF):
                                   (t % SLAB) * HALF + (lo % HALF) + w]

        h1dA, h1dB = chunked_bmm_ag_v(h1cv, hrow_moving, "cc2")

        # ---- S9+S10: hT = (De[Ec,:] h1d)^T / DSCALE + eps*hx -------------
        h1e_ps = [ps.tile([128, F], fp32, tag="ps", name=f"h1e{i}")
                  for i in range(HALF // F)]
        for j, ej in slab_stream(deT_d, fp8d, NT, "ej"):
            for blk in range(HALF // F):
                nc.tensor.matmul(h1e_ps[blk][:], vtile(h1dA, h1dB, j),
                                 ej[:, blk * F:(blk + 1) * F],
                                 start=(j == 0), stop=(j == NT - 1))
        hT = med.tile([D, HALF], fp32, tag="hxT")
        for blk in range(HALF // F):
            sl = slice(blk * F, (blk + 1) * F)
            nc.vector.scalar_tensor_tensor(hT[:, sl], h1e_ps[blk][:],
                                           1.0 / DSCALE, ehxT[:, sl],
                                           op0=Alu.mult, op1=Alu.add)
        hv = med.tile([128, HALF], bf16, tag="h1av")
        for t in range(HT):
            pt = ps.tile([128, 128], fp32, tag="ps")
            nc.tensor.transpose(pt[:], hT[:, t * 128:(t + 1) * 128], ident[:])
            nc.vector.tensor_copy(hv[:, t * 128:(t + 1) * 128], pt[:])

        # ---- S11: out = H h (partial over Ec), bf16 T-layout chunked AR --
        s_bn = small.tile([D, 1], fp32, tag="s_bn")
        nc.vector.tensor_scalar_add(s_bn[:], bnv_t[:], BN_EPS)
        nc.scalar.activation(s_bn[:], s_bn[:], Act.Sqrt)
        nc.vector.reciprocal(s_bn[:], s_bn[:])
        nc.vector.tensor_mul(s_bn[:], s_bn[:], bng_t[:])
        t_bn = small.tile([D, 1], fp32, tag="t_bn")
        nc.vector.tensor_mul(t_bn[:], bnm_t[:], s_bn[:])
        nc.vector.tensor_tensor(t_bn[:], bnb_t[:], t_bn[:], op=Alu.subtract)

        # two half-width ReduceScatters: core even gets summed cols 0:HALF,
        # odd the rest; the first RS's epilogue overlaps the second RS.
        Q = HALF // 2
        cc3_in = [dram.tile([256, Q], bf16, tag=f"cc3i{h}", name=f"cc3i{h}")
                  for h in range(2)]
        cc3_out = [dram.tile([128, Q], bf16, tag=f"cc3o{h}", name=f"cc3o{h}")
                   for h in range(2)]
        for chunk in range(2):
            pss = [ps.tile([128, F], fp32, tag="ps", name=f"out_{chunk}_{i}")
                   for i in range(HALF // F)]
            for t in range(HT):
                for blk in range(HALF // F):
                    lo = chunk * HALF + blk * F
                    nc.tensor.matmul(pss[blk][:],
                                     hv[:, t * 128:(t + 1) * 128],
                                     htr_res[:, t * N + lo:t * N + lo + F],
                                     start=(t == 0), stop=(t == HT - 1))
            ccv = med.tile([128, HALF + ZPAD], bf16, tag=f"ccv{chunk}")
            for blk in range(HALF // F):
                sl = slice(blk * F, (blk + 1) * F)
                nc.vector.tensor_copy(ccv[:, sl], pss[blk][:])
            for h in range(2):
                nc.sync.dma_start(
                    out=cc3_in[h][chunk * 128:(chunk + 1) * 128, :],
                    in_=ccv[:, h * Q:(h + 1) * Q])
        for h in range(2):
            nc.gpsimd.collective_compute(
                "ReduceScatter", Alu.add, replica_groups=PAIRS,
                ins=[cc3_in[h].opt()], outs=[cc3_out[h].opt()])
        for h in range(2):
            res3 = med.tile([128, HALF + ZPAD], bf16,
                            tag=f"resv{h}")
            nc.sync.dma_start(out=res3[:, 0:Q], in_=cc3_out[h][:])
            of = med.tile([D, Q], fp32, tag=f"of{h}")
            nc.scalar.activation(of[:], res3[:, 0:Q], Act.Lrelu, alpha=0.01)
            nc.vector.tensor_scalar(of[:], of[:], s_bn[:], t_bn[:],
                                    op0=Alu.mult, op1=Alu.add)
            nc.sync.dma_start(out=y_d.ap()[:, h * Q:(h + 1) * Q], in_=of[:])

    nc.finalize()
    return nc


def _get_nc():
    if "nc" not in _CACHE:
        _CACHE["nc"] = _build()
    return _CACHE["nc"]


def _tile128(a):
    """[K*128, W] -> [128, K*W] block-transposed stream layout."""
    K = a.shape[0] // 128
    return np.ascontiguousarray(
        a.reshape(K, 128, a.shape[1]).transpose(1, 0, 2).reshape(
            128, K * a.shape[1]))


def _shard(inputs):
    import ml_dtypes
    bf16 = ml_dtypes.bfloat16
    fp8 = ml_dtypes.float8_e4m3
    fp8d = ml_dtypes.float8_e3m4

    H = np.asarray(inputs["incident_mat"], dtype=np.float32)
    Dv = np.asarray(inputs["degree_v"], dtype=np.float32)
    De = np.asarray(inputs["degree_e"], dtype=np.float32)
    x = np.asarray(inputs["x"], dtype=np.float32)
    em = np.asarray(inputs["e_masks"])
    w = np.asarray(inputs["mlp_W"], dtype=np.float32)
    b = np.asarray(inputs["mlp_b"], dtype=np.float32)
    th = np.asarray(inputs["theta_att"], dtype=np.float32).reshape(D)
    eps = np.full((D, 1), float(np.asarray(inputs["eps"]).reshape(-1)[0]),
                  dtype=np.float32)

    def col(v):
        return np.ascontiguousarray(
            np.asarray(v, dtype=np.float32).reshape(D, 1))

    bng, bnb = col(inputs["bn_gamma"]), col(inputs["bn_beta"])
    bnm, bnv = col(inputs["bn_mean"]), col(inputs["bn_var"])

    whi = w.astype(bf16)
    th_hi = th.astype(bf16)
    th_lo = (th - th_hi.astype(np.float32)).astype(bf16)
    th2 = np.ascontiguousarray(np.stack(
        [th_hi.astype(np.float32), th_lo.astype(np.float32)], axis=1)
    ).astype(bf16)
    brow = np.ascontiguousarray(b.reshape(1, D))

    in_maps = []
    for g in range(B):
        Hg8 = H[g].astype(fp8)
        HgT8 = np.ascontiguousarray(H[g].T).astype(fp8)
        dege_full = H[g].sum(axis=0, dtype=np.float32)
        xv = _tile128(x[g]).astype(bf16)
        DvT = (Dv[g].T * DSCALE).astype(fp8d)
        DeT = (De[g].T * DSCALE).astype(fp8d)
        for c in range(2):
            lo, hi = c * HALF, (c + 1) * HALF
            # hrow chunk-major: [128, 2*HT*HALF], chunk c2 at t*HALF+e
            hrow_g = Hg8[lo:hi, :].reshape(HT, 128, 2, HALF)
            hrow_t = np.ascontiguousarray(
                hrow_g.transpose(1, 2, 0, 3).reshape(128, 2 * HT * HALF))
            in_maps.append({
                "xv": xv,
                "hcol": _tile128(np.ascontiguousarray(Hg8[:, lo:hi])),
                "htr": np.ascontiguousarray(HgT8[lo:hi, :]),
                "hrow": hrow_t,
                "dvT": _tile128(np.ascontiguousarray(DvT[:, lo:hi])),
                "deT": _tile128(np.ascontiguousarray(DeT[:, lo:hi])),
                "dege": np.ascontiguousarray(
                    dege_full[lo:hi].reshape(1, HALF)),
                "whi": whi, "th2": th2, "brow": brow,
                "mask": np.ascontiguousarray(
                    em[g, lo:hi].astype(np.float32).reshape(1, HALF)),
                "eps": eps,
                "bng": bng, "bnb": bnb, "bnm": bnm, "bnv": bnv,
            })
    return in_maps


def kernel(**inputs):
    from concourse.bass_utils import run_bass_kernel_spmd

    nc = _get_nc()
    in_maps = _shard(inputs)
    res = run_bass_kernel_spmd(nc, in_maps, list(range(NCORES)))
    out = np.empty((B, N, D), dtype=np.float32)
    for g in range(B):
        out[g, :HALF, :] = res.results[2 * g]["y"].T
        out[g, HALF:, :] = res.results[2 * g + 1]["y"].T
    return out


# revision 24
# speedup vs baseline: 2.5277x; 1.0182x over previous
"""HGNN layer (hypergraph message passing) Trainium2 kernel, 8 NeuronCores.

Sharding: one graph per PAIR of cores; within a pair each core owns half the
hyperedge (Ec) / node (Nc) range. Host pre-casts the big matrices: the 0/1
incident matrix H ships as fp8e4 (exact) in the three layouts the PE needs;
Dv/De ship as fp8e3 scaled by 64 (descale folded into later evacuations); x
ships bf16 in block-transposed stationary layout. Streams use host-tiled
[128, k*HALF] layouts so slab DMAs move 1 MB at a time on the ACT HWDGE
queue. Dataflow computes hxx = H^T x first, then hx = hxx W + b (x) dege
(bias as a rank-1 PE accumulate against a host-computed edge-degree row);
attention scores use a hi/lo bf16 split of hxx and theta. Softmax is
unnormalized; z rides the first AllReduce as a bf16 hi/lo pair; 1/z and the
Dv descale fold into the h1c evacuation. htr stays SBUF-resident for its two
uses. Each of the 3 pair-AllReduces is split into two half-width bf16
collectives whose payloads are pre-transposed into the consumer's layout, so
the second half overlaps the first half's consumers and there is zero
post-AR rearrangement."""

import numpy as np

B, N, E, D = 4, 4096, 4096, 128
HALF = N // 2
NCORES = 8
PAIRS = [[0, 1], [2, 3], [4, 5], [6, 7]]
BN_EPS = 1e-5
F = 512                 # moving free-dim per matmul
NT = N // 128           # 32 k-tiles over a full 4096 dim
HT = HALF // 128        # 16 k-tiles over a half
SLAB = 4                # k-tiles per stream DMA (1 MB slabs)
DSCALE = 64.0           # host-side scale on Dv/De before fp8e3 cast
ZPAD = 16               # extra bf16 cols on the first AR chunk for z hi/lo

_CACHE = {}


def _build():
    import concourse.bacc as bacc
    import concourse.mybir as mybir
    import concourse.tile as tile
    from concourse.masks import make_identity
    from contextlib import ExitStack

    fp32 = mybir.dt.float32
    bf16 = mybir.dt.bfloat16
    fp8 = mybir.dt.float8e4
    fp8d = mybir.dt.float8e3
    Act = mybir.ActivationFunctionType
    Alu = mybir.AluOpType

    nc = bacc.Bacc("TRN2", target_bir_lowering=False, debug=False,
                   num_devices=NCORES)

    xv_d = nc.dram_tensor("xv", [128, N], bf16, kind="ExternalInput")
    hcol_d = nc.dram_tensor("hcol", [128, NT * HALF], fp8, kind="ExternalInput")
    htr_d = nc.dram_tensor("htr", [HALF, N], fp8, kind="ExternalInput")
    hrow_d = nc.dram_tensor("hrow", [128, 2 * HT * HALF], fp8,
                            kind="ExternalInput")
    dvT_d = nc.dram_tensor("dvT", [128, NT * HALF], fp8d, kind="ExternalInput")
    deT_d = nc.dram_tensor("deT", [128, NT * HALF], fp8d, kind="ExternalInput")
    dege_d = nc.dram_tensor("dege", [1, HALF], fp32, kind="ExternalInput")
    whi_d = nc.dram_tensor("whi", [D, D], bf16, kind="ExternalInput")
    th2_d = nc.dram_tensor("th2", [D, 2], bf16, kind="ExternalInput")
    brow_d = nc.dram_tensor("brow", [1, D], fp32, kind="ExternalInput")
    mask_d = nc.dram_tensor("mask", [1, HALF], fp32, kind="ExternalInput")
    eps_d = nc.dram_tensor("eps", [D, 1], fp32, kind="ExternalInput")
    bng_d = nc.dram_tensor("bng", [D, 1], fp32, kind="ExternalInput")
    bnb_d = nc.dram_tensor("bnb", [D, 1], fp32, kind="ExternalInput")
    bnm_d = nc.dram_tensor("bnm", [D, 1], fp32, kind="ExternalInput")
    bnv_d = nc.dram_tensor("bnv", [D, 1], fp32, kind="ExternalInput")
    y_d = nc.dram_tensor("y", [D, HALF], fp32, kind="ExternalOutput")

    with tile.TileContext(nc) as tc, ExitStack() as ctx:
        const = ctx.enter_context(tc.tile_pool(name="const", bufs=1))
        resident = ctx.enter_context(tc.tile_pool(name="resident", bufs=1))
        stream = ctx.enter_context(tc.tile_pool(name="stream", bufs=3))
        med = ctx.enter_context(tc.tile_pool(name="med", bufs=1))
        small = ctx.enter_context(tc.tile_pool(name="small", bufs=1))
        ps = ctx.enter_context(tc.tile_pool(name="ps", bufs=8, space="PSUM"))
        dram = ctx.enter_context(tc.tile_pool(name="dram", bufs=1, space="DRAM"))

        ident = const.tile([128, 128], fp32)
        make_identity(nc, ident)
        one11 = const.tile([1, 1], fp32)
        nc.vector.memset(one11[:], 1.0)
        ones_row = const.tile([1, 128], fp32)
        nc.vector.memset(ones_row[:], 1.0)
        ones2 = const.tile([2, 1], fp32)
        nc.vector.memset(ones2[:], 1.0)
        c64 = const.tile([128, 1], fp32)
        nc.vector.memset(c64[:], 1.0 / DSCALE)

        xv = const.tile([128, N], bf16)
        nc.sync.dma_start(out=xv[:], in_=xv_d.ap())

        def load_param(dt_):
            t = const.tile([D, 1], fp32, tag=dt_.name + "_p")
            nc.sync.dma_start(out=t[:], in_=dt_.ap())
            return t

        whi_t = const.tile([D, D], bf16)
        nc.sync.dma_start(out=whi_t[:], in_=whi_d.ap())
        th2_t = const.tile([D, 2], bf16)
        nc.sync.dma_start(out=th2_t[:], in_=th2_d.ap())
        brow_t = const.tile([1, D], fp32)
        nc.sync.dma_start(out=brow_t[:], in_=brow_d.ap())
        dege_t = const.tile([1, HALF], fp32)
        nc.sync.dma_start(out=dege_t[:], in_=dege_d.ap())
        eps_t = load_param(eps_d)
        bng_t = load_param(bng_d)
        bnb_t = load_param(bnb_d)
        bnm_t = load_param(bnm_d)
        bnv_t = load_param(bnv_d)
        mask_t = const.tile([1, HALF], fp32)
        nc.sync.dma_start(out=mask_t[:], in_=mask_d.ap())

        # dummy collective to absorb the first-collective warmup cost while
        # S2 streams (result unused)
        warm_in = dram.tile([1, 16], bf16, tag="warmi")
        warm_out = dram.tile([2, 16], bf16, tag="warmo")
        nc.gpsimd.collective_compute(
            "AllGather", Alu.bypass, replica_groups=PAIRS,
            ins=[warm_in.opt()], outs=[warm_out.opt()])

        # htr resident fp8 (sync queue; hcol stream rides the ACT queue).
        # Loads are interleaved into the S2 loop so hcol keeps priority.
        htr_res = resident.tile([128, HT * N], fp8)

        def slab_stream(dram_t, dt, n_tiles, name):
            """Yield (k_tile_index, moving_tile_fn) streaming 1MB slabs."""
            for s in range(n_tiles // SLAB):
                sb = stream.tile([128, SLAB * HALF], dt, tag="slab",
                                 name=name)
                nc.scalar.dma_start(
                    out=sb[:],
                    in_=dram_t.ap()[:, s * SLAB * HALF:(s + 1) * SLAB * HALF])
                for jj in range(SLAB):
                    j = s * SLAB + jj
                    yield j, sb[:, jj * HALF:(jj + 1) * HALF]

        # ---- S2: hxxT [D, HALF] = (H[:,Ec]^T x)^T ------------------------
        hxx_ps = [ps.tile([128, F], fp32, tag="ps", name=f"hxx{i}")
                  for i in range(HALF // F)]
        def load_htr(t):
            nc.scalar.dma_start(out=htr_res[:, t * N:(t + 1) * N],
                                in_=htr_d.ap()[t * 128:(t + 1) * 128, :])

        htr_loaded = 0
        for j, hj in slab_stream(hcol_d, fp8, NT, "hj"):
            for blk in range(HALF // F):
                nc.tensor.matmul(hxx_ps[blk][:],
                                 xv[:, j * D:(j + 1) * D],
                                 hj[:, blk * F:(blk + 1) * F],
                                 start=(j == 0), stop=(j == NT - 1))
            # 2 htr tiles per slab, starting after the 3rd slab is queued,
            # on the same ACT FIFO so hcol keeps strict priority
            if j % SLAB == SLAB - 1 and j // SLAB >= 2:
                load_htr(htr_loaded)
                load_htr(htr_loaded + 1)
                htr_loaded += 2
        while htr_loaded < HT:
            load_htr(htr_loaded)
            htr_loaded += 1
        hxxT = med.tile([D, HALF], fp32, tag="hxxT")
        hxx_hi = med.tile([D, HALF], bf16, tag="hxx_hi")
        hxx_lo = med.tile([D, HALF], bf16, tag="hxx_lo")
        tmp32 = med.tile([D, HALF], fp32, tag="ehxT")
        for blk in range(HALF // F):
            sl = slice(blk * F, (blk + 1) * F)
            nc.vector.tensor_copy(hxxT[:, sl], hxx_ps[blk][:])
        nc.vector.tensor_copy(hxx_hi[:], hxxT[:])
        nc.vector.tensor_copy(tmp32[:], hxx_hi[:])
        nc.vector.tensor_tensor(tmp32[:], hxxT[:], tmp32[:], op=Alu.subtract)
        nc.vector.tensor_copy(hxx_lo[:], tmp32[:])

        # ---- S3: hxT = W^T hxx + b (x) dege ; st = th^T hxx --------------
        hxT = med.tile([D, HALF], fp32, tag="hxT")
        st_sb = small.tile([1, HALF], fp32, tag="st_sb")
        for blk in range(HALF // F):
            sl = slice(blk * F, (blk + 1) * F)
            hx2 = ps.tile([128, F], fp32, tag="ps", name=f"hx2_{blk}")
            nc.tensor.matmul(hx2[:], whi_t[:], hxx_hi[:, sl],
                             start=True, stop=False)
            nc.tensor.matmul(hx2[:], whi_t[:], hxx_lo[:, sl],
                             start=False, stop=False)
            nc.tensor.matmul(hx2[:], brow_t[:], dege_t[:, sl],
                             start=False, stop=True)
            nc.vector.tensor_copy(hxT[:, sl], hx2[:])
            st2 = ps.tile([2, F], fp32, tag="ps", name=f"st2_{blk}")
            nc.tensor.matmul(st2[:], th2_t[:], hxx_hi[:, sl],
                             start=True, stop=False)
            nc.tensor.matmul(st2[:], th2_t[:], hxx_lo[:, sl],
                             start=False, stop=True)
            s2sb = med.tile([2, F], fp32, tag="s2sb", name=f"s2sb{blk}")
            nc.vector.tensor_copy(s2sb[:], st2[0:2, :])
            sp = ps.tile([1, F], fp32, tag="ps", name=f"sp{blk}")
            nc.tensor.matmul(sp[:], ones2[:], s2sb[:], start=True, stop=True)
            nc.vector.tensor_copy(st_sb[:, sl], sp[:])

        # ---- S4: softmax pieces (in-place on st_sb) ----------------------
        attn_u = st_sb
        nc.scalar.activation(attn_u[:], st_sb[:], Act.Exp)
        nc.vector.tensor_mul(attn_u[:], attn_u[:], mask_t[:])
        z_t = small.tile([1, 1], fp32, tag="z_t")
        nc.vector.reduce_sum(z_t[:], attn_u[:], axis=mybir.AxisListType.X)
        # z hi/lo bf16 pieces
        zhi = small.tile([1, 1], bf16, tag="zhi")
        zlo = small.tile([1, 1], bf16, tag="zlo")
        zf = small.tile([1, 1], fp32, tag="zf")
        nc.vector.tensor_copy(zhi[:], z_t[:])
        nc.vector.tensor_copy(zf[:], zhi[:])
        nc.vector.tensor_tensor(zf[:], z_t[:], zf[:], op=Alu.subtract)
        nc.vector.tensor_copy(zlo[:], zf[:])
        attnv = med.tile([128, HT], fp32, tag="attnv")
        for t in range(HT):
            pt = ps.tile([128, 1], fp32, tag="ps")
            nc.tensor.matmul(pt[:], attn_u[:, t * 128:(t + 1) * 128], one11[:],
                             start=True, stop=True)
            nc.vector.tensor_copy(attnv[:, t:t + 1], pt[:])
        ehxT = med.tile([D, HALF], fp32, tag="ehxT")
        nc.vector.tensor_scalar_mul(ehxT[:], hxT[:], eps_t[:])

        # ---- S5: h1av [128, HT*D] bf16 = attn * hx (e-part tiles) --------
        h1av = med.tile([128, HALF], bf16, tag="h1av")
        for t in range(HT):
            pt = ps.tile([128, 128], fp32, tag="ps")
            nc.tensor.transpose(pt[:], hxT[:, t * 128:(t + 1) * 128], ident[:])
            nc.vector.tensor_scalar_mul(h1av[:, t * 128:(t + 1) * 128], pt[:],
                                        attnv[:, t:t + 1])

        def chunked_bmm_ag_v(stationary, moving_of, tagbase, with_z=False):
            """Two half-width partial bmms; payload pre-transposed to the
            consumer's v-layout, cast bf16, AllGathered per chunk with a
            local DVE add of the two rank blocks (cheaper than ncfw
            AllReduce). Returns (resA, resB) bf16 [128, HALF(+ZPAD)]."""
            outs = []
            for chunk in range(2):
                w = HALF + ZPAD if (with_z and chunk == 0) else HALF
                pss = [ps.tile([128, F], fp32, tag="ps",
                               name=f"{tagbase}_{chunk}_{i}")
                       for i in range(HALF // F)]
                for t in range(HT):
                    for blk in range(HALF // F):
                        nc.tensor.matmul(
                            pss[blk][:],
                            stationary[:, t * 128:(t + 1) * 128],
                            moving_of(t, chunk * HALF + blk * F, F),
                            start=(t == 0), stop=(t == HT - 1))
                ccT = med.tile([D, HALF], fp32, tag="ccsbT")
                for blk in range(HALF // F):
                    sl = slice(blk * F, (blk + 1) * F)
                    nc.vector.tensor_copy(ccT[:, sl], pss[blk][:])
                ccv = med.tile([128, HALF + ZPAD], bf16,
                               tag=f"ccv{chunk}")
                for t in range(HT):
                    pt = ps.tile([128, 128], fp32, tag="ps")
                    nc.tensor.transpose(pt[:], ccT[:, t * 128:(t + 1) * 128],
                                        ident[:])
                    nc.vector.tensor_copy(ccv[:, t * 128:(t + 1) * 128], pt[:])
                if with_z and chunk == 0:
                    nc.vector.memset(ccv[:, HALF:], 0.0)
                    nc.vector.tensor_copy(ccv[0:1, HALF:HALF + 1], zhi[:])
                    nc.vector.tensor_copy(ccv[0:1, HALF + 1:HALF + 2], zlo[:])
                cc_in = dram.tile([128, w], bf16, tag=f"{tagbase}i{chunk}")
                cc_out = dram.tile([256, w], bf16, tag=f"{tagbase}o{chunk}")
                nc.sync.dma_start(out=cc_in[:], in_=ccv[:, 0:w])
                nc.gpsimd.collective_compute(
                    "AllGather", Alu.bypass, replica_groups=PAIRS,
                    ins=[cc_in.opt()], outs=[cc_out.opt()])
                res = med.tile([128, HALF + ZPAD], bf16,
                               tag=f"resv{chunk}")
                agt = med.tile([128, HALF + ZPAD], bf16, tag="agtmp")
                nc.sync.dma_start(out=res[:, 0:w], in_=cc_out[0:128, :])
                nc.sync.dma_start(out=agt[:, 0:w], in_=cc_out[128:256, :])
                nc.vector.tensor_tensor(res[:, 0:w], res[:, 0:w],
                                        agt[:, 0:w], op=Alu.add)
                outs.append(res)
            return outs

        # ---- S6: h1b = H h1a (partial over Ec), v-layout chunked AG ------
        h1bA, h1bB = chunked_bmm_ag_v(
            h1av,
            lambda t, lo, w: htr_res[:, t * N + lo:t * N + lo + w],
            "cc1", with_z=True)

        # rz = 1/(z), folded with 1/DSCALE, broadcast to [128, 1]
        rz = small.tile([1, 1], fp32, tag="rz")
        zs = small.tile([1, 1], fp32, tag="zs")
        nc.vector.tensor_copy(rz[:], h1bA[0:1, HALF:HALF + 1])
        nc.vector.tensor_copy(zs[:], h1bA[0:1, HALF + 1:HALF + 2])
        nc.vector.tensor_tensor(rz[:], rz[:], zs[:], op=Alu.add)
        nc.vector.reciprocal(rz[:], rz[:])
        rz_ps = ps.tile([128, 1], fp32, tag="ps")
        nc.tensor.matmul(rz_ps[:], ones_row[:], rz[:], start=True, stop=True)
        rz_bc = small.tile([128, 1], fp32, tag="rz_bc")
        nc.vector.tensor_copy(rz_bc[:], rz_ps[:])
        nc.vector.tensor_mul(rz_bc[:], rz_bc[:], c64[:])

        def vtile(resA, resB, j):
            src = resA if j < HT else resB
            jj = j % HT
            return src[:, jj * 128:(jj + 1) * 128]

        # ---- S7: h1cT = (Dv[Nc,:] h1b)^T * rz/DSCALE ---------------------
        h1c_ps = [ps.tile([128, F], fp32, tag="ps", name=f"h1c{i}")
                  for i in range(HALF // F)]
        for j, dj in slab_stream(dvT_d, fp8d, NT, "dj"):
            for blk in range(HALF // F):
                nc.tensor.matmul(h1c_ps[blk][:], vtile(h1bA, h1bB, j),
                                 dj[:, blk * F:(blk + 1) * F],
                                 start=(j == 0), stop=(j == NT - 1))
        h1cT = med.tile([D, HALF], fp32, tag="hxxT")
        for blk in range(HALF // F):
            sl = slice(blk * F, (blk + 1) * F)
            nc.vector.tensor_scalar_mul(h1cT[:, sl], h1c_ps[blk][:], rz_bc[:])
        h1cv = med.tile([128, HALF], bf16, tag="h1cv")
        for t in range(HT):
            pt = ps.tile([128, 128], fp32, tag="ps")
            nc.tensor.transpose(pt[:], h1cT[:, t * 128:(t + 1) * 128], ident[:])
            nc.vector.tensor_copy(h1cv[:, t * 128:(t + 1) * 128], pt[:])

        # ---- S8: h1d = H[Nc,:]^T h1c (partial over Nc), chunked AR -------
        hrow_state = {}

        def hrow_moving(t, lo, w):
            chunk = lo // HALF
            slab_i = t // SLAB
            key = (chunk, slab_i)
            if key not in hrow_state:
                sb = stream.tile([128, SLAB * HALF], fp8, tag="slab",
                                 name=f"rj{chunk}")
                base = chunk * HT * HALF + slab_i * SLAB * HALF
                nc.scalar.dma_start(
                    out=sb[:], in_=hrow_d.ap()[:, base:base + SLAB * HALF])
                hrow_state[key] = sb
            return hrow_state[key][:, (t % SLAB) * HALF + (lo % HALF):
                                   (t % SLAB) * HALF + (lo % HALF) + w]

        h1dA, h1dB = chunked_bmm_ag_v(h1cv, hrow_moving, "cc2")

        # ---- S9+S10: hT = (De[Ec,:] h1d)^T / DSCALE + eps*hx -------------
        h1e_ps = [ps.tile([128, F], fp32, tag="ps", name=f"h1e{i}")
                  for i in range(HALF // F)]
        for j, ej in slab_stream(deT_d, fp8d, NT, "ej"):
            for blk in range(HALF // F):
                nc.tensor.matmul(h1e_ps[blk][:], vtile(h1dA, h1dB, j),
                                 ej[:, blk * F:(blk + 1) * F],
                                 start=(j == 0), stop=(j == NT - 1))
        hT = med.tile([D, HALF], fp32, tag="hxT")
        for blk in range(HALF // F):
            sl = slice(blk * F, (blk + 1) * F)
            nc.vector.scalar_tensor_tensor(hT[:, sl], h1e_ps[blk][:],
                                           1.0 / DSCALE, ehxT[:, sl],
                                           op0=Alu.mult, op1=Alu.add)
        hv = med.tile([128, HALF], bf16, tag="h1av")
        for t in range(HT):
            pt = ps.tile([128, 128], fp32, tag="ps")
            nc.tensor.transpose(pt[:], hT[:, t * 128:(t + 1) * 128], ident[:])
            nc.vector.tensor_copy(hv[:, t * 128:(t + 1) * 128], pt[:])

        # ---- S11: out = H h (partial over Ec), bf16 T-layout chunked AR --
        s_bn = small.tile([D, 1], fp32, tag="s_bn")
        nc.vector.tensor_scalar_add(s_bn[:], bnv_t[:], BN_EPS)
        nc.scalar.activation(s_bn[:], s_bn[:], Act.Sqrt)
        nc.vector.reciprocal(s_bn[:], s_bn[:])
        nc.vector.tensor_mul(s_bn[:], s_bn[:], bng_t[:])
        t_bn = small.tile([D, 1], fp32, tag="t_bn")
        nc.vector.tensor_mul(t_bn[:], bnm_t[:], s_bn[:])
        nc.vector.tensor_tensor(t_bn[:], bnb_t[:], t_bn[:], op=Alu.subtract)

        # two half-width ReduceScatters (core even gets summed cols 0:HALF,
        # odd the rest). S11's matmuls are grouped into quarter-pairs so
        # RS h=0 fires after only half the matmuls; its epilogue overlaps
        # the rest of S11 and RS h=1.
        Q = HALF // 2
        cc3_in = [dram.tile([256, Q], bf16, tag=f"cc3i{h}", name=f"cc3i{h}")
                  for h in range(2)]
        cc3_out = [dram.tile([128, Q], bf16, tag=f"cc3o{h}", name=f"cc3o{h}")
                   for h in range(2)]
        for h in range(2):
            # blocks covering cols [h*Q:(h+1)*Q] of both n-chunks
            pss = [ps.tile([128, F], fp32, tag="ps", name=f"out_{h}_{i}")
                   for i in range(4)]
            for t in range(HT):
                for i in range(4):
                    chunk, blk = divmod(i, 2)
                    lo = chunk * HALF + h * Q + blk * F
                    nc.tensor.matmul(pss[i][:],
                                     hv[:, t * 128:(t + 1) * 128],
                                     htr_res[:, t * N + lo:t * N + lo + F],
                                     start=(t == 0), stop=(t == HT - 1))
            ccv = med.tile([128, HALF + ZPAD], bf16, tag=f"ccv{h}")
            for i in range(4):
                chunk, blk = divmod(i, 2)
                sl = slice(chunk * Q + blk * F, chunk * Q + (blk + 1) * F)
                nc.vector.tensor_copy(ccv[:, sl], pss[i][:])
            for chunk in range(2):
                nc.sync.dma_start(
                    out=cc3_in[h][chunk * 128:(chunk + 1) * 128, :],
                    in_=ccv[:, chunk * Q:(chunk + 1) * Q])
            nc.gpsimd.collective_compute(
                "ReduceScatter", Alu.add, replica_groups=PAIRS,
                ins=[cc3_in[h].opt()], outs=[cc3_out[h].opt()])
        for h in range(2):
            res3 = med.tile([128, HALF + ZPAD], bf16, tag=f"resv{h}")
            nc.sync.dma_start(out=res3[:, 0:Q], in_=cc3_out[h][:])
            of = med.tile([D, Q], fp32, tag=f"of{h}")
            nc.scalar.activation(of[:], res3[:, 0:Q], Act.Lrelu, alpha=0.01)
            nc.vector.tensor_scalar(of[:], of[:], s_bn[:], t_bn[:],
                                    op0=Alu.mult, op1=Alu.add)
            nc.sync.dma_start(out=y_d.ap()[:, h * Q:(h + 1) * Q], in_=of[:])

    nc.finalize()
    return nc


def _get_nc():
    if "nc" not in _CACHE:
        _CACHE["nc"] = _build()
    return _CACHE["nc"]


def _tile128(a):
    """[K*128, W] -> [128, K*W] block-transposed stream layout."""
    K = a.shape[0] // 128
    return np.ascontiguousarray(
        a.reshape(K, 128, a.shape[1]).transpose(1, 0, 2).reshape(
            128, K * a.shape[1]))


def _shard(inputs):
    import ml_dtypes
    bf16 = ml_dtypes.bfloat16
    fp8 = ml_dtypes.float8_e4m3
    fp8d = ml_dtypes.float8_e3m4

    H = np.asarray(inputs["incident_mat"], dtype=np.float32)
    Dv = np.asarray(inputs["degree_v"], dtype=np.float32)
    De = np.asarray(inputs["degree_e"], dtype=np.float32)
    x = np.asarray(inputs["x"], dtype=np.float32)
    em = np.asarray(inputs["e_masks"])
    w = np.asarray(inputs["mlp_W"], dtype=np.float32)
    b = np.asarray(inputs["mlp_b"], dtype=np.float32)
    th = np.asarray(inputs["theta_att"], dtype=np.float32).reshape(D)
    eps = np.full((D, 1), float(np.asarray(inputs["eps"]).reshape(-1)[0]),
                  dtype=np.float32)

    def col(v):
        return np.ascontiguousarray(
            np.asarray(v, dtype=np.float32).reshape(D, 1))

    bng, bnb = col(inputs["bn_gamma"]), col(inputs["bn_beta"])
    bnm, bnv = col(inputs["bn_mean"]), col(inputs["bn_var"])

    whi = w.astype(bf16)
    th_hi = th.astype(bf16)
    th_lo = (th - th_hi.astype(np.float32)).astype(bf16)
    th2 = np.ascontiguousarray(np.stack(
        [th_hi.astype(np.float32), th_lo.astype(np.float32)], axis=1)
    ).astype(bf16)
    brow = np.ascontiguousarray(b.reshape(1, D))

    in_maps = []
    for g in range(B):
        Hg8 = H[g].astype(fp8)
        HgT8 = np.ascontiguousarray(H[g].T).astype(fp8)
        dege_full = H[g].sum(axis=0, dtype=np.float32)
        xv = _tile128(x[g]).astype(bf16)
        DvT = (Dv[g].T * DSCALE).astype(fp8d)
        DeT = (De[g].T * DSCALE).astype(fp8d)
        for c in range(2):
            lo, hi = c * HALF, (c + 1) * HALF
            # hrow chunk-major: [128, 2*HT*HALF], chunk c2 at t*HALF+e
            hrow_g = Hg8[lo:hi, :].reshape(HT, 128, 2, HALF)
            hrow_t = np.ascontiguousarray(
                hrow_g.transpose(1, 2, 0, 3).reshape(128, 2 * HT * HALF))
            in_maps.append({
                "xv": xv,
                "hcol": _tile128(np.ascontiguousarray(Hg8[:, lo:hi])),
                "htr": np.ascontiguousarray(HgT8[lo:hi, :]),
                "hrow": hrow_t,
                "dvT": _tile128(np.ascontiguousarray(DvT[:, lo:hi])),
                "deT": _tile128(np.ascontiguousarray(DeT[:, lo:hi])),
                "dege": np.ascontiguousarray(
                    dege_full[lo:hi].reshape(1, HALF)),
                "whi": whi, "th2": th2, "brow": brow,
                "mask": np.ascontiguousarray(
                    em[g, lo:hi].astype(np.float32).reshape(1, HALF)),
                "eps": eps,
                "bng": bng, "bnb": bnb, "bnm": bnm, "bnv": bnv,
            })
    return in_maps


def kernel(**inputs):
    from concourse.bass_utils import run_bass_kernel_spmd

    nc = _get_nc()
    in_maps = _shard(inputs)
    res = run_bass_kernel_spmd(nc, in_maps, list(range(NCORES)))
    out = np.empty((B, N, D), dtype=np.float32)
    for g in range(B):
        out[g, :HALF, :] = res.results[2 * g]["y"].T
        out[g, HALF:, :] = res.results[2 * g + 1]["y"].T
    return out


# revision 28
# speedup vs baseline: 2.6000x; 1.0286x over previous
"""HGNN layer (hypergraph message passing) Trainium2 kernel, 8 NeuronCores.

Sharding: one graph per PAIR of cores; within a pair each core owns half the
hyperedge (Ec) / node (Nc) range. Host pre-casts the big matrices: the 0/1
incident matrix H ships as fp8e4 (exact) in the three layouts the PE needs;
Dv/De ship as fp8e3 scaled by 64 (descale folded into later evacuations); x
ships bf16 in block-transposed stationary layout. Streams use host-tiled
[128, k*HALF] layouts so slab DMAs move 1 MB at a time on the ACT HWDGE
queue. Dataflow computes hxx = H^T x first, then hx = hxx W + b (x) dege
(bias as a rank-1 PE accumulate against a host-computed edge-degree row);
attention scores use a hi/lo bf16 split of hxx and theta. Softmax is
unnormalized; z rides the first AllReduce as a bf16 hi/lo pair; 1/z and the
Dv descale fold into the h1c evacuation. htr stays SBUF-resident for its two
uses. Each of the 3 pair-AllReduces is split into two half-width bf16
collectives whose payloads are pre-transposed into the consumer's layout, so
the second half overlaps the first half's consumers and there is zero
post-AR rearrangement."""

import numpy as np

B, N, E, D = 4, 4096, 4096, 128
HALF = N // 2
NCORES = 8
PAIRS = [[0, 1], [2, 3], [4, 5], [6, 7]]
BN_EPS = 1e-5
F = 512                 # moving free-dim per matmul
NT = N // 128           # 32 k-tiles over a full 4096 dim
HT = HALF // 128        # 16 k-tiles over a half
SLAB = 4                # k-tiles per stream DMA (1 MB slabs)
DSCALE = 64.0           # host-side scale on Dv/De before fp8e3 cast
ZPAD = 16               # extra bf16 cols on the first AR chunk for z hi/lo

_CACHE = {}


def _build():
    import concourse.bacc as bacc
    import concourse.mybir as mybir
    import concourse.tile as tile
    from concourse.masks import make_identity
    from contextlib import ExitStack

    fp32 = mybir.dt.float32
    bf16 = mybir.dt.bfloat16
    fp8 = mybir.dt.float8e4
    fp8d = mybir.dt.float8e3
    Act = mybir.ActivationFunctionType
    Alu = mybir.AluOpType

    nc = bacc.Bacc("TRN2", target_bir_lowering=False, debug=False,
                   num_devices=NCORES)

    xv_d = nc.dram_tensor("xv", [128, N], bf16, kind="ExternalInput")
    hcol_d = nc.dram_tensor("hcol", [128, NT * HALF], fp8, kind="ExternalInput")
    htr_d = nc.dram_tensor("htr", [HALF, N], fp8, kind="ExternalInput")
    hrow_d = nc.dram_tensor("hrow", [128, 2 * HT * HALF], fp8,
                            kind="ExternalInput")
    dvT_d = nc.dram_tensor("dvT", [128, NT * HALF], fp8d, kind="ExternalInput")
    deT_d = nc.dram_tensor("deT", [128, NT * HALF], fp8d, kind="ExternalInput")
    dege_d = nc.dram_tensor("dege", [1, HALF], fp32, kind="ExternalInput")
    whi_d = nc.dram_tensor("whi", [D, D], bf16, kind="ExternalInput")
    th2_d = nc.dram_tensor("th2", [D, 1], fp32, kind="ExternalInput")
    brow_d = nc.dram_tensor("brow", [1, D], fp32, kind="ExternalInput")
    mask_d = nc.dram_tensor("mask", [1, HALF], fp32, kind="ExternalInput")
    eps_d = nc.dram_tensor("eps", [D, 1], fp32, kind="ExternalInput")
    bng_d = nc.dram_tensor("bng", [D, 1], fp32, kind="ExternalInput")
    bnb_d = nc.dram_tensor("bnb", [D, 1], fp32, kind="ExternalInput")
    bnm_d = nc.dram_tensor("bnm", [D, 1], fp32, kind="ExternalInput")
    bnv_d = nc.dram_tensor("bnv", [D, 1], fp32, kind="ExternalInput")
    y_d = nc.dram_tensor("y", [D, HALF], fp32, kind="ExternalOutput")

    with tile.TileContext(nc) as tc, ExitStack() as ctx:
        const = ctx.enter_context(tc.tile_pool(name="const", bufs=1))
        resident = ctx.enter_context(tc.tile_pool(name="resident", bufs=1))
        stream = ctx.enter_context(tc.tile_pool(name="stream", bufs=3))
        med = ctx.enter_context(tc.tile_pool(name="med", bufs=1))
        small = ctx.enter_context(tc.tile_pool(name="small", bufs=1))
        ps = ctx.enter_context(tc.tile_pool(name="ps", bufs=8, space="PSUM"))
        dram = ctx.enter_context(tc.tile_pool(name="dram", bufs=1, space="DRAM"))

        ident = const.tile([128, 128], fp32)
        make_identity(nc, ident)
        one11 = const.tile([1, 1], fp32)
        nc.vector.memset(one11[:], 1.0)
        ones_row = const.tile([1, 128], fp32)
        nc.vector.memset(ones_row[:], 1.0)
        ones2 = const.tile([2, 1], fp32)
        nc.vector.memset(ones2[:], 1.0)
        c64 = const.tile([128, 1], fp32)
        nc.vector.memset(c64[:], 1.0 / DSCALE)

        xv = const.tile([128, N], bf16)
        nc.sync.dma_start(out=xv[:], in_=xv_d.ap())

        def load_param(dt_):
            t = const.tile([D, 1], fp32, tag=dt_.name + "_p")
            nc.sync.dma_start(out=t[:], in_=dt_.ap())
            return t

        whi_t = const.tile([D, D], bf16)
        nc.sync.dma_start(out=whi_t[:], in_=whi_d.ap())
        thf_t = const.tile([D, 1], fp32)
        nc.sync.dma_start(out=thf_t[:], in_=th2_d.ap())
        brow_t = const.tile([1, D], fp32)
        nc.sync.dma_start(out=brow_t[:], in_=brow_d.ap())
        dege_t = const.tile([1, HALF], fp32)
        nc.sync.dma_start(out=dege_t[:], in_=dege_d.ap())
        eps_t = load_param(eps_d)
        bng_t = load_param(bng_d)
        bnb_t = load_param(bnb_d)
        bnm_t = load_param(bnm_d)
        bnv_t = load_param(bnv_d)
        mask_t = const.tile([1, HALF], fp32)
        nc.sync.dma_start(out=mask_t[:], in_=mask_d.ap())

        # dummy collective to absorb the first-collective warmup cost while
        # S2 streams (result unused)
        warm_in = dram.tile([1, 16], bf16, tag="warmi")
        warm_out = dram.tile([2, 16], bf16, tag="warmo")
        nc.gpsimd.collective_compute(
            "AllGather", Alu.bypass, replica_groups=PAIRS,
            ins=[warm_in.opt()], outs=[warm_out.opt()])

        # htr resident fp8 (sync queue; hcol stream rides the ACT queue).
        # Loads are interleaved into the S2 loop so hcol keeps priority.
        htr_res = resident.tile([128, HT * N], fp8)

        def slab_stream(dram_t, dt, n_tiles, name):
            """Yield (k_tile_index, moving_tile_fn) streaming 1MB slabs."""
            for s in range(n_tiles // SLAB):
                sb = stream.tile([128, SLAB * HALF], dt, tag="slab",
                                 name=name)
                nc.scalar.dma_start(
                    out=sb[:],
                    in_=dram_t.ap()[:, s * SLAB * HALF:(s + 1) * SLAB * HALF])
                for jj in range(SLAB):
                    j = s * SLAB + jj
                    yield j, sb[:, jj * HALF:(jj + 1) * HALF]

        # ---- S2: hxxT [D, HALF] = (H[:,Ec]^T x)^T ------------------------
        hxx_ps = [ps.tile([128, F], fp32, tag="ps", name=f"hxx{i}")
                  for i in range(HALF // F)]
        def load_htr(t):
            nc.scalar.dma_start(out=htr_res[:, t * N:(t + 1) * N],
                                in_=htr_d.ap()[t * 128:(t + 1) * 128, :])

        htr_loaded = 0
        for j, hj in slab_stream(hcol_d, fp8, NT, "hj"):
            for blk in range(HALF // F):
                nc.tensor.matmul(hxx_ps[blk][:],
                                 xv[:, j * D:(j + 1) * D],
                                 hj[:, blk * F:(blk + 1) * F],
                                 start=(j == 0), stop=(j == NT - 1))
            # 2 htr tiles per slab, starting after the 3rd slab is queued,
            # on the same ACT FIFO so hcol keeps strict priority
            if j % SLAB == SLAB - 1 and j // SLAB >= 2:
                load_htr(htr_loaded)
                load_htr(htr_loaded + 1)
                htr_loaded += 2
        while htr_loaded < HT:
            load_htr(htr_loaded)
            htr_loaded += 1
        hxxT = med.tile([D, HALF], fp32, tag="hxxT")
        hxx_hi = med.tile([D, HALF], bf16, tag="hxx_hi")
        for blk in range(HALF // F):
            sl = slice(blk * F, (blk + 1) * F)
            nc.vector.tensor_copy(hxxT[:, sl], hxx_ps[blk][:])
            nc.vector.tensor_copy(hxx_hi[:, sl], hxx_ps[blk][:])

        # ---- S3: hxT = W^T hxx + b (x) dege ; st = th^T hxx (fp32) -------
        hxT = med.tile([D, HALF], fp32, tag="hxT")
        st_sb = small.tile([1, HALF], fp32, tag="st_sb")
        for blk in range(HALF // F):
            sl = slice(blk * F, (blk + 1) * F)
            hx2 = ps.tile([128, F], fp32, tag="ps", name=f"hx2_{blk}")
            nc.tensor.matmul(hx2[:], whi_t[:], hxx_hi[:, sl],
                             start=True, stop=False)
            nc.tensor.matmul(hx2[:], brow_t[:], dege_t[:, sl],
                             start=False, stop=True)
            nc.vector.tensor_copy(hxT[:, sl], hx2[:])
            sp = ps.tile([1, F], fp32, tag="ps", name=f"sp{blk}")
            nc.tensor.matmul(sp[:], thf_t[:], hxxT[:, sl],
                             start=True, stop=True)
            nc.vector.tensor_copy(st_sb[:, sl], sp[:])

        # ---- S4: softmax pieces (in-place on st_sb) ----------------------
        attn_u = st_sb
        nc.scalar.activation(attn_u[:], st_sb[:], Act.Exp)
        nc.vector.tensor_mul(attn_u[:], attn_u[:], mask_t[:])
        z_t = small.tile([1, 1], fp32, tag="z_t")
        nc.vector.reduce_sum(z_t[:], attn_u[:], axis=mybir.AxisListType.X)
        # z hi/lo bf16 pieces
        zhi = small.tile([1, 1], bf16, tag="zhi")
        zlo = small.tile([1, 1], bf16, tag="zlo")
        zf = small.tile([1, 1], fp32, tag="zf")
        nc.vector.tensor_copy(zhi[:], z_t[:])
        nc.vector.tensor_copy(zf[:], zhi[:])
        nc.vector.tensor_tensor(zf[:], z_t[:], zf[:], op=Alu.subtract)
        nc.vector.tensor_copy(zlo[:], zf[:])
        attnv = med.tile([128, HT], fp32, tag="attnv")
        for t in range(HT):
            pt = ps.tile([128, 1], fp32, tag="ps")
            nc.tensor.matmul(pt[:], attn_u[:, t * 128:(t + 1) * 128], one11[:],
                             start=True, stop=True)
            nc.vector.tensor_copy(attnv[:, t:t + 1], pt[:])
        ehxT = med.tile([D, HALF], fp32, tag="ehxT")
        nc.vector.tensor_scalar_mul(ehxT[:], hxT[:], eps_t[:])

        # ---- S5: h1av [128, HT*D] bf16 = attn * hx (e-part tiles) --------
        h1av = med.tile([128, HALF], bf16, tag="h1av")
        for t in range(HT):
            pt = ps.tile([128, 128], fp32, tag="ps")
            nc.tensor.transpose(pt[:], hxT[:, t * 128:(t + 1) * 128], ident[:])
            nc.vector.tensor_scalar_mul(h1av[:, t * 128:(t + 1) * 128], pt[:],
                                        attnv[:, t:t + 1])

        def chunked_bmm_ag_v(stationary, moving_of, tagbase, with_z=False):
            """Two half-width partial bmms; payload pre-transposed to the
            consumer's v-layout, cast bf16, AllGathered per chunk with a
            local DVE add of the two rank blocks (cheaper than ncfw
            AllReduce). Returns (resA, resB) bf16 [128, HALF(+ZPAD)]."""
            outs = []
            for chunk in range(2):
                w = HALF + ZPAD if (with_z and chunk == 0) else HALF
                pss = [ps.tile([128, F], fp32, tag="ps",
                               name=f"{tagbase}_{chunk}_{i}")
                       for i in range(HALF // F)]
                for t in range(HT):
                    for blk in range(HALF // F):
                        nc.tensor.matmul(
                            pss[blk][:],
                            stationary[:, t * 128:(t + 1) * 128],
                            moving_of(t, chunk * HALF + blk * F, F),
                            start=(t == 0), stop=(t == HT - 1))
                ccT = med.tile([D, HALF], fp32, tag="ccsbT")
                for blk in range(HALF // F):
                    sl = slice(blk * F, (blk + 1) * F)
                    nc.vector.tensor_copy(ccT[:, sl], pss[blk][:])
                ccv = med.tile([128, HALF + ZPAD], bf16,
                               tag=f"ccv{chunk}")
                for t in range(HT):
                    pt = ps.tile([128, 128], fp32, tag="ps")
                    nc.tensor.transpose(pt[:], ccT[:, t * 128:(t + 1) * 128],
                                        ident[:])
                    nc.vector.tensor_copy(ccv[:, t * 128:(t + 1) * 128], pt[:])
                if with_z and chunk == 0:
                    nc.vector.memset(ccv[:, HALF:], 0.0)
                    nc.vector.tensor_copy(ccv[0:1, HALF:HALF + 1], zhi[:])
                    nc.vector.tensor_copy(ccv[0:1, HALF + 1:HALF + 2], zlo[:])
                cc_in = dram.tile([128, w], bf16, tag=f"{tagbase}i{chunk}")
                cc_out = dram.tile([256, w], bf16, tag=f"{tagbase}o{chunk}")
                nc.sync.dma_start(out=cc_in[:], in_=ccv[:, 0:w])
                nc.gpsimd.collective_compute(
                    "AllGather", Alu.bypass, replica_groups=PAIRS,
                    ins=[cc_in.opt()], outs=[cc_out.opt()])
                res = med.tile([128, HALF + ZPAD], bf16,
                               tag=f"resv{chunk}")
                agt = med.tile([128, HALF + ZPAD], bf16, tag="agtmp")
                nc.sync.dma_start(out=res[:, 0:w], in_=cc_out[0:128, :])
                nc.sync.dma_start(out=agt[:, 0:w], in_=cc_out[128:256, :])
                nc.vector.tensor_tensor(res[:, 0:w], res[:, 0:w],
                                        agt[:, 0:w], op=Alu.add)
                outs.append(res)
            return outs

        # ---- S6: h1b = H h1a (partial over Ec), v-layout chunked AG ------
        h1bA, h1bB = chunked_bmm_ag_v(
            h1av,
            lambda t, lo, w: htr_res[:, t * N + lo:t * N + lo + w],
            "cc1", with_z=True)

        # rz = 1/(z), folded with 1/DSCALE, broadcast to [128, 1]
        rz = small.tile([1, 1], fp32, tag="rz")
        zs = small.tile([1, 1], fp32, tag="zs")
        nc.vector.tensor_copy(rz[:], h1bA[0:1, HALF:HALF + 1])
        nc.vector.tensor_copy(zs[:], h1bA[0:1, HALF + 1:HALF + 2])
        nc.vector.tensor_tensor(rz[:], rz[:], zs[:], op=Alu.add)
        nc.vector.reciprocal(rz[:], rz[:])
        rz_ps = ps.tile([128, 1], fp32, tag="ps")
        nc.tensor.matmul(rz_ps[:], ones_row[:], rz[:], start=True, stop=True)
        rz_bc = small.tile([128, 1], fp32, tag="rz_bc")
        nc.vector.tensor_copy(rz_bc[:], rz_ps[:])
        nc.vector.tensor_mul(rz_bc[:], rz_bc[:], c64[:])

        def vtile(resA, resB, j):
            src = resA if j < HT else resB
            jj = j % HT
            return src[:, jj * 128:(jj + 1) * 128]

        # ---- S7: h1cT = (Dv[Nc,:] h1b)^T * rz/DSCALE ---------------------
        h1c_ps = [ps.tile([128, F], fp32, tag="ps", name=f"h1c{i}")
                  for i in range(HALF // F)]
        for j, dj in slab_stream(dvT_d, fp8d, NT, "dj"):
            for blk in range(HALF // F):
                nc.tensor.matmul(h1c_ps[blk][:], vtile(h1bA, h1bB, j),
                                 dj[:, blk * F:(blk + 1) * F],
                                 start=(j == 0), stop=(j == NT - 1))
        h1cT = med.tile([D, HALF], fp32, tag="hxxT")
        for blk in range(HALF // F):
            sl = slice(blk * F, (blk + 1) * F)
            nc.vector.tensor_scalar_mul(h1cT[:, sl], h1c_ps[blk][:], rz_bc[:])
        h1cv = med.tile([128, HALF], bf16, tag="h1cv")
        for t in range(HT):
            pt = ps.tile([128, 128], fp32, tag="ps")
            nc.tensor.transpose(pt[:], h1cT[:, t * 128:(t + 1) * 128], ident[:])
            nc.vector.tensor_copy(h1cv[:, t * 128:(t + 1) * 128], pt[:])

        # ---- S8: h1d = H[Nc,:]^T h1c (partial over Nc), chunked AR -------
        hrow_state = {}

        def hrow_moving(t, lo, w):
            chunk = lo // HALF
            slab_i = t // SLAB
            key = (chunk, slab_i)
            if key not in hrow_state:
                sb = stream.tile([128, SLAB * HALF], fp8, tag="slab",
                                 name=f"rj{chunk}")
                base = chunk * HT * HALF + slab_i * SLAB * HALF
                nc.scalar.dma_start(
                    out=sb[:], in_=hrow_d.ap()[:, base:base + SLAB * HALF])
                hrow_state[key] = sb
            return hrow_state[key][:, (t % SLAB) * HALF + (lo % HALF):
                                   (t % SLAB) * HALF + (lo % HALF) + w]

        h1dA, h1dB = chunked_bmm_ag_v(h1cv, hrow_moving, "cc2")

        # ---- S9+S10: hT = (De[Ec,:] h1d)^T / DSCALE + eps*hx -------------
        h1e_ps = [ps.tile([128, F], fp32, tag="ps", name=f"h1e{i}")
                  for i in range(HALF // F)]
        for j, ej in slab_stream(deT_d, fp8d, NT, "ej"):
            for blk in range(HALF // F):
                nc.tensor.matmul(h1e_ps[blk][:], vtile(h1dA, h1dB, j),
                                 ej[:, blk * F:(blk + 1) * F],
                                 start=(j == 0), stop=(j == NT - 1))
        hT = med.tile([D, HALF], fp32, tag="hxT")
        for blk in range(HALF // F):
            sl = slice(blk * F, (blk + 1) * F)
            nc.vector.scalar_tensor_tensor(hT[:, sl], h1e_ps[blk][:],
                                           1.0 / DSCALE, ehxT[:, sl],
                                           op0=Alu.mult, op1=Alu.add)
        hv = med.tile([128, HALF], bf16, tag="h1av")
        for t in range(HT):
            pt = ps.tile([128, 128], fp32, tag="ps")
            nc.tensor.transpose(pt[:], hT[:, t * 128:(t + 1) * 128], ident[:])
            nc.vector.tensor_copy(hv[:, t * 128:(t + 1) * 128], pt[:])

        # ---- S11: out = H h (partial over Ec), bf16 T-layout chunked AR --
        s_bn = small.tile([D, 1], fp32, tag="s_bn")
        nc.vector.tensor_scalar_add(s_bn[:], bnv_t[:], BN_EPS)
        nc.scalar.activation(s_bn[:], s_bn[:], Act.Sqrt)
        nc.vector.reciprocal(s_bn[:], s_bn[:])
        nc.vector.tensor_mul(s_bn[:], s_bn[:], bng_t[:])
        t_bn = small.tile([D, 1], fp32, tag="t_bn")
        nc.vector.tensor_mul(t_bn[:], bnm_t[:], s_bn[:])
        nc.vector.tensor_tensor(t_bn[:], bnb_t[:], t_bn[:], op=Alu.subtract)

        # two half-width ReduceScatters (core even gets summed cols 0:HALF,
        # odd the rest). S11's matmuls are grouped into quarter-pairs so
        # RS h=0 fires after only half the matmuls; its epilogue overlaps
        # the rest of S11 and RS h=1.
        Q = HALF // 2
        cc3_in = [dram.tile([256, Q], bf16, tag=f"cc3i{h}", name=f"cc3i{h}")
                  for h in range(2)]
        cc3_out = [dram.tile([128, Q], bf16, tag=f"cc3o{h}", name=f"cc3o{h}")
                   for h in range(2)]
        for h in range(2):
            # blocks covering cols [h*Q:(h+1)*Q] of both n-chunks
            pss = [ps.tile([128, F], fp32, tag="ps", name=f"out_{h}_{i}")
                   for i in range(4)]
            for t in range(HT):
                for i in range(4):
                    chunk, blk = divmod(i, 2)
                    lo = chunk * HALF + h * Q + blk * F
                    nc.tensor.matmul(pss[i][:],
                                     hv[:, t * 128:(t + 1) * 128],
                                     htr_res[:, t * N + lo:t * N + lo + F],
                                     start=(t == 0), stop=(t == HT - 1))
            ccv = med.tile([128, HALF + ZPAD], bf16, tag=f"ccv{h}")
            for i in range(4):
                chunk, blk = divmod(i, 2)
                sl = slice(chunk * Q + blk * F, chunk * Q + (blk + 1) * F)
                nc.vector.tensor_copy(ccv[:, sl], pss[i][:])
            for chunk in range(2):
                nc.sync.dma_start(
                    out=cc3_in[h][chunk * 128:(chunk + 1) * 128, :],
                    in_=ccv[:, chunk * Q:(chunk + 1) * Q])
            nc.gpsimd.collective_compute(
                "ReduceScatter", Alu.add, replica_groups=PAIRS,
                ins=[cc3_in[h].opt()], outs=[cc3_out[h].opt()])
        for h in range(2):
            res3 = med.tile([128, HALF + ZPAD], bf16, tag=f"resv{h}")
            nc.sync.dma_start(out=res3[:, 0:Q], in_=cc3_out[h][:])
            of = med.tile([D, Q], fp32, tag=f"of{h}")
            nc.scalar.activation(of[:], res3[:, 0:Q], Act.Lrelu, alpha=0.01)
            nc.vector.tensor_scalar(of[:], of[:], s_bn[:], t_bn[:],
                                    op0=Alu.mult, op1=Alu.add)
            nc.sync.dma_start(out=y_d.ap()[:, h * Q:(h + 1) * Q], in_=of[:])

    nc.finalize()
    return nc


def _get_nc():
    if "nc" not in _CACHE:
        _CACHE["nc"] = _build()
    return _CACHE["nc"]


def _tile128(a):
    """[K*128, W] -> [128, K*W] block-transposed stream layout."""
    K = a.shape[0] // 128
    return np.ascontiguousarray(
        a.reshape(K, 128, a.shape[1]).transpose(1, 0, 2).reshape(
            128, K * a.shape[1]))


def _shard(inputs):
    import ml_dtypes
    bf16 = ml_dtypes.bfloat16
    fp8 = ml_dtypes.float8_e4m3
    fp8d = ml_dtypes.float8_e3m4

    H = np.asarray(inputs["incident_mat"], dtype=np.float32)
    Dv = np.asarray(inputs["degree_v"], dtype=np.float32)
    De = np.asarray(inputs["degree_e"], dtype=np.float32)
    x = np.asarray(inputs["x"], dtype=np.float32)
    em = np.asarray(inputs["e_masks"])
    w = np.asarray(inputs["mlp_W"], dtype=np.float32)
    b = np.asarray(inputs["mlp_b"], dtype=np.float32)
    th = np.asarray(inputs["theta_att"], dtype=np.float32).reshape(D)
    eps = np.full((D, 1), float(np.asarray(inputs["eps"]).reshape(-1)[0]),
                  dtype=np.float32)

    def col(v):
        return np.ascontiguousarray(
            np.asarray(v, dtype=np.float32).reshape(D, 1))

    bng, bnb = col(inputs["bn_gamma"]), col(inputs["bn_beta"])
    bnm, bnv = col(inputs["bn_mean"]), col(inputs["bn_var"])

    whi = w.astype(bf16)
    th2 = np.ascontiguousarray(th.reshape(D, 1))
    brow = np.ascontiguousarray(b.reshape(1, D))

    in_maps = []
    for g in range(B):
        Hg8 = H[g].astype(fp8)
        HgT8 = np.ascontiguousarray(H[g].T).astype(fp8)
        dege_full = H[g].sum(axis=0, dtype=np.float32)
        xv = _tile128(x[g]).astype(bf16)
        DvT = (Dv[g].T * DSCALE).astype(fp8d)
        DeT = (De[g].T * DSCALE).astype(fp8d)
        for c in range(2):
            lo, hi = c * HALF, (c + 1) * HALF
            # hrow chunk-major: [128, 2*HT*HALF], chunk c2 at t*HALF+e
            hrow_g = Hg8[lo:hi, :].reshape(HT, 128, 2, HALF)
            hrow_t = np.ascontiguousarray(
                hrow_g.transpose(1, 2, 0, 3).reshape(128, 2 * HT * HALF))
            in_maps.append({
                "xv": xv,
                "hcol": _tile128(np.ascontiguousarray(Hg8[:, lo:hi])),
                "htr": np.ascontiguousarray(HgT8[lo:hi, :]),
                "hrow": hrow_t,
                "dvT": _tile128(np.ascontiguousarray(DvT[:, lo:hi])),
                "deT": _tile128(np.ascontiguousarray(DeT[:, lo:hi])),
                "dege": np.ascontiguousarray(
                    dege_full[lo:hi].reshape(1, HALF)),
                "whi": whi, "th2": th2, "brow": brow,
                "mask": np.ascontiguousarray(
                    em[g, lo:hi].astype(np.float32).reshape(1, HALF)),
                "eps": eps,
                "bng": bng, "bnb": bnb, "bnm": bnm, "bnv": bnv,
            })
    return in_maps


def kernel(**inputs):
    from concourse.bass_utils import run_bass_kernel_spmd

    nc = _get_nc()
    in_maps = _shard(inputs)
    res = run_bass_kernel_spmd(nc, in_maps, list(range(NCORES)))
    out = np.empty((B, N, D), dtype=np.float32)
    for g in range(B):
        out[g, :HALF, :] = res.results[2 * g]["y"].T
        out[g, HALF:, :] = res.results[2 * g + 1]["y"].T
    return out


# revision 35
# speedup vs baseline: 2.7631x; 1.0627x over previous
"""HGNN layer (hypergraph message passing) Trainium2 kernel, 8 NeuronCores.

Sharding: one graph per PAIR of cores; within a pair each core owns half the
hyperedge (Ec) / node (Nc) range. Host pre-casts the big matrices: the 0/1
incident matrix H ships as fp8e4 (exact) in the three layouts the PE needs;
Dv/De ship as fp8e3 scaled by 64 (descale folded into later evacuations); x
ships bf16 in block-transposed stationary layout. Streams use host-tiled
[128, k*HALF] layouts so slab DMAs move 1 MB at a time on the ACT HWDGE
queue. Dataflow computes hxx = H^T x first, then hx = hxx W + b (x) dege
(bias as a rank-1 PE accumulate against a host-computed edge-degree row);
attention scores use a hi/lo bf16 split of hxx and theta. Softmax is
unnormalized; z rides the first AllReduce as a bf16 hi/lo pair; 1/z and the
Dv descale fold into the h1c evacuation. htr stays SBUF-resident for its two
uses. Each of the 3 pair-AllReduces is split into two half-width bf16
collectives whose payloads are pre-transposed into the consumer's layout, so
the second half overlaps the first half's consumers and there is zero
post-AR rearrangement."""

import numpy as np

B, N, E, D = 4, 4096, 4096, 128
HALF = N // 2
NCORES = 8
PAIRS = [[0, 1], [2, 3], [4, 5], [6, 7]]
BN_EPS = 1e-5
F = 512                 # moving free-dim per matmul
NT = N // 128           # 32 k-tiles over a full 4096 dim
HT = HALF // 128        # 16 k-tiles over a half
SLAB = 4                # k-tiles per stream DMA (1 MB slabs)
DSCALE = 64.0           # host-side scale on Dv/De before fp8e3 cast
ZPAD = 16               # extra bf16 cols on the first AR chunk for z hi/lo

_CACHE = {}


def _build():
    import concourse.bacc as bacc
    import concourse.mybir as mybir
    import concourse.tile as tile
    from concourse.masks import make_identity
    from contextlib import ExitStack

    fp32 = mybir.dt.float32
    bf16 = mybir.dt.bfloat16
    fp8 = mybir.dt.float8e4
    fp8d = mybir.dt.float8e3
    Act = mybir.ActivationFunctionType
    Alu = mybir.AluOpType

    nc = bacc.Bacc("TRN2", target_bir_lowering=False, debug=False,
                   num_devices=NCORES)

    xv_d = nc.dram_tensor("xv", [128, N], bf16, kind="ExternalInput")
    hcol_d = nc.dram_tensor("hcol", [128, NT * HALF], fp8, kind="ExternalInput")
    # htr in quarter-major tiled layout: [p, q*(HT*Q) + t*Q + e], quarter q
    # covers output cols q*1024:(q+1)*1024
    htr_d = nc.dram_tensor("htr", [128, 2 * HT * HALF], fp8,
                           kind="ExternalInput")
    hrow_d = nc.dram_tensor("hrow", [128, 2 * HT * HALF], fp8,
                            kind="ExternalInput")
    dvT_d = nc.dram_tensor("dvT", [128, NT * HALF], fp8d, kind="ExternalInput")
    deT_d = nc.dram_tensor("deT", [128, NT * HALF], fp8d, kind="ExternalInput")
    dege_d = nc.dram_tensor("dege", [1, HALF], fp32, kind="ExternalInput")
    whi_d = nc.dram_tensor("whi", [D, D], bf16, kind="ExternalInput")
    th2_d = nc.dram_tensor("th2", [D, 1], fp32, kind="ExternalInput")
    brow_d = nc.dram_tensor("brow", [1, D], fp32, kind="ExternalInput")
    mask_d = nc.dram_tensor("mask", [1, HALF], fp32, kind="ExternalInput")
    eps_d = nc.dram_tensor("eps", [D, 1], fp32, kind="ExternalInput")
    bng_d = nc.dram_tensor("bng", [D, 1], fp32, kind="ExternalInput")
    bnb_d = nc.dram_tensor("bnb", [D, 1], fp32, kind="ExternalInput")
    bnm_d = nc.dram_tensor("bnm", [D, 1], fp32, kind="ExternalInput")
    bnv_d = nc.dram_tensor("bnv", [D, 1], fp32, kind="ExternalInput")
    y_d = nc.dram_tensor("y", [D, HALF], fp32, kind="ExternalOutput")

    with tile.TileContext(nc) as tc, ExitStack() as ctx:
        const = ctx.enter_context(tc.tile_pool(name="const", bufs=1))
        stream = ctx.enter_context(tc.tile_pool(name="stream", bufs=4))
        qstream_pool = ctx.enter_context(tc.tile_pool(name="qstream", bufs=6))
        med = ctx.enter_context(tc.tile_pool(name="med", bufs=1))
        small = ctx.enter_context(tc.tile_pool(name="small", bufs=1))
        ps = ctx.enter_context(tc.tile_pool(name="ps", bufs=8, space="PSUM"))
        dram = ctx.enter_context(tc.tile_pool(name="dram", bufs=1, space="DRAM"))

        ident = const.tile([128, 128], fp32)
        make_identity(nc, ident)
        one11 = const.tile([1, 1], fp32)
        nc.vector.memset(one11[:], 1.0)
        ones_row = const.tile([1, 128], fp32)
        nc.vector.memset(ones_row[:], 1.0)
        c64 = const.tile([128, 1], fp32)
        nc.vector.memset(c64[:], 1.0 / DSCALE)

        xv = const.tile([128, N], bf16)
        nc.sync.dma_start(out=xv[:], in_=xv_d.ap())

        def load_param(dt_):
            t = const.tile([D, 1], fp32, tag=dt_.name + "_p")
            nc.sync.dma_start(out=t[:], in_=dt_.ap())
            return t

        whi_t = const.tile([D, D], bf16)
        nc.sync.dma_start(out=whi_t[:], in_=whi_d.ap())
        thf_t = const.tile([D, 1], fp32)
        nc.sync.dma_start(out=thf_t[:], in_=th2_d.ap())
        brow_t = const.tile([1, D], fp32)
        nc.sync.dma_start(out=brow_t[:], in_=brow_d.ap())
        dege_t = const.tile([1, HALF], fp32)
        nc.sync.dma_start(out=dege_t[:], in_=dege_d.ap())
        eps_t = load_param(eps_d)
        bng_t = load_param(bng_d)
        bnb_t = load_param(bnb_d)
        bnm_t = load_param(bnm_d)
        bnv_t = load_param(bnv_d)
        mask_t = const.tile([1, HALF], fp32)
        nc.sync.dma_start(out=mask_t[:], in_=mask_d.ap())

        # dummy collectives keep ncfw warm between real collective clusters
        # (a cold cc stream adds ~10us to the next collective)
        warm_n = [0]

        def warm_cc():
            wi = dram.tile([1, 16], bf16, tag=f"warmi{warm_n[0]}",
                           name=f"warmi{warm_n[0]}")
            wo = dram.tile([2, 16], bf16, tag=f"warmo{warm_n[0]}",
                           name=f"warmo{warm_n[0]}")
            warm_n[0] += 1
            nc.gpsimd.collective_compute(
                "AllGather", Alu.bypass, replica_groups=PAIRS,
                ins=[wi.opt()], outs=[wo.opt()])

        warm_cc()

        # htr quarter-slab stream: quarter q (1024 output cols), 4 t-tiles
        # per 1 MB slab, on the ACT queue
        def make_qstate():
            return {}

        def qtile(state, t, q, name):
            Q4 = HALF // 2
            key = (q, t // SLAB)
            if key not in state:
                sb = qstream_pool.tile([128, SLAB * Q4], fp8, tag="qslab",
                                       name=name)
                base = q * (HT * Q4) + (t // SLAB) * SLAB * Q4
                nc.scalar.dma_start(
                    out=sb[:], in_=htr_d.ap()[:, base:base + SLAB * Q4])
                state[key] = sb
            return state[key][:, (t % SLAB) * Q4:(t % SLAB + 1) * Q4]

        def slab_stream(dram_t, dt, n_tiles, name):
            """Yield (k_tile_index, moving_tile_fn) streaming 1MB slabs."""
            for s in range(n_tiles // SLAB):
                sb = stream.tile([128, SLAB * HALF], dt, tag="slab",
                                 name=name)
                nc.scalar.dma_start(
                    out=sb[:],
                    in_=dram_t.ap()[:, s * SLAB * HALF:(s + 1) * SLAB * HALF])
                for jj in range(SLAB):
                    j = s * SLAB + jj
                    yield j, sb[:, jj * HALF:(jj + 1) * HALF]

        # ---- S2: hxxT [D, HALF] = (H[:,Ec]^T x)^T ------------------------
        hxx_ps = [ps.tile([128, F], fp32, tag="ps", name=f"hxx{i}")
                  for i in range(HALF // F)]
        for j, hj in slab_stream(hcol_d, fp8, NT, "hj"):
            for blk in range(HALF // F):
                nc.tensor.matmul(hxx_ps[blk][:],
                                 xv[:, j * D:(j + 1) * D],
                                 hj[:, blk * F:(blk + 1) * F],
                                 start=(j == 0), stop=(j == NT - 1))
        hxxT = med.tile([D, HALF], fp32, tag="hxxT")
        hxx_hi = med.tile([D, HALF], bf16, tag="hxx_hi")
        for blk in range(HALF // F):
            sl = slice(blk * F, (blk + 1) * F)
            nc.vector.tensor_copy(hxxT[:, sl], hxx_ps[blk][:])
            nc.vector.tensor_copy(hxx_hi[:, sl], hxx_ps[blk][:])

        # ---- S3: hxT = W^T hxx + b (x) dege ; st = th^T hxx (fp32) -------
        hxT = med.tile([D, HALF], fp32, tag="hxT")
        st_sb = small.tile([1, HALF], fp32, tag="st_sb")
        for blk in range(HALF // F):
            sl = slice(blk * F, (blk + 1) * F)
            hx2 = ps.tile([128, F], fp32, tag="ps", name=f"hx2_{blk}")
            nc.tensor.matmul(hx2[:], whi_t[:], hxx_hi[:, sl],
                             start=True, stop=False)
            nc.tensor.matmul(hx2[:], brow_t[:], dege_t[:, sl],
                             start=False, stop=True)
            nc.vector.tensor_copy(hxT[:, sl], hx2[:])
            sp = ps.tile([1, F], fp32, tag="ps", name=f"sp{blk}")
            nc.tensor.matmul(sp[:], thf_t[:], hxxT[:, sl],
                             start=True, stop=True)
            nc.vector.tensor_copy(st_sb[:, sl], sp[:])

        # ---- S4: softmax pieces (in-place on st_sb) ----------------------
        attn_u = st_sb
        nc.scalar.activation(attn_u[:], st_sb[:], Act.Exp)
        nc.vector.tensor_mul(attn_u[:], attn_u[:], mask_t[:])
        z_t = small.tile([1, 1], fp32, tag="z_t")
        nc.vector.reduce_sum(z_t[:], attn_u[:], axis=mybir.AxisListType.X)
        # z hi/lo bf16 pieces
        zhi = small.tile([1, 1], bf16, tag="zhi")
        zlo = small.tile([1, 1], bf16, tag="zlo")
        zf = small.tile([1, 1], fp32, tag="zf")
        nc.vector.tensor_copy(zhi[:], z_t[:])
        nc.vector.tensor_copy(zf[:], zhi[:])
        nc.vector.tensor_tensor(zf[:], z_t[:], zf[:], op=Alu.subtract)
        nc.vector.tensor_copy(zlo[:], zf[:])
        attnv = med.tile([128, HT], fp32, tag="attnv")
        for t in range(HT):
            pt = ps.tile([128, 1], fp32, tag="ps")
            nc.tensor.matmul(pt[:], attn_u[:, t * 128:(t + 1) * 128], one11[:],
                             start=True, stop=True)
            nc.vector.tensor_copy(attnv[:, t:t + 1], pt[:])
        ehxT = med.tile([D, HALF], fp32, tag="ehxT")
        nc.vector.tensor_scalar_mul(ehxT[:], hxT[:], eps_t[:])
        warm_cc()

        # ---- S5: h1av [128, HT*D] bf16 = attn * hx (e-part tiles) --------
        h1av = med.tile([128, HALF], bf16, tag="h1av")
        for t in range(HT):
            pt = ps.tile([128, 128], fp32, tag="ps")
            nc.tensor.transpose(pt[:], hxT[:, t * 128:(t + 1) * 128], ident[:])
            nc.vector.tensor_scalar_mul(h1av[:, t * 128:(t + 1) * 128], pt[:],
                                        attnv[:, t:t + 1])

        def chunked_bmm_ag_v(stationary, moving_of, tagbase, with_z=False):
            """Two half-width partial bmms; payload pre-transposed to the
            consumer's v-layout, cast bf16, AllGathered per chunk with a
            local DVE add of the two rank blocks (cheaper than ncfw
            AllReduce). Returns (resA, resB) bf16 [128, HALF(+ZPAD)]."""
            outs = []
            for chunk in range(2):
                w = HALF + ZPAD if (with_z and chunk == 0) else HALF
                pss = [ps.tile([128, F], fp32, tag="ps",
                               name=f"{tagbase}_{chunk}_{i}")
                       for i in range(HALF // F)]
                for t in range(HT):
                    for blk in range(HALF // F):
                        nc.tensor.matmul(
                            pss[blk][:],
                            stationary[:, t * 128:(t + 1) * 128],
                            moving_of(t, chunk * HALF + blk * F, F),
                            start=(t == 0), stop=(t == HT - 1))
                ccT = med.tile([D, HALF], fp32, tag="ccsbT")
                for blk in range(HALF // F):
                    sl = slice(blk * F, (blk + 1) * F)
                    nc.vector.tensor_copy(ccT[:, sl], pss[blk][:])
                ccv = med.tile([128, HALF + ZPAD], bf16,
                               tag=f"ccv{chunk}")
                for t in range(HT):
                    pt = ps.tile([128, 128], fp32, tag="ps")
                    nc.tensor.transpose(pt[:], ccT[:, t * 128:(t + 1) * 128],
                                        ident[:])
                    nc.vector.tensor_copy(ccv[:, t * 128:(t + 1) * 128], pt[:])
                if with_z and chunk == 0:
                    nc.vector.memset(ccv[:, HALF:], 0.0)
                    nc.vector.tensor_copy(ccv[0:1, HALF:HALF + 1], zhi[:])
                    nc.vector.tensor_copy(ccv[0:1, HALF + 1:HALF + 2], zlo[:])
                cc_in = dram.tile([128, w], bf16, tag=f"{tagbase}i{chunk}")
                cc_out = dram.tile([256, w], bf16, tag=f"{tagbase}o{chunk}")
                nc.sync.dma_start(out=cc_in[:], in_=ccv[:, 0:w])
                nc.gpsimd.collective_compute(
                    "AllGather", Alu.bypass, replica_groups=PAIRS,
                    ins=[cc_in.opt()], outs=[cc_out.opt()])
                res = med.tile([128, HALF + ZPAD], bf16,
                               tag=f"resv{chunk}")
                agt = med.tile([128, HALF + ZPAD], bf16, tag="agtmp")
                nc.sync.dma_start(out=res[:, 0:w], in_=cc_out[0:128, :])
                nc.sync.dma_start(out=agt[:, 0:w], in_=cc_out[128:256, :])
                nc.vector.tensor_tensor(res[:, 0:w], res[:, 0:w],
                                        agt[:, 0:w], op=Alu.add)
                outs.append(res)
            return outs

        # ---- S6: h1b = H h1a (partial over Ec), v-layout chunked AG ------
        htr_s6 = make_qstate()

        def htr_moving(t, lo, w):
            q, off = divmod(lo, HALF // 2)
            return qtile(htr_s6, t, q, "htq6")[:, off:off + w]

        h1bA, h1bB = chunked_bmm_ag_v(h1av, htr_moving, "cc1", with_z=True)

        # rz = 1/(z), folded with 1/DSCALE, broadcast to [128, 1]
        rz = small.tile([1, 1], fp32, tag="rz")
        zs = small.tile([1, 1], fp32, tag="zs")
        nc.vector.tensor_copy(rz[:], h1bA[0:1, HALF:HALF + 1])
        nc.vector.tensor_copy(zs[:], h1bA[0:1, HALF + 1:HALF + 2])
        nc.vector.tensor_tensor(rz[:], rz[:], zs[:], op=Alu.add)
        nc.vector.reciprocal(rz[:], rz[:])
        rz_ps = ps.tile([128, 1], fp32, tag="ps")
        nc.tensor.matmul(rz_ps[:], ones_row[:], rz[:], start=True, stop=True)
        rz_bc = small.tile([128, 1], fp32, tag="rz_bc")
        nc.vector.tensor_copy(rz_bc[:], rz_ps[:])
        nc.vector.tensor_mul(rz_bc[:], rz_bc[:], c64[:])

        def vtile(resA, resB, j):
            src = resA if j < HT else resB
            jj = j % HT
            return src[:, jj * 128:(jj + 1) * 128]

        # ---- S7: h1cT = (Dv[Nc,:] h1b)^T * rz/DSCALE ---------------------
        h1c_ps = [ps.tile([128, F], fp32, tag="ps", name=f"h1c{i}")
                  for i in range(HALF // F)]
        for j, dj in slab_stream(dvT_d, fp8d, NT, "dj"):
            for blk in range(HALF // F):
                nc.tensor.matmul(h1c_ps[blk][:], vtile(h1bA, h1bB, j),
                                 dj[:, blk * F:(blk + 1) * F],
                                 start=(j == 0), stop=(j == NT - 1))
        h1cT = med.tile([D, HALF], fp32, tag="hxxT")
        for blk in range(HALF // F):
            sl = slice(blk * F, (blk + 1) * F)
            nc.vector.tensor_scalar_mul(h1cT[:, sl], h1c_ps[blk][:], rz_bc[:])
        h1cv = med.tile([128, HALF], bf16, tag="h1cv")
        for t in range(HT):
            pt = ps.tile([128, 128], fp32, tag="ps")
            nc.tensor.transpose(pt[:], h1cT[:, t * 128:(t + 1) * 128], ident[:])
            nc.vector.tensor_copy(h1cv[:, t * 128:(t + 1) * 128], pt[:])
        warm_cc()

        # ---- S8: h1d = H[Nc,:]^T h1c (partial over Nc), chunked AR -------
        hrow_state = {}

        def hrow_moving(t, lo, w):
            chunk = lo // HALF
            slab_i = t // SLAB
            key = (chunk, slab_i)
            if key not in hrow_state:
                sb = stream.tile([128, SLAB * HALF], fp8, tag="slab",
                                 name=f"rj{chunk}")
                base = chunk * HT * HALF + slab_i * SLAB * HALF
                nc.scalar.dma_start(
                    out=sb[:], in_=hrow_d.ap()[:, base:base + SLAB * HALF])
                hrow_state[key] = sb
            return hrow_state[key][:, (t % SLAB) * HALF + (lo % HALF):
                                   (t % SLAB) * HALF + (lo % HALF) + w]

        h1dA, h1dB = chunked_bmm_ag_v(h1cv, hrow_moving, "cc2")

        # ---- S9+S10: hT = (De[Ec,:] h1d)^T / DSCALE + eps*hx -------------
        h1e_ps = [ps.tile([128, F], fp32, tag="ps", name=f"h1e{i}")
                  for i in range(HALF // F)]
        for j, ej in slab_stream(deT_d, fp8d, NT, "ej"):
            for blk in range(HALF // F):
                nc.tensor.matmul(h1e_ps[blk][:], vtile(h1dA, h1dB, j),
                                 ej[:, blk * F:(blk + 1) * F],
                                 start=(j == 0), stop=(j == NT - 1))
        hT = med.tile([D, HALF], fp32, tag="hxT")
        for blk in range(HALF // F):
            sl = slice(blk * F, (blk + 1) * F)
            nc.vector.scalar_tensor_tensor(hT[:, sl], h1e_ps[blk][:],
                                           1.0 / DSCALE, ehxT[:, sl],
                                           op0=Alu.mult, op1=Alu.add)
        hv = med.tile([128, HALF], bf16, tag="h1av")
        for t in range(HT):
            pt = ps.tile([128, 128], fp32, tag="ps")
            nc.tensor.transpose(pt[:], hT[:, t * 128:(t + 1) * 128], ident[:])
            nc.vector.tensor_copy(hv[:, t * 128:(t + 1) * 128], pt[:])
        warm_cc()

        # ---- S11: out = H h (partial over Ec), bf16 T-layout chunked AR --
        s_bn = small.tile([D, 1], fp32, tag="s_bn")
        nc.vector.tensor_scalar_add(s_bn[:], bnv_t[:], BN_EPS)
        nc.scalar.activation(s_bn[:], s_bn[:], Act.Sqrt)
        nc.vector.reciprocal(s_bn[:], s_bn[:])
        nc.vector.tensor_mul(s_bn[:], s_bn[:], bng_t[:])
        t_bn = small.tile([D, 1], fp32, tag="t_bn")
        nc.vector.tensor_mul(t_bn[:], bnm_t[:], s_bn[:])
        nc.vector.tensor_tensor(t_bn[:], bnb_t[:], t_bn[:], op=Alu.subtract)

        # two half-width ReduceScatters (core even gets summed cols 0:HALF,
        # odd the rest). S11's matmuls are grouped into quarter-pairs so
        # RS h=0 fires after only half the matmuls; its epilogue overlaps
        # the rest of S11 and RS h=1.
        Q = HALF // 2
        cc3_in = [dram.tile([256, Q], bf16, tag=f"cc3i{h}", name=f"cc3i{h}")
                  for h in range(2)]
        cc3_out = [dram.tile([128, Q], bf16, tag=f"cc3o{h}", name=f"cc3o{h}")
                   for h in range(2)]
        htr_s11 = make_qstate()
        for h in range(2):
            # blocks covering cols [h*Q:(h+1)*Q] of both n-chunks
            pss = [ps.tile([128, F], fp32, tag="ps", name=f"out_{h}_{i}")
                   for i in range(4)]
            for t in range(HT):
                for i in range(4):
                    chunk, blk = divmod(i, 2)
                    q = 2 * chunk + h
                    nc.tensor.matmul(
                        pss[i][:],
                        hv[:, t * 128:(t + 1) * 128],
                        qtile(htr_s11, t, q, "htq11")[:, blk * F:
                                                      (blk + 1) * F],
                        start=(t == 0), stop=(t == HT - 1))
            ccv = med.tile([128, HALF + ZPAD], bf16, tag=f"ccv{h}")
            for i in range(4):
                chunk, blk = divmod(i, 2)
                sl = slice(chunk * Q + blk * F, chunk * Q + (blk + 1) * F)
                nc.vector.tensor_copy(ccv[:, sl], pss[i][:])
            for chunk in range(2):
                nc.sync.dma_start(
                    out=cc3_in[h][chunk * 128:(chunk + 1) * 128, :],
                    in_=ccv[:, chunk * Q:(chunk + 1) * Q])
            nc.gpsimd.collective_compute(
                "ReduceScatter", Alu.add, replica_groups=PAIRS,
                ins=[cc3_in[h].opt()], outs=[cc3_out[h].opt()])
        for h in range(2):
            res3 = med.tile([128, HALF + ZPAD], bf16, tag=f"resv{h}")
            nc.sync.dma_start(out=res3[:, 0:Q], in_=cc3_out[h][:])
            of = med.tile([D, Q], fp32, tag=f"of{h}")
            nc.scalar.activation(of[:], res3[:, 0:Q], Act.Lrelu, alpha=0.01)
            nc.vector.tensor_scalar(of[:], of[:], s_bn[:], t_bn[:],
                                    op0=Alu.mult, op1=Alu.add)
            nc.sync.dma_start(out=y_d.ap()[:, h * Q:(h + 1) * Q], in_=of[:])

    nc.finalize()
    return nc


def _get_nc():
    if "nc" not in _CACHE:
        _CACHE["nc"] = _build()
    return _CACHE["nc"]


def _tile128(a):
    """[K*128, W] -> [128, K*W] block-transposed stream layout."""
    K = a.shape[0] // 128
    return np.ascontiguousarray(
        a.reshape(K, 128, a.shape[1]).transpose(1, 0, 2).reshape(
            128, K * a.shape[1]))


def _shard(inputs):
    import ml_dtypes
    bf16 = ml_dtypes.bfloat16
    fp8 = ml_dtypes.float8_e4m3
    fp8d = ml_dtypes.float8_e3m4

    H = np.asarray(inputs["incident_mat"], dtype=np.float32)
    Dv = np.asarray(inputs["degree_v"], dtype=np.float32)
    De = np.asarray(inputs["degree_e"], dtype=np.float32)
    x = np.asarray(inputs["x"], dtype=np.float32)
    em = np.asarray(inputs["e_masks"])
    w = np.asarray(inputs["mlp_W"], dtype=np.float32)
    b = np.asarray(inputs["mlp_b"], dtype=np.float32)
    th = np.asarray(inputs["theta_att"], dtype=np.float32).reshape(D)
    eps = np.full((D, 1), float(np.asarray(inputs["eps"]).reshape(-1)[0]),
                  dtype=np.float32)

    def col(v):
        return np.ascontiguousarray(
            np.asarray(v, dtype=np.float32).reshape(D, 1))

    bng, bnb = col(inputs["bn_gamma"]), col(inputs["bn_beta"])
    bnm, bnv = col(inputs["bn_mean"]), col(inputs["bn_var"])

    whi = w.astype(bf16)
    th2 = np.ascontiguousarray(th.reshape(D, 1))
    brow = np.ascontiguousarray(b.reshape(1, D))

    in_maps = []
    for g in range(B):
        Hg8 = H[g].astype(fp8)
        HgT8 = np.ascontiguousarray(H[g].T).astype(fp8)
        dege_full = H[g].sum(axis=0, dtype=np.float32)
        xv = _tile128(x[g]).astype(bf16)
        DvT = (Dv[g].T * DSCALE).astype(fp8d)
        DeT = (De[g].T * DSCALE).astype(fp8d)
        for c in range(2):
            lo, hi = c * HALF, (c + 1) * HALF
            # hrow chunk-major: [128, 2*HT*HALF], chunk c2 at t*HALF+e
            hrow_g = Hg8[lo:hi, :].reshape(HT, 128, 2, HALF)
            hrow_t = np.ascontiguousarray(
                hrow_g.transpose(1, 2, 0, 3).reshape(128, 2 * HT * HALF))
            htr_q = np.ascontiguousarray(
                HgT8[lo:hi, :].reshape(HT, 128, 4, HALF // 2)
                .transpose(1, 2, 0, 3).reshape(128, 2 * HT * HALF))
            in_maps.append({
                "xv": xv,
                "hcol": _tile128(np.ascontiguousarray(Hg8[:, lo:hi])),
                "htr": htr_q,
                "hrow": hrow_t,
                "dvT": _tile128(np.ascontiguousarray(DvT[:, lo:hi])),
                "deT": _tile128(np.ascontiguousarray(DeT[:, lo:hi])),
                "dege": np.ascontiguousarray(
                    dege_full[lo:hi].reshape(1, HALF)),
                "whi": whi, "th2": th2, "brow": brow,
                "mask": np.ascontiguousarray(
                    em[g, lo:hi].astype(np.float32).reshape(1, HALF)),
                "eps": eps,
                "bng": bng, "bnb": bnb, "bnm": bnm, "bnv": bnv,
            })
    return in_maps


def kernel(**inputs):
    from concourse.bass_utils import run_bass_kernel_spmd

    nc = _get_nc()
    in_maps = _shard(inputs)
    res = run_bass_kernel_spmd(nc, in_maps, list(range(NCORES)))
    out = np.empty((B, N, D), dtype=np.float32)
    for g in range(B):
        out[g, :HALF, :] = res.results[2 * g]["y"].T
        out[g, HALF:, :] = res.results[2 * g + 1]["y"].T
    return out
